# revision 20
# baseline (speedup 1.0000x reference)
"""Trainium2 Bass kernel for nn_BioClassifier: whitening + sequential Oja scan + readout.

v2: restructured for critical-path latency. Same block-parallel-scan math as v1
(chunk the 2048-sample Oja scan into 16 blocks of K=128; per block a fixed-point
"ring" on K x K matrices closes the sequential recurrence exactly):
    Y = Y0 A,  U = (X - T0 A) B,   A = (I - lr*SU(C))^-1, B = (I + lr*SU(G))^-1
    C = U^T X, G = Y^T Y,  T0^T X = Syy
Key v2 changes vs v1:
  * lr folded into sxx_lr/syy_lr (bf16), iteration reordered so each ring cycle
    is 10 serial engine-hops (B-chain: z2->z2s->g->gm->b1->B; A-chain: ct->nt->
    a1->A) with r1/s prep hidden under the B-chain; iter-0 A-update is 3 vec ops
    (A1 = I + SU(s0), s0 = sxx_lr - syy_lr), no matmuls.
  * Y0 correction form: P_{n} = W^{(n-1)} X_n accumulates in an OPEN PSUM group
    during ring_{n-1} (off critical path); epilogue closes it with the rank-K
    correction  Y0_n = P_n + lr * Y * (U^T X_n), so the master-W update and the
    14 Y0 matmuls leave the serial path entirely.
  * single fp32 master W [H,D]; Wb = cast(W) on scalar; WTb = PE-transpose of Wb
    (bf16 transpose == transpose of bf16 cast, exact) - drops the WT fp32 master
    and its vector-engine update entirely.
  * XTall/Sxxall stored bf16 (Sxx pre-scaled by lr at whiten time).
  * engine rebalance: ring bounces on DVE, z2s/copies on scalar, xc-sub and
    the iter-0 mask-mul on gpsimd (which cannot touch PSUM); whitening +
    masters + P-opens emitted in priority bands (and the master-update tail
    emitted mid-ring of the NEXT block) so the in-order engine streams place
    them behind each block's critical chain.
  * RING_ITERS=4 (validated offline: rel err 1.49e-2 vs the 2e-2 gate;
    RING_ITERS=5 gives 8.7e-3 at ~+60us).
All 8 cores run the identical program (the scan is inherently sequential;
core 0's output is returned).
"""

import os
import sys
from contextlib import ExitStack

sys.path.insert(0, "/opt/trn_rl_repo")

import numpy as np
import ml_dtypes

import concourse.bass as bass
import concourse.mybir as mybir
from concourse.tile import TileContext
from concourse.masks import make_identity
from concourse.bass_utils import run_bass_kernel_spmd
from concourse.vector_clock import ScopedClock

LR = 1e-3
B, D, H, O = 2048, 784, 256, 10
K = 128
NBLK = B // K
DP, DC = 112, 7          # D = 784 = 7 * 112
HP, HC = 128, 2          # H = 256 = 2 * 128
DS = D // 2              # 392: matmul free-dim split for D-wide outputs

RING_ITERS = int(os.environ.get("RING_ITERS", "4"))
LOOKAHEAD = 4
N_CORES = 8

f32 = mybir.dt.float32
bf16 = mybir.dt.bfloat16
AT = mybir.AluOpType


def _install_ntff_hook():
    """The agent image's `antenv` lacks `axon_hooks`, so trace=True degrades.
    Synthesize the module and register the ctypes NTFF hook from trn_boot."""
    import types
    import antenv

    if getattr(antenv, "axon_hooks", None) is not None:
        return
    mod = types.ModuleType("antenv.axon_hooks")
    _hook_box = [None]
    mod.set_axon_ntff_profile_hook = lambda h: _hook_box.__setitem__(0, h)
    mod.get_axon_ntff_profile_hook = lambda: _hook_box[0]
    sys.modules["antenv.axon_hooks"] = mod
    antenv.axon_hooks = mod
    try:
        sys.path.insert(0, "/root/.axon_site")
        from trn_agent_boot.trn_boot import _ntff_profile_via_ctypes

        hook = _ntff_profile_via_ctypes("/opt/axon/libaxon_pjrt.so")
        if hook is not None:
            mod.set_axon_ntff_profile_hook(hook)
    except Exception:
        pass


try:
    _install_ntff_hook()
except Exception:
    pass

_drain_patched = False


def _patch_drain():
    """This walrus build only supports one sync-wait per CTRL instruction;
    split the Tile kernel-tail drain into one drain per semaphore wait."""
    global _drain_patched
    if _drain_patched:
        return

    def patched(self, tick_clock, wait_clock):
        drain_inst = self.nc.sync.drain()
        wait_clock.add_sem_waits(
            drain_inst.ins, ScopedClock({None: tick_clock.global_clock})
        )
        mi = drain_inst.ins
        si = mi.sync_info
        if si is not None and len(si.on_wait) > 1:
            waits = list(si.on_wait)
            mi.sync_info = mybir.SyncInfo(
                on_wait=[waits[0]], on_update=list(si.on_update)
            )
            for w in waits[1:]:
                d2 = self.nc.sync.drain()
                d2.ins.sync_info = mybir.SyncInfo(on_wait=[w], on_update=[])
        self.nc.all_engine_barrier()
        assert self.sems is not None
        popped = self.nc._tile_sem_poison_stack.pop()
        assert popped is self._sem_poison
        self.nc.clear_and_free_semaphores(list(self.sems.allocated().values()))
        self.nc.all_engine_barrier()

    TileContext._drain_and_barrier = patched
    _drain_patched = True


def _split_multiwait(nc, limit=1):
    """This walrus build supports only `limit` sync-waits per instruction.
    Hoist extra waits onto NoOps inserted just before, in the same engine
    stream (engines are in-order, so earlier waits are strictly safe)."""
    n_split = 0
    for f in nc.m.functions:
        for bb in f.blocks:
            insts = list(bb.instructions)
            if not any(
                i.sync_info is not None and len(i.sync_info.on_wait) > limit
                for i in insts
            ):
                continue
            new = []
            for inst in insts:
                si = inst.sync_info
                if si is not None and len(si.on_wait) > limit:
                    waits = list(si.on_wait)
                    for j, w in enumerate(waits[: len(waits) - limit]):
                        nop = mybir.InstNoOp(
                            name=f"{inst.name}-hw{j}", engine=inst.engine,
                            ins=[], outs=[],
                        )
                        nop.sync_info = mybir.SyncInfo(on_wait=[w], on_update=[])
                        new.append(nop)
                        n_split += 1
                    inst.sync_info = mybir.SyncInfo(
                        on_wait=waits[len(waits) - limit:],
                        on_update=list(si.on_update),
                    )
                new.append(inst)
            bb.instructions = new
    return n_split


def build_nc(ring_iters=RING_ITERS):
    _patch_drain()
    nc = bass.Bass()
    x_d = nc.dram_tensor("x", [B, D], f32, kind="ExternalInput")
    mu_d = nc.dram_tensor("mu_b", [128, D], f32, kind="ExternalInput")
    pt_d = nc.dram_tensor("pt", [DP, DC, D], bf16, kind="ExternalInput")
    w_d = nc.dram_tensor("w", [HP, HC * D], f32, kind="ExternalInput")
    wb_d = nc.dram_tensor("w_bf", [HP, HC * D], bf16, kind="ExternalInput")
    wtb_d = nc.dram_tensor("wt_bf", [DP, DC * H], bf16, kind="ExternalInput")
    rt_d = nc.dram_tensor("rt", [HP, HC, O], bf16, kind="ExternalInput")
    bb_d = nc.dram_tensor("b_b", [128, O], f32, kind="ExternalInput")
    out_d = nc.dram_tensor("out", [B, O], f32, kind="ExternalOutput")
    def pri_crit(b):
        # critical path of block b
        return 1000 + b * 1000

    def pri_fill(b):
        # fill work of block b: must rank BELOW crit of b+1 (it runs during
        # ring_{b+1}) but above crit of b+2
        return 1000 + (b + 1) * 1000 + 500

    def pri_whit(b):
        # whiten(b) must complete before epilogue of b-1: rank just below
        # crit(b-1), above fill bands of earlier blocks
        return 1000 + (b - 1) * 1000 + 400

    def pri_out(b):
        return 20_000_000 + b * 1000

    with TileContext(nc) as tc, ExitStack() as ctx:
        persist = ctx.enter_context(tc.tile_pool(name="persist", bufs=1))
        xpool = ctx.enter_context(tc.tile_pool(name="xpool", bufs=5))
        small = ctx.enter_context(tc.tile_pool(name="small", bufs=2))
        psA = ctx.enter_context(tc.tile_pool(name="psA", bufs=3, space="PSUM"))
        psB = ctx.enter_context(tc.tile_pool(name="psB", bufs=3, space="PSUM"))
        psT = ctx.enter_context(tc.tile_pool(name="psT", bufs=2, space="PSUM"))

        ident = persist.tile([128, 128], f32, tag="ident")
        make_identity(nc, ident)
        identb = persist.tile([128, 128], bf16, tag="identb")
        nc.vector.tensor_copy(identb, ident)
        # 0/1 masks (lr is folded into sxx_lr / syy_lr)
        maskSL = persist.tile([K, K], f32, tag="maskSL")
        nc.gpsimd.memset(maskSL, 1.0)
        nc.gpsimd.affine_select(
            out=maskSL, in_=maskSL, compare_op=AT.is_gt, fill=0.0,
            base=0, pattern=[[-1, K]], channel_multiplier=1,
        )
        maskSU = persist.tile([K, K], f32, tag="maskSU")
        nc.gpsimd.memset(maskSU, 1.0)
        nc.vector.tensor_sub(maskSU, maskSU, ident)
        nc.vector.tensor_sub(maskSU, maskSU, maskSL)

        mu_t = persist.tile([128, D], f32, tag="mu")
        nc.sync.dma_start(out=mu_t, in_=mu_d[:, :])
        pt_t = persist.tile([DP, DC, D], bf16, tag="pt")
        nc.sync.dma_start(out=pt_t, in_=pt_d[:, :, :])
        WTb = persist.tile([DP, DC * H], bf16, tag="WTb")
        nc.sync.dma_start(out=WTb, in_=wtb_d[:, :])
        Wb = persist.tile([HP, HC * D], bf16, tag="Wb")
        nc.sync.dma_start(out=Wb, in_=wb_d[:, :])
        W = persist.tile([HP, HC * D], f32, tag="W")
        nc.sync.dma_start(out=W, in_=w_d[:, :])
        RT = persist.tile([HP, HC, O], bf16, tag="RT")
        nc.sync.dma_start(out=RT, in_=rt_d[:, :, :])
        bb = persist.tile([128, O], f32, tag="bb")
        nc.sync.dma_start(out=bb, in_=bb_d[:, :])

        Xall = persist.tile([DP, NBLK, DC, K], bf16, tag="Xall")
        XTall = persist.tile([K, NBLK, D], bf16, tag="XTall")
        Sxxall = persist.tile([K, NBLK, K], bf16, tag="Sxxall")

        # ---------------- whitening ----------------
        def whiten(bi):
            xt = xpool.tile([128, D], f32, tag="xraw")
            nc.sync.dma_start(out=xt, in_=x_d[bi * K:(bi + 1) * K, :])
            xc = xpool.tile([128, D], f32, tag="xc")
            eng = nc.vector if bi == 0 else nc.gpsimd
            eng.tensor_sub(xc, xt, mu_t)
            xcb = xpool.tile([128, D], bf16, tag="xcb")
            nc.vector.tensor_copy(xcb, xc)
            xct = xpool.tile([DP, DC * K], bf16, tag="xct")
            for p in range(3):          # paired transposes -> one copy per pair
                tp = psT.tile([DP, 2 * K], bf16, tag="tt")
                nc.tensor.transpose(
                    tp[:, 0:K], xcb[:, (2 * p) * DP:(2 * p + 1) * DP], identb
                )
                nc.tensor.transpose(
                    tp[:, K:2 * K], xcb[:, (2 * p + 1) * DP:(2 * p + 2) * DP],
                    identb,
                )
                nc.scalar.copy(xct[:, (2 * p) * K:(2 * p + 2) * K], tp)
            tp = psT.tile([DP, K], bf16, tag="tt")
            nc.tensor.transpose(tp, xcb[:, 6 * DP:7 * DP], identb)
            nc.scalar.copy(xct[:, 6 * K:7 * K], tp)
            XTb = XTall[:, bi, :]
            for s in range(2):
                ps = psB.tile([K, DS], f32, tag="big")
                for ic in range(DC):
                    nc.tensor.matmul(
                        ps, xct[:, ic * K:(ic + 1) * K],
                        pt_t[:, ic, s * DS:(s + 1) * DS],
                        start=(ic == 0), stop=(ic == DC - 1),
                    )
                nc.vector.tensor_add(
                    XTb[:, s * DS:(s + 1) * DS], ps, xc[:, s * DS:(s + 1) * DS]
                )
            xa = Xall[:, bi, :, :]
            for p in range(3):
                tp = psT.tile([DP, 2 * K], bf16, tag="tt")
                nc.tensor.transpose(
                    tp[:, 0:K], XTb[:, (2 * p) * DP:(2 * p + 1) * DP], identb
                )
                nc.tensor.transpose(
                    tp[:, K:2 * K], XTb[:, (2 * p + 1) * DP:(2 * p + 2) * DP],
                    identb,
                )
                nc.scalar.copy(xa[:, 2 * p:2 * p + 2, :], tp)
            tp = psT.tile([DP, K], bf16, tag="tt")
            nc.tensor.transpose(tp, XTb[:, 6 * DP:7 * DP], identb)
            nc.scalar.copy(xa[:, 6, :], tp)
            ps = psA.tile([K, K], f32, tag="kk")
            for ic in range(DC):
                nc.tensor.matmul(
                    ps, xa[:, ic, :], xa[:, ic, :],
                    start=(ic == 0), stop=(ic == DC - 1),
                )
            nc.scalar.mul(Sxxall[:, bi, :], ps, LR)

        tc.cur_priority = 0
        whiten(0)
        tc.cur_priority = pri_whit(1)
        whiten(1)
        tc.cur_priority = pri_whit(2)
        whiten(2)
        tc.cur_priority = pri_whit(3)
        whiten(3)

        Psb = {}  # block -> SBUF f32 tile [HP, HC*K] holding W^(stale) X_block

        def open_P(nb):
            """Accumulate P_nb = W^(current) X_nb into an SBUF f32 tile."""
            pt_sb = small.tile([HP, HC * K], f32, tag="Psb")
            for hc in range(HC):
                ps = psB.tile([HP, K], f32, tag="big")
                for ic in range(DC):
                    nc.tensor.matmul(
                        ps,
                        WTb[:, ic * H + hc * HP:ic * H + (hc + 1) * HP],
                        Xall[:, nb, ic, :],
                        start=(ic == 0), stop=(ic == DC - 1),
                    )
                nc.scalar.copy(pt_sb[:, hc * K:(hc + 1) * K], ps)
            Psb[nb] = pt_sb

        # P_1 with the initial weights (correction applied in epilogue of block 0).
        # Priority BELOW crit(0): open_P(1) waits on whiten(1), and at higher
        # priority it would head-of-line-block block 0's PE stream.
        tc.cur_priority = 1450
        open_P(1)
        y0q_next = None
        pending_tail = [None]

        for b in range(NBLK):
            # ================= HEAD (critical) =================
            tc.cur_priority = pri_crit(b)
            if b == 0:
                y0q = small.tile([HP, HC * K], bf16, tag="y0")
                for hc in range(HC):
                    ps = psB.tile([HP, K], f32, tag="big")
                    for ic in range(DC):
                        nc.tensor.matmul(
                            ps,
                            WTb[:, ic * H + hc * HP:ic * H + (hc + 1) * HP],
                            Xall[:, 0, ic, :],
                            start=(ic == 0), stop=(ic == DC - 1),
                        )
                    nc.vector.tensor_copy(y0q[:, hc * K:(hc + 1) * K], ps)
            else:
                y0q = y0q_next
            ps_syy = psA.tile([K, K], f32, tag="kk")
            for hc in range(HC):
                nc.tensor.matmul(
                    ps_syy, y0q[:, hc * K:(hc + 1) * K],
                    y0q[:, hc * K:(hc + 1) * K],
                    start=(hc == 0), stop=(hc == HC - 1),
                )
            syy_lr = small.tile([K, K], bf16, tag="syl")
            nc.scalar.mul(syy_lr, ps_syy, LR)
            syy_ng = small.tile([K, K], bf16, tag="syn")
            nc.vector.tensor_scalar_mul(syy_ng, ps_syy, -LR)

            # ---- head fill (off critical path) ----
            tc.cur_priority = pri_fill(b)
            y0t = small.tile([K, H], bf16, tag="y0t")
            for hc in range(HC):
                tp = psT.tile([128, K], bf16, tag="tt")
                nc.tensor.transpose(tp, y0q[:, hc * K:(hc + 1) * K], identb)
                nc.scalar.copy(y0t[:, hc * HP:(hc + 1) * HP], tp)
            # ================= RING (critical) =================
            tc.cur_priority = pri_crit(b)
            sxx = Sxxall[:, b, :]
            s_sb = small.tile([K, K], bf16, tag="s")
            nc.vector.scalar_tensor_tensor(
                s_sb, ps_syy, -LR, sxx, op0=AT.mult, op1=AT.add
            )
            tA0 = small.tile([K, K], bf16, tag="ta")
            nc.gpsimd.tensor_mul(tA0, s_sb, maskSU)
            A = small.tile([K, K], bf16, tag="A")
            nc.vector.tensor_add(A, tA0, ident)
            Bm = small.tile([K, K], bf16, tag="B")
            Bprev = identb
            for m in range(1, ring_iters):
                if m == 2:
                    # emit the previous block's master-update chain here so the
                    # scheduler places it after this block's head/early ring in
                    # every engine stream; t0t must follow it (reads updated Wb)
                    if pending_tail[0] is not None:
                        pending_tail[0]()
                        pending_tail[0] = None
                    tc.cur_priority = pri_fill(b - 1) if b else pri_fill(0)
                    t0t = small.tile([K, D], bf16, tag="t0t")
                    for s in range(2):
                        ps = psB.tile([K, DS], f32, tag="big")
                        for hc in range(HC):
                            nc.tensor.matmul(
                                ps, y0q[:, hc * K:(hc + 1) * K],
                                Wb[:, hc * D + s * DS:hc * D + (s + 1) * DS],
                                start=(hc == 0), stop=(hc == HC - 1),
                            )
                        nc.vector.tensor_copy(t0t[:, s * DS:(s + 1) * DS], ps)
                    tc.cur_priority = pri_crit(b)
                z2 = psA.tile([K, K], f32, tag="kk")
                nc.tensor.matmul(z2, syy_ng, A, start=True, stop=True)
                z2s = small.tile([K, K], bf16, tag="z2")
                nc.scalar.copy(z2s, z2)
                r1 = psA.tile([K, K], f32, tag="kk")
                nc.tensor.matmul(r1, A, syy_lr, start=True, stop=True)
                s_sb = small.tile([K, K], bf16, tag="s")
                nc.vector.tensor_sub(s_sb, sxx, r1)
                g = psA.tile([K, K], f32, tag="kk")
                nc.tensor.matmul(g, A, z2s, start=True, stop=True)
                gm = small.tile([K, K], bf16, tag="gm")
                nc.vector.tensor_mul(gm, g, maskSL)
                b1 = psA.tile([K, K], f32, tag="kk")
                nc.tensor.matmul(b1, gm, Bprev, start=True, stop=True)
                nc.vector.tensor_add(Bm, b1, ident)
                Bprev = Bm
                ct = psA.tile([K, K], f32, tag="kk")
                nc.tensor.matmul(ct, s_sb, Bm, start=True, stop=True)
                nt = small.tile([K, K], bf16, tag="nt")
                nc.vector.tensor_mul(nt, ct, maskSL)
                a1 = psA.tile([K, K], f32, tag="kk")
                nc.tensor.matmul(a1, nt, A, start=True, stop=True)
                nc.vector.tensor_add(A, a1, ident)
            # final B-update (B_R from A_R) with the A-only epilogue work
            # (yt, q, then q-transposes and qX = q @ X_{b+1}) interleaved so
            # only ONE matmul (C' = B_R^T qX) remains after B_R
            z2 = psA.tile([K, K], f32, tag="kk")
            nc.tensor.matmul(z2, syy_ng, A, start=True, stop=True)
            z2s = small.tile([K, K], bf16, tag="z2")
            nc.scalar.copy(z2s, z2)
            ps_yt = psB.tile([K, H], f32, tag="big")
            nc.tensor.matmul(ps_yt, A, y0t, start=True, stop=True)
            g = psA.tile([K, K], f32, tag="kk")
            nc.tensor.matmul(g, A, z2s, start=True, stop=True)
            gm = small.tile([K, K], bf16, tag="gm")
            nc.vector.tensor_mul(gm, g, maskSL)
            ps_q0 = psB.tile([K, DS], f32, tag="big")
            nc.tensor.matmul(ps_q0, A, t0t[:, 0:DS], start=True, stop=True)
            b1 = psA.tile([K, K], f32, tag="kk")
            nc.tensor.matmul(b1, gm, Bprev, start=True, stop=True)
            ps_q1 = psB.tile([K, DS], f32, tag="big")
            nc.tensor.matmul(ps_q1, A, t0t[:, DS:D], start=True, stop=True)

            # ================= EPILOGUE (critical) =================
            yt = small.tile([K, H], bf16, tag="yt")
            nc.vector.tensor_copy(yt, ps_yt)
            q = small.tile([K, D], bf16, tag="q")
            nc.vector.tensor_sub(q[:, 0:DS], XTall[:, b, 0:DS], ps_q0)
            nc.vector.tensor_add(Bm, b1, ident)
            nc.vector.tensor_sub(q[:, DS:D], XTall[:, b, DS:D], ps_q1)
            if b + 1 < NBLK:
                # qT = transpose(q) then qX = q @ X_{b+1}: depends only on q
                # (A-side), overlaps the final B-chain on the PE
                qT = small.tile([DP, DC * K], bf16, tag="U")
                for p in range(3):
                    tp = psT.tile([DP, 2 * K], bf16, tag="tt")
                    nc.tensor.transpose(
                        tp[:, 0:K], q[:, (2 * p) * DP:(2 * p + 1) * DP], identb
                    )
                    nc.tensor.transpose(
                        tp[:, K:2 * K], q[:, (2 * p + 1) * DP:(2 * p + 2) * DP],
                        identb,
                    )
                    nc.scalar.copy(qT[:, (2 * p) * K:(2 * p + 2) * K], tp)
                tp = psT.tile([DP, K], bf16, tag="tt")
                nc.tensor.transpose(tp, q[:, 6 * DP:7 * DP], identb)
                nc.scalar.copy(qT[:, 6 * K:7 * K], tp)
                psqx = psA.tile([K, K], f32, tag="kk")
                for ic in range(DC):
                    nc.tensor.matmul(
                        psqx, qT[:, ic * K:(ic + 1) * K], Xall[:, b + 1, ic, :],
                        start=(ic == 0), stop=(ic == DC - 1),
                    )
                qxs = small.tile([K, K], bf16, tag="qx")
                nc.scalar.copy(qxs, psqx)
                psc = psA.tile([K, K], f32, tag="kk")
                nc.tensor.matmul(psc, Bm, qxs, start=True, stop=True)
                clr = small.tile([K, K], bf16, tag="clr")
                nc.scalar.mul(clr, psc, LR)
                y0q_next = small.tile([HP, HC * K], bf16, tag="y0")
                for hc in range(HC):
                    cps = psA.tile([K, K], f32, tag="kk")
                    nc.tensor.matmul(
                        cps, yt[:, hc * HP:(hc + 1) * HP], clr,
                        start=True, stop=True,
                    )
                    nc.vector.tensor_add(
                        y0q_next[:, hc * K:(hc + 1) * K], cps,
                        Psb[b + 1][:, hc * K:(hc + 1) * K],
                    )

            # ================= FILL TAIL =================
            tc.cur_priority = pri_fill(b)
            relu_y = small.tile([HP, HC * K], bf16, tag="ry")
            for hc in range(HC):
                ps2 = psA.tile([K, K], f32, tag="kk")
                nc.tensor.matmul(
                    ps2, y0t[:, hc * HP:(hc + 1) * HP], A, start=True, stop=True
                )
                nc.scalar.activation(
                    relu_y[:, hc * K:(hc + 1) * K], ps2,
                    mybir.ActivationFunctionType.Relu,
                )
            lg = psA.tile([K, O], f32, tag="kk")
            for hc in range(HC):
                nc.tensor.matmul(
                    lg, relu_y[:, hc * K:(hc + 1) * K], RT[:, hc, :],
                    start=(hc == 0), stop=(hc == HC - 1),
                )
            lgs = small.tile([K, O], f32, tag="lgs")
            tc.cur_priority = pri_out(b)
            nc.vector.tensor_add(lgs, lg, bb)
            nc.sync.dma_start(out=out_d[b * K:(b + 1) * K, :], in_=lgs)
            tc.cur_priority = pri_fill(b)

            if b + LOOKAHEAD < NBLK:
                tc.cur_priority = pri_whit(b + LOOKAHEAD)
                whiten(b + LOOKAHEAD)
                tc.cur_priority = pri_fill(b)
            if b + 1 < NBLK:
                # masters: W += lr * Y U^T (fp32, in place); Wb = cast(W);
                # WTb = PE-transpose of Wb. Deferred: emitted inside the NEXT
                # block's ring so engine streams order it behind that block's
                # critical head.
                def make_tail(b, yt, q, Bm):
                    def tail():
                        tc.cur_priority = pri_fill(b)
                        ut = small.tile([K, D], bf16, tag="ut")
                        for s in range(2):
                            ps = psB.tile([K, DS], f32, tag="big")
                            nc.tensor.matmul(
                                ps, Bm, q[:, s * DS:(s + 1) * DS],
                                start=True, stop=True,
                            )
                            nc.scalar.copy(ut[:, s * DS:(s + 1) * DS], ps)
                        for hc in range(HC):
                            for s in range(2):
                                ps = psB.tile([HP, DS], f32, tag="big")
                                nc.tensor.matmul(
                                    ps, yt[:, hc * HP:(hc + 1) * HP],
                                    ut[:, s * DS:(s + 1) * DS],
                                    start=True, stop=True,
                                )
                                wsl = W[:, hc * D + s * DS:hc * D + (s + 1) * DS]
                                nc.vector.scalar_tensor_tensor(
                                    wsl, ps, LR, wsl, op0=AT.mult, op1=AT.add
                                )
                                nc.scalar.copy(
                                    Wb[:, hc * D + s * DS:hc * D + (s + 1) * DS],
                                    wsl,
                                )
                        for dc in range(DC):
                            tp = psT.tile([DP, 2 * K], bf16, tag="tt")
                            nc.tensor.transpose(
                                tp[:, 0:HP],
                                Wb[:, 0 * D + dc * DP:0 * D + (dc + 1) * DP],
                                identb,
                            )
                            nc.tensor.transpose(
                                tp[:, HP:2 * HP],
                                Wb[:, 1 * D + dc * DP:1 * D + (dc + 1) * DP],
                                identb,
                            )
                            nc.scalar.copy(WTb[:, dc * H:(dc + 1) * H], tp)
                        if b + 2 < NBLK:
                            open_P(b + 2)
                    return tail

                pending_tail[0] = make_tail(b, yt, q, Bm)

    _split_multiwait(nc)
    return nc


def prep_inputs(x, whiten_mean, whiten_mat, oja_W, readout_W, readout_b):
    """Host-side layout/dtype prep (no contractions)."""
    x = np.ascontiguousarray(x, dtype=np.float32)
    mu_b = np.broadcast_to(
        np.asarray(whiten_mean, dtype=np.float32)[None, :], (128, D)
    ).copy()
    P = np.asarray(whiten_mat, dtype=np.float32) - np.eye(D, dtype=np.float32)
    # pt[dp, ic, dout] = P^T[ic*112+dp, dout] = P[dout, ic*112+dp]
    pt = np.ascontiguousarray(
        P.T.reshape(DC, DP, D).transpose(1, 0, 2).astype(ml_dtypes.bfloat16)
    )
    Wf = np.asarray(oja_W, dtype=np.float32)
    w = np.ascontiguousarray(
        Wf.reshape(HC, HP, D).transpose(1, 0, 2).reshape(HP, HC * D)
    )
    wtb = np.ascontiguousarray(
        Wf.T.reshape(DC, DP, H).transpose(1, 0, 2).reshape(DP, DC * H)
    ).astype(ml_dtypes.bfloat16)
    Rf = np.asarray(readout_W, dtype=np.float32)
    rt = np.ascontiguousarray(
        Rf.T.reshape(HC, HP, O).transpose(1, 0, 2).astype(ml_dtypes.bfloat16)
    )
    b_b = np.broadcast_to(
        np.asarray(readout_b, dtype=np.float32)[None, :], (128, O)
    ).copy()
    return {
        "x": x, "mu_b": mu_b, "pt": pt, "w": w, "rt": rt, "b_b": b_b,
        "w_bf": w.astype(ml_dtypes.bfloat16), "wt_bf": wtb,
    }


_cached_nc = None


def _get_nc():
    global _cached_nc
    if _cached_nc is None:
        _cached_nc = build_nc()
    return _cached_nc


def kernel(x, whiten_mean, whiten_mat, oja_W, readout_W, readout_b, **run_kwargs):
    nc = _get_nc()
    ins = prep_inputs(x, whiten_mean, whiten_mat, oja_W, readout_W, readout_b)
    res = run_bass_kernel_spmd(
        nc, [ins] * N_CORES, core_ids=list(range(N_CORES)), **run_kwargs
    )
    out = res.results[0]["out"]
    if run_kwargs:
        kernel.last_result = res
    return out


# revision 21
# speedup vs baseline: 1.0023x; 1.0023x over previous
"""Trainium2 Bass kernel for nn_BioClassifier: whitening + sequential Oja scan + readout.

v2: restructured for critical-path latency. Same block-parallel-scan math as v1
(chunk the 2048-sample Oja scan into 16 blocks of K=128; per block a fixed-point
"ring" on K x K matrices closes the sequential recurrence exactly):
    Y = Y0 A,  U = (X - T0 A) B,   A = (I - lr*SU(C))^-1, B = (I + lr*SU(G))^-1
    C = U^T X, G = Y^T Y,  T0^T X = Syy
Key v2 changes vs v1:
  * lr folded into sxx_lr/syy_lr (bf16), iteration reordered so each ring cycle
    is 10 serial engine-hops (B-chain: z2->z2s->g->gm->b1->B; A-chain: ct->nt->
    a1->A) with r1/s prep hidden under the B-chain; iter-0 A-update is 3 vec ops
    (A1 = I + SU(s0), s0 = sxx_lr - syy_lr), no matmuls.
  * Y0 correction form: P_{n} = W^{(n-1)} X_n accumulates in an OPEN PSUM group
    during ring_{n-1} (off critical path); epilogue closes it with the rank-K
    correction  Y0_n = P_n + lr * Y * (U^T X_n), so the master-W update and the
    14 Y0 matmuls leave the serial path entirely.
  * single fp32 master W [H,D]; Wb = cast(W) on scalar; WTb = PE-transpose of Wb
    (bf16 transpose == transpose of bf16 cast, exact) - drops the WT fp32 master
    and its vector-engine update entirely.
  * XTall/Sxxall stored bf16 (Sxx pre-scaled by lr at whiten time).
  * engine rebalance: ring bounces on DVE, z2s/copies on scalar, xc-sub and
    the iter-0 mask-mul on gpsimd (which cannot touch PSUM); whitening +
    masters + P-opens emitted in priority bands (and the master-update tail
    emitted mid-ring of the NEXT block) so the in-order engine streams place
    them behind each block's critical chain.
  * RING_ITERS=4 (validated offline: rel err 1.49e-2 vs the 2e-2 gate;
    RING_ITERS=5 gives 8.7e-3 at ~+60us).
All 8 cores run the identical program (the scan is inherently sequential;
core 0's output is returned).
"""

import os
import sys
from contextlib import ExitStack

sys.path.insert(0, "/opt/trn_rl_repo")

import numpy as np
import ml_dtypes

import concourse.bass as bass
import concourse.mybir as mybir
from concourse.tile import TileContext
from concourse.masks import make_identity
from concourse.bass_utils import run_bass_kernel_spmd
from concourse.vector_clock import ScopedClock

LR = 1e-3
B, D, H, O = 2048, 784, 256, 10
K = 128
NBLK = B // K
DP, DC = 112, 7          # D = 784 = 7 * 112
HP, HC = 128, 2          # H = 256 = 2 * 128
DS = D // 2              # 392: matmul free-dim split for D-wide outputs

RING_ITERS = int(os.environ.get("RING_ITERS", "4"))
LOOKAHEAD = 4
N_CORES = 8

f32 = mybir.dt.float32
bf16 = mybir.dt.bfloat16
AT = mybir.AluOpType


def _install_ntff_hook():
    """The agent image's `antenv` lacks `axon_hooks`, so trace=True degrades.
    Synthesize the module and register the ctypes NTFF hook from trn_boot."""
    import types
    import antenv

    if getattr(antenv, "axon_hooks", None) is not None:
        return
    mod = types.ModuleType("antenv.axon_hooks")
    _hook_box = [None]
    mod.set_axon_ntff_profile_hook = lambda h: _hook_box.__setitem__(0, h)
    mod.get_axon_ntff_profile_hook = lambda: _hook_box[0]
    sys.modules["antenv.axon_hooks"] = mod
    antenv.axon_hooks = mod
    try:
        sys.path.insert(0, "/root/.axon_site")
        from trn_agent_boot.trn_boot import _ntff_profile_via_ctypes

        hook = _ntff_profile_via_ctypes("/opt/axon/libaxon_pjrt.so")
        if hook is not None:
            mod.set_axon_ntff_profile_hook(hook)
    except Exception:
        pass


try:
    _install_ntff_hook()
except Exception:
    pass

_drain_patched = False


def _patch_drain():
    """This walrus build only supports one sync-wait per CTRL instruction;
    split the Tile kernel-tail drain into one drain per semaphore wait."""
    global _drain_patched
    if _drain_patched:
        return

    def patched(self, tick_clock, wait_clock):
        drain_inst = self.nc.sync.drain()
        wait_clock.add_sem_waits(
            drain_inst.ins, ScopedClock({None: tick_clock.global_clock})
        )
        mi = drain_inst.ins
        si = mi.sync_info
        if si is not None and len(si.on_wait) > 1:
            waits = list(si.on_wait)
            mi.sync_info = mybir.SyncInfo(
                on_wait=[waits[0]], on_update=list(si.on_update)
            )
            for w in waits[1:]:
                d2 = self.nc.sync.drain()
                d2.ins.sync_info = mybir.SyncInfo(on_wait=[w], on_update=[])
        self.nc.all_engine_barrier()
        assert self.sems is not None
        popped = self.nc._tile_sem_poison_stack.pop()
        assert popped is self._sem_poison
        self.nc.clear_and_free_semaphores(list(self.sems.allocated().values()))
        self.nc.all_engine_barrier()

    TileContext._drain_and_barrier = patched
    _drain_patched = True


def _split_multiwait(nc, limit=1):
    """This walrus build supports only `limit` sync-waits per instruction.
    Hoist extra waits onto NoOps inserted just before, in the same engine
    stream (engines are in-order, so earlier waits are strictly safe)."""
    n_split = 0
    for f in nc.m.functions:
        for bb in f.blocks:
            insts = list(bb.instructions)
            if not any(
                i.sync_info is not None and len(i.sync_info.on_wait) > limit
                for i in insts
            ):
                continue
            new = []
            for inst in insts:
                si = inst.sync_info
                if si is not None and len(si.on_wait) > limit:
                    waits = list(si.on_wait)
                    for j, w in enumerate(waits[: len(waits) - limit]):
                        nop = mybir.InstNoOp(
                            name=f"{inst.name}-hw{j}", engine=inst.engine,
                            ins=[], outs=[],
                        )
                        nop.sync_info = mybir.SyncInfo(on_wait=[w], on_update=[])
                        new.append(nop)
                        n_split += 1
                    inst.sync_info = mybir.SyncInfo(
                        on_wait=waits[len(waits) - limit:],
                        on_update=list(si.on_update),
                    )
                new.append(inst)
            bb.instructions = new
    return n_split


def build_nc(ring_iters=RING_ITERS):
    _patch_drain()
    nc = bass.Bass()
    x_d = nc.dram_tensor("x", [B, D], f32, kind="ExternalInput")
    mu_d = nc.dram_tensor("mu_b", [128, D], f32, kind="ExternalInput")
    pt_d = nc.dram_tensor("pt", [DP, DC, D], bf16, kind="ExternalInput")
    w_d = nc.dram_tensor("w", [HP, HC * D], f32, kind="ExternalInput")
    wb_d = nc.dram_tensor("w_bf", [HP, HC * D], bf16, kind="ExternalInput")
    wtb_d = nc.dram_tensor("wt_bf", [DP, DC * H], bf16, kind="ExternalInput")
    rt_d = nc.dram_tensor("rt", [HP, HC, O], bf16, kind="ExternalInput")
    bb_d = nc.dram_tensor("b_b", [128, O], f32, kind="ExternalInput")
    out_d = nc.dram_tensor("out", [B, O], f32, kind="ExternalOutput")
    def pri_crit(b):
        # critical path of block b
        return 1000 + b * 1000

    def pri_fill(b):
        # fill work of block b: must rank BELOW crit of b+1 (it runs during
        # ring_{b+1}) but above crit of b+2
        return 1000 + (b + 1) * 1000 + 500

    def pri_whit(b):
        # whiten(b) must complete before epilogue of b-1: rank just below
        # crit(b-1), above fill bands of earlier blocks
        return 1000 + (b - 1) * 1000 + 400

    def pri_out(b):
        return 20_000_000 + b * 1000

    with TileContext(nc) as tc, ExitStack() as ctx:
        persist = ctx.enter_context(tc.tile_pool(name="persist", bufs=1))
        xpool = ctx.enter_context(tc.tile_pool(name="xpool", bufs=5))
        small = ctx.enter_context(tc.tile_pool(name="small", bufs=2))
        psA = ctx.enter_context(tc.tile_pool(name="psA", bufs=3, space="PSUM"))
        psB = ctx.enter_context(tc.tile_pool(name="psB", bufs=3, space="PSUM"))
        psT = ctx.enter_context(tc.tile_pool(name="psT", bufs=2, space="PSUM"))

        ident = persist.tile([128, 128], f32, tag="ident")
        make_identity(nc, ident)
        identb = persist.tile([128, 128], bf16, tag="identb")
        nc.vector.tensor_copy(identb, ident)
        # 0/1 masks (lr is folded into sxx_lr / syy_lr)
        maskSL = persist.tile([K, K], f32, tag="maskSL")
        nc.gpsimd.memset(maskSL, 1.0)
        nc.gpsimd.affine_select(
            out=maskSL, in_=maskSL, compare_op=AT.is_gt, fill=0.0,
            base=0, pattern=[[-1, K]], channel_multiplier=1,
        )
        maskSU = persist.tile([K, K], f32, tag="maskSU")
        nc.gpsimd.memset(maskSU, 1.0)
        nc.vector.tensor_sub(maskSU, maskSU, ident)
        nc.vector.tensor_sub(maskSU, maskSU, maskSL)

        mu_t = persist.tile([128, D], f32, tag="mu")
        nc.sync.dma_start(out=mu_t, in_=mu_d[:, :])
        pt_t = persist.tile([DP, DC, D], bf16, tag="pt")
        nc.sync.dma_start(out=pt_t, in_=pt_d[:, :, :])
        WTb = persist.tile([DP, DC * H], bf16, tag="WTb")
        nc.sync.dma_start(out=WTb, in_=wtb_d[:, :])
        Wb = persist.tile([HP, HC * D], bf16, tag="Wb")
        nc.sync.dma_start(out=Wb, in_=wb_d[:, :])
        W = persist.tile([HP, HC * D], f32, tag="W")
        nc.sync.dma_start(out=W, in_=w_d[:, :])
        RT = persist.tile([HP, HC, O], bf16, tag="RT")
        nc.sync.dma_start(out=RT, in_=rt_d[:, :, :])
        bb = persist.tile([128, O], f32, tag="bb")
        nc.sync.dma_start(out=bb, in_=bb_d[:, :])

        Xall = persist.tile([DP, NBLK, DC, K], bf16, tag="Xall")
        XTall = persist.tile([K, NBLK, D], bf16, tag="XTall")
        Sxxall = persist.tile([K, NBLK, K], bf16, tag="Sxxall")

        # ---------------- whitening ----------------
        def whiten(bi):
            xt = xpool.tile([128, D], f32, tag="xraw")
            nc.sync.dma_start(out=xt, in_=x_d[bi * K:(bi + 1) * K, :])
            xc = xpool.tile([128, D], f32, tag="xc")
            eng = nc.vector if bi == 0 else nc.gpsimd
            eng.tensor_sub(xc, xt, mu_t)
            xcb = xpool.tile([128, D], bf16, tag="xcb")
            nc.vector.tensor_copy(xcb, xc)
            xct = xpool.tile([DP, DC * K], bf16, tag="xct")
            for p in range(3):          # paired transposes -> one copy per pair
                tp = psT.tile([DP, 2 * K], bf16, tag="tt")
                nc.tensor.transpose(
                    tp[:, 0:K], xcb[:, (2 * p) * DP:(2 * p + 1) * DP], identb
                )
                nc.tensor.transpose(
                    tp[:, K:2 * K], xcb[:, (2 * p + 1) * DP:(2 * p + 2) * DP],
                    identb,
                )
                nc.scalar.copy(xct[:, (2 * p) * K:(2 * p + 2) * K], tp)
            tp = psT.tile([DP, K], bf16, tag="tt")
            nc.tensor.transpose(tp, xcb[:, 6 * DP:7 * DP], identb)
            nc.scalar.copy(xct[:, 6 * K:7 * K], tp)
            XTb = XTall[:, bi, :]
            for s in range(2):
                ps = psB.tile([K, DS], f32, tag="big")
                for ic in range(DC):
                    nc.tensor.matmul(
                        ps, xct[:, ic * K:(ic + 1) * K],
                        pt_t[:, ic, s * DS:(s + 1) * DS],
                        start=(ic == 0), stop=(ic == DC - 1),
                    )
                nc.vector.tensor_add(
                    XTb[:, s * DS:(s + 1) * DS], ps, xc[:, s * DS:(s + 1) * DS]
                )
            xa = Xall[:, bi, :, :]
            for p in range(3):
                tp = psT.tile([DP, 2 * K], bf16, tag="tt")
                nc.tensor.transpose(
                    tp[:, 0:K], XTb[:, (2 * p) * DP:(2 * p + 1) * DP], identb
                )
                nc.tensor.transpose(
                    tp[:, K:2 * K], XTb[:, (2 * p + 1) * DP:(2 * p + 2) * DP],
                    identb,
                )
                nc.scalar.copy(xa[:, 2 * p:2 * p + 2, :], tp)
            tp = psT.tile([DP, K], bf16, tag="tt")
            nc.tensor.transpose(tp, XTb[:, 6 * DP:7 * DP], identb)
            nc.scalar.copy(xa[:, 6, :], tp)
            ps = psA.tile([K, K], f32, tag="kk")
            for ic in range(DC):
                nc.tensor.matmul(
                    ps, xa[:, ic, :], xa[:, ic, :],
                    start=(ic == 0), stop=(ic == DC - 1),
                )
            nc.scalar.mul(Sxxall[:, bi, :], ps, LR)

        tc.cur_priority = 0
        whiten(0)
        tc.cur_priority = pri_whit(1)
        whiten(1)
        tc.cur_priority = pri_whit(2)
        whiten(2)
        tc.cur_priority = pri_whit(3)
        whiten(3)

        Psb = {}  # block -> SBUF f32 tile [HP, HC*K] holding W^(stale) X_block

        def open_P(nb):
            """Accumulate P_nb = W^(current) X_nb into an SBUF f32 tile."""
            pt_sb = small.tile([HP, HC * K], f32, tag="Psb")
            for hc in range(HC):
                ps = psB.tile([HP, K], f32, tag="big")
                for ic in range(DC):
                    nc.tensor.matmul(
                        ps,
                        WTb[:, ic * H + hc * HP:ic * H + (hc + 1) * HP],
                        Xall[:, nb, ic, :],
                        start=(ic == 0), stop=(ic == DC - 1),
                    )
                nc.scalar.copy(pt_sb[:, hc * K:(hc + 1) * K], ps)
            Psb[nb] = pt_sb

        # P_1 with the initial weights (correction applied in epilogue of block 0).
        # Priority BELOW crit(0): open_P(1) waits on whiten(1), and at higher
        # priority it would head-of-line-block block 0's PE stream.
        tc.cur_priority = 1450
        open_P(1)
        y0q_next = None
        pending_tail = [None]

        for b in range(NBLK):
            # ================= HEAD (critical) =================
            tc.cur_priority = pri_crit(b)
            if b == 0:
                y0q = small.tile([HP, HC * K], bf16, tag="y0")
                for hc in range(HC):
                    ps = psB.tile([HP, K], f32, tag="big")
                    for ic in range(DC):
                        nc.tensor.matmul(
                            ps,
                            WTb[:, ic * H + hc * HP:ic * H + (hc + 1) * HP],
                            Xall[:, 0, ic, :],
                            start=(ic == 0), stop=(ic == DC - 1),
                        )
                    nc.vector.tensor_copy(y0q[:, hc * K:(hc + 1) * K], ps)
            else:
                y0q = y0q_next
            ps_syy = psA.tile([K, K], f32, tag="kk")
            for hc in range(HC):
                nc.tensor.matmul(
                    ps_syy, y0q[:, hc * K:(hc + 1) * K],
                    y0q[:, hc * K:(hc + 1) * K],
                    start=(hc == 0), stop=(hc == HC - 1),
                )
            syy_lr = small.tile([K, K], bf16, tag="syl")
            nc.scalar.mul(syy_lr, ps_syy, LR)
            syy_ng = small.tile([K, K], bf16, tag="syn")
            nc.vector.tensor_scalar_mul(syy_ng, ps_syy, -LR)

            # ---- head fill (off critical path) ----
            tc.cur_priority = pri_fill(b)
            y0t = small.tile([K, H], bf16, tag="y0t")
            for hc in range(HC):
                tp = psT.tile([128, K], bf16, tag="tt")
                nc.tensor.transpose(tp, y0q[:, hc * K:(hc + 1) * K], identb)
                nc.scalar.copy(y0t[:, hc * HP:(hc + 1) * HP], tp)
            # ================= RING (critical) =================
            tc.cur_priority = pri_crit(b)
            sxx = Sxxall[:, b, :]
            s_sb = small.tile([K, K], bf16, tag="s")
            nc.vector.scalar_tensor_tensor(
                s_sb, ps_syy, -LR, sxx, op0=AT.mult, op1=AT.add
            )
            tA0 = small.tile([K, K], bf16, tag="ta")
            nc.gpsimd.tensor_mul(tA0, s_sb, maskSU)
            A = small.tile([K, K], bf16, tag="A")
            nc.vector.tensor_add(A, tA0, ident)
            Bm = small.tile([K, K], bf16, tag="B")
            Bprev = identb
            for m in range(1, ring_iters):
                if m == 2:
                    # emit the previous block's master-update chain here so the
                    # scheduler places it after this block's head/early ring in
                    # every engine stream; t0t must follow it (reads updated Wb)
                    if pending_tail[0] is not None:
                        pending_tail[0]()
                        pending_tail[0] = None
                    tc.cur_priority = pri_fill(b - 1) if b else pri_fill(0)
                    t0t = small.tile([K, D], bf16, tag="t0t")
                    for s in range(2):
                        ps = psB.tile([K, DS], f32, tag="big")
                        for hc in range(HC):
                            nc.tensor.matmul(
                                ps, y0q[:, hc * K:(hc + 1) * K],
                                Wb[:, hc * D + s * DS:hc * D + (s + 1) * DS],
                                start=(hc == 0), stop=(hc == HC - 1),
                            )
                        nc.vector.tensor_copy(t0t[:, s * DS:(s + 1) * DS], ps)
                    tc.cur_priority = pri_crit(b)
                z2 = psA.tile([K, K], f32, tag="kk")
                nc.tensor.matmul(z2, syy_ng, A, start=True, stop=True)
                z2s = small.tile([K, K], bf16, tag="z2")
                nc.scalar.copy(z2s, z2)
                r1 = psA.tile([K, K], f32, tag="kk")
                nc.tensor.matmul(r1, A, syy_lr, start=True, stop=True)
                s_sb = small.tile([K, K], bf16, tag="s")
                nc.vector.tensor_sub(s_sb, sxx, r1)
                g = psA.tile([K, K], f32, tag="kk")
                nc.tensor.matmul(g, A, z2s, start=True, stop=True)
                gm = small.tile([K, K], bf16, tag="gm")
                nc.vector.tensor_mul(gm, g, maskSL)
                b1 = psA.tile([K, K], f32, tag="kk")
                nc.tensor.matmul(b1, gm, Bprev, start=True, stop=True)
                nc.vector.tensor_add(Bm, b1, ident)
                Bprev = Bm
                ct = psA.tile([K, K], f32, tag="kk")
                nc.tensor.matmul(ct, s_sb, Bm, start=True, stop=True)
                nt = small.tile([K, K], bf16, tag="nt")
                nc.vector.tensor_mul(nt, ct, maskSL)
                a1 = psA.tile([K, K], f32, tag="kk")
                nc.tensor.matmul(a1, nt, A, start=True, stop=True)
                nc.vector.tensor_add(A, a1, ident)
            # final B-update (B_R from A_R) with the A-only epilogue work
            # (yt, q, then q-transposes and qX = q @ X_{b+1}) interleaved so
            # only ONE matmul (C' = B_R^T qX) remains after B_R
            z2 = psA.tile([K, K], f32, tag="kk")
            nc.tensor.matmul(z2, syy_ng, A, start=True, stop=True)
            z2s = small.tile([K, K], bf16, tag="z2")
            nc.scalar.copy(z2s, z2)
            ps_yt = psB.tile([K, H], f32, tag="big")
            nc.tensor.matmul(ps_yt, A, y0t, start=True, stop=True)
            g = psA.tile([K, K], f32, tag="kk")
            nc.tensor.matmul(g, A, z2s, start=True, stop=True)
            gm = small.tile([K, K], bf16, tag="gm")
            nc.vector.tensor_mul(gm, g, maskSL)
            ps_q0 = psB.tile([K, DS], f32, tag="big")
            nc.tensor.matmul(ps_q0, A, t0t[:, 0:DS], start=True, stop=True)
            b1 = psA.tile([K, K], f32, tag="kk")
            nc.tensor.matmul(b1, gm, Bprev, start=True, stop=True)
            ps_q1 = psB.tile([K, DS], f32, tag="big")
            nc.tensor.matmul(ps_q1, A, t0t[:, DS:D], start=True, stop=True)

            # ================= EPILOGUE (critical) =================
            nc.vector.tensor_add(Bm, b1, ident)
            yt = small.tile([K, H], bf16, tag="yt")
            nc.vector.tensor_copy(yt, ps_yt)
            q = small.tile([K, D], bf16, tag="q")
            nc.vector.tensor_sub(q[:, 0:DS], XTall[:, b, 0:DS], ps_q0)
            nc.vector.tensor_sub(q[:, DS:D], XTall[:, b, DS:D], ps_q1)
            if b + 1 < NBLK:
                # qT = transpose(q) then qX = q @ X_{b+1}: depends only on q
                # (A-side), overlaps the final B-chain on the PE
                qT = small.tile([DP, DC * K], bf16, tag="U")
                for p in range(3):
                    tp = psT.tile([DP, 2 * K], bf16, tag="tt")
                    nc.tensor.transpose(
                        tp[:, 0:K], q[:, (2 * p) * DP:(2 * p + 1) * DP], identb
                    )
                    nc.tensor.transpose(
                        tp[:, K:2 * K], q[:, (2 * p + 1) * DP:(2 * p + 2) * DP],
                        identb,
                    )
                    nc.scalar.copy(qT[:, (2 * p) * K:(2 * p + 2) * K], tp)
                tp = psT.tile([DP, K], bf16, tag="tt")
                nc.tensor.transpose(tp, q[:, 6 * DP:7 * DP], identb)
                nc.scalar.copy(qT[:, 6 * K:7 * K], tp)
                psqx = psA.tile([K, K], f32, tag="kk")
                for ic in range(DC):
                    nc.tensor.matmul(
                        psqx, qT[:, ic * K:(ic + 1) * K], Xall[:, b + 1, ic, :],
                        start=(ic == 0), stop=(ic == DC - 1),
                    )
                qxs = small.tile([K, K], bf16, tag="qx")
                nc.scalar.copy(qxs, psqx)
                psc = psA.tile([K, K], f32, tag="kk")
                nc.tensor.matmul(psc, Bm, qxs, start=True, stop=True)
                clr = small.tile([K, K], bf16, tag="clr")
                nc.scalar.mul(clr, psc, LR)
                y0q_next = small.tile([HP, HC * K], bf16, tag="y0")
                for hc in range(HC):
                    cps = psA.tile([K, K], f32, tag="kk")
                    nc.tensor.matmul(
                        cps, yt[:, hc * HP:(hc + 1) * HP], clr,
                        start=True, stop=True,
                    )
                    nc.vector.tensor_add(
                        y0q_next[:, hc * K:(hc + 1) * K], cps,
                        Psb[b + 1][:, hc * K:(hc + 1) * K],
                    )
                ut = small.tile([K, D], bf16, tag="ut")
                for s in range(2):
                    ps = psB.tile([K, DS], f32, tag="big")
                    nc.tensor.matmul(
                        ps, Bm, q[:, s * DS:(s + 1) * DS], start=True, stop=True
                    )
                    nc.scalar.copy(ut[:, s * DS:(s + 1) * DS], ps)

            # ================= FILL TAIL =================
            tc.cur_priority = pri_fill(b)
            relu_y = small.tile([HP, HC * K], bf16, tag="ry")
            for hc in range(HC):
                ps2 = psA.tile([K, K], f32, tag="kk")
                nc.tensor.matmul(
                    ps2, y0t[:, hc * HP:(hc + 1) * HP], A, start=True, stop=True
                )
                nc.scalar.activation(
                    relu_y[:, hc * K:(hc + 1) * K], ps2,
                    mybir.ActivationFunctionType.Relu,
                )
            lg = psA.tile([K, O], f32, tag="kk")
            for hc in range(HC):
                nc.tensor.matmul(
                    lg, relu_y[:, hc * K:(hc + 1) * K], RT[:, hc, :],
                    start=(hc == 0), stop=(hc == HC - 1),
                )
            lgs = small.tile([K, O], f32, tag="lgs")
            tc.cur_priority = pri_out(b)
            nc.vector.tensor_add(lgs, lg, bb)
            nc.sync.dma_start(out=out_d[b * K:(b + 1) * K, :], in_=lgs)
            tc.cur_priority = pri_fill(b)

            if b + LOOKAHEAD < NBLK:
                tc.cur_priority = pri_whit(b + LOOKAHEAD)
                whiten(b + LOOKAHEAD)
                tc.cur_priority = pri_fill(b)
            if b + 1 < NBLK:
                # masters: W += lr * Y U^T (fp32, in place); Wb = cast(W);
                # WTb = PE-transpose of Wb. Deferred: emitted inside the NEXT
                # block's ring so engine streams order it behind that block's
                # critical head.
                def make_tail(b, yt, ut):
                    def tail():
                        tc.cur_priority = pri_fill(b)
                        for hc in range(HC):
                            for s in range(2):
                                ps = psB.tile([HP, DS], f32, tag="big")
                                nc.tensor.matmul(
                                    ps, yt[:, hc * HP:(hc + 1) * HP],
                                    ut[:, s * DS:(s + 1) * DS],
                                    start=True, stop=True,
                                )
                                wsl = W[:, hc * D + s * DS:hc * D + (s + 1) * DS]
                                nc.vector.scalar_tensor_tensor(
                                    wsl, ps, LR, wsl, op0=AT.mult, op1=AT.add
                                )
                                nc.scalar.copy(
                                    Wb[:, hc * D + s * DS:hc * D + (s + 1) * DS],
                                    wsl,
                                )
                        for dc in range(DC):
                            tp = psT.tile([DP, 2 * K], bf16, tag="tt")
                            nc.tensor.transpose(
                                tp[:, 0:HP],
                                Wb[:, 0 * D + dc * DP:0 * D + (dc + 1) * DP],
                                identb,
                            )
                            nc.tensor.transpose(
                                tp[:, HP:2 * HP],
                                Wb[:, 1 * D + dc * DP:1 * D + (dc + 1) * DP],
                                identb,
                            )
                            nc.scalar.copy(WTb[:, dc * H:(dc + 1) * H], tp)
                        if b + 2 < NBLK:
                            open_P(b + 2)
                    return tail

                pending_tail[0] = make_tail(b, yt, ut)

    _split_multiwait(nc)
    return nc


def prep_inputs(x, whiten_mean, whiten_mat, oja_W, readout_W, readout_b):
    """Host-side layout/dtype prep (no contractions)."""
    x = np.ascontiguousarray(x, dtype=np.float32)
    mu_b = np.broadcast_to(
        np.asarray(whiten_mean, dtype=np.float32)[None, :], (128, D)
    ).copy()
    P = np.asarray(whiten_mat, dtype=np.float32) - np.eye(D, dtype=np.float32)
    # pt[dp, ic, dout] = P^T[ic*112+dp, dout] = P[dout, ic*112+dp]
    pt = np.ascontiguousarray(
        P.T.reshape(DC, DP, D).transpose(1, 0, 2).astype(ml_dtypes.bfloat16)
    )
    Wf = np.asarray(oja_W, dtype=np.float32)
    w = np.ascontiguousarray(
        Wf.reshape(HC, HP, D).transpose(1, 0, 2).reshape(HP, HC * D)
    )
    wtb = np.ascontiguousarray(
        Wf.T.reshape(DC, DP, H).transpose(1, 0, 2).reshape(DP, DC * H)
    ).astype(ml_dtypes.bfloat16)
    Rf = np.asarray(readout_W, dtype=np.float32)
    rt = np.ascontiguousarray(
        Rf.T.reshape(HC, HP, O).transpose(1, 0, 2).astype(ml_dtypes.bfloat16)
    )
    b_b = np.broadcast_to(
        np.asarray(readout_b, dtype=np.float32)[None, :], (128, O)
    ).copy()
    return {
        "x": x, "mu_b": mu_b, "pt": pt, "w": w, "rt": rt, "b_b": b_b,
        "w_bf": w.astype(ml_dtypes.bfloat16), "wt_bf": wtb,
    }


_cached_nc = None


def _get_nc():
    global _cached_nc
    if _cached_nc is None:
        _cached_nc = build_nc()
    return _cached_nc


def kernel(x, whiten_mean, whiten_mat, oja_W, readout_W, readout_b, **run_kwargs):
    nc = _get_nc()
    ins = prep_inputs(x, whiten_mean, whiten_mat, oja_W, readout_W, readout_b)
    res = run_bass_kernel_spmd(
        nc, [ins] * N_CORES, core_ids=list(range(N_CORES)), **run_kwargs
    )
    out = res.results[0]["out"]
    if run_kwargs:
        kernel.last_result = res
    return out


# revision 22
# speedup vs baseline: 1.0233x; 1.0209x over previous
"""Trainium2 Bass kernel for nn_BioClassifier: whitening + sequential Oja scan + readout.

v2: restructured for critical-path latency. Same block-parallel-scan math as v1
(chunk the 2048-sample Oja scan into 16 blocks of K=128; per block a fixed-point
"ring" on K x K matrices closes the sequential recurrence exactly):
    Y = Y0 A,  U = (X - T0 A) B,   A = (I - lr*SU(C))^-1, B = (I + lr*SU(G))^-1
    C = U^T X, G = Y^T Y,  T0^T X = Syy
Key v2 changes vs v1:
  * lr folded into sxx_lr/syy_lr (bf16), iteration reordered so each ring cycle
    is 10 serial engine-hops (B-chain: z2->z2s->g->gm->b1->B; A-chain: ct->nt->
    a1->A) with r1/s prep hidden under the B-chain; iter-0 A-update is 3 vec ops
    (A1 = I + SU(s0), s0 = sxx_lr - syy_lr), no matmuls.
  * Y0 correction form: P_{n} = W^{(n-1)} X_n accumulates in an OPEN PSUM group
    during ring_{n-1} (off critical path); epilogue closes it with the rank-K
    correction  Y0_n = P_n + lr * Y * (U^T X_n), so the master-W update and the
    14 Y0 matmuls leave the serial path entirely.
  * single fp32 master W [H,D]; Wb = cast(W) on scalar; WTb = PE-transpose of Wb
    (bf16 transpose == transpose of bf16 cast, exact) - drops the WT fp32 master
    and its vector-engine update entirely.
  * XTall/Sxxall stored bf16 (Sxx pre-scaled by lr at whiten time).
  * engine rebalance: ring bounces on DVE, z2s/copies on scalar, xc-sub and
    the iter-0 mask-mul on gpsimd (which cannot touch PSUM); whitening +
    masters + P-opens emitted in priority bands (and the master-update tail
    emitted mid-ring of the NEXT block) so the in-order engine streams place
    them behind each block's critical chain.
  * RING_ITERS=4 (validated offline: rel err 1.49e-2 vs the 2e-2 gate;
    RING_ITERS=5 gives 8.7e-3 at ~+60us).
All 8 cores run the identical program (the scan is inherently sequential;
core 0's output is returned).
"""

import os
import sys
from contextlib import ExitStack

sys.path.insert(0, "/opt/trn_rl_repo")

import numpy as np
import ml_dtypes

import concourse.bass as bass
import concourse.mybir as mybir
from concourse.tile import TileContext
from concourse.masks import make_identity
from concourse.bass_utils import run_bass_kernel_spmd
from concourse.vector_clock import ScopedClock

LR = 1e-3
B, D, H, O = 2048, 784, 256, 10
K = 128
NBLK = B // K
DP, DC = 112, 7          # D = 784 = 7 * 112
HP, HC = 128, 2          # H = 256 = 2 * 128
DS = D // 2              # 392: matmul free-dim split for D-wide outputs

RING_ITERS = int(os.environ.get("RING_ITERS", "4"))
LOOKAHEAD = 4
N_CORES = 8

f32 = mybir.dt.float32
bf16 = mybir.dt.bfloat16
AT = mybir.AluOpType


def _install_ntff_hook():
    """The agent image's `antenv` lacks `axon_hooks`, so trace=True degrades.
    Synthesize the module and register the ctypes NTFF hook from trn_boot."""
    import types
    import antenv

    if getattr(antenv, "axon_hooks", None) is not None:
        return
    mod = types.ModuleType("antenv.axon_hooks")
    _hook_box = [None]
    mod.set_axon_ntff_profile_hook = lambda h: _hook_box.__setitem__(0, h)
    mod.get_axon_ntff_profile_hook = lambda: _hook_box[0]
    sys.modules["antenv.axon_hooks"] = mod
    antenv.axon_hooks = mod
    try:
        sys.path.insert(0, "/root/.axon_site")
        from trn_agent_boot.trn_boot import _ntff_profile_via_ctypes

        hook = _ntff_profile_via_ctypes("/opt/axon/libaxon_pjrt.so")
        if hook is not None:
            mod.set_axon_ntff_profile_hook(hook)
    except Exception:
        pass


try:
    _install_ntff_hook()
except Exception:
    pass

_drain_patched = False


def _patch_drain():
    """This walrus build only supports one sync-wait per CTRL instruction;
    split the Tile kernel-tail drain into one drain per semaphore wait."""
    global _drain_patched
    if _drain_patched:
        return

    def patched(self, tick_clock, wait_clock):
        drain_inst = self.nc.sync.drain()
        wait_clock.add_sem_waits(
            drain_inst.ins, ScopedClock({None: tick_clock.global_clock})
        )
        mi = drain_inst.ins
        si = mi.sync_info
        if si is not None and len(si.on_wait) > 1:
            waits = list(si.on_wait)
            mi.sync_info = mybir.SyncInfo(
                on_wait=[waits[0]], on_update=list(si.on_update)
            )
            for w in waits[1:]:
                d2 = self.nc.sync.drain()
                d2.ins.sync_info = mybir.SyncInfo(on_wait=[w], on_update=[])
        self.nc.all_engine_barrier()
        assert self.sems is not None
        popped = self.nc._tile_sem_poison_stack.pop()
        assert popped is self._sem_poison
        self.nc.clear_and_free_semaphores(list(self.sems.allocated().values()))
        self.nc.all_engine_barrier()

    TileContext._drain_and_barrier = patched
    _drain_patched = True


def _split_multiwait(nc, limit=1):
    """This walrus build supports only `limit` sync-waits per instruction.
    Hoist extra waits onto NoOps inserted just before, in the same engine
    stream (engines are in-order, so earlier waits are strictly safe)."""
    n_split = 0
    for f in nc.m.functions:
        for bb in f.blocks:
            insts = list(bb.instructions)
            if not any(
                i.sync_info is not None and len(i.sync_info.on_wait) > limit
                for i in insts
            ):
                continue
            new = []
            for inst in insts:
                si = inst.sync_info
                if si is not None and len(si.on_wait) > limit:
                    waits = list(si.on_wait)
                    for j, w in enumerate(waits[: len(waits) - limit]):
                        nop = mybir.InstNoOp(
                            name=f"{inst.name}-hw{j}", engine=inst.engine,
                            ins=[], outs=[],
                        )
                        nop.sync_info = mybir.SyncInfo(on_wait=[w], on_update=[])
                        new.append(nop)
                        n_split += 1
                    inst.sync_info = mybir.SyncInfo(
                        on_wait=waits[len(waits) - limit:],
                        on_update=list(si.on_update),
                    )
                new.append(inst)
            bb.instructions = new
    return n_split


def build_nc(ring_iters=RING_ITERS):
    _patch_drain()
    nc = bass.Bass()
    x_d = nc.dram_tensor("x", [B, D], f32, kind="ExternalInput")
    mu_d = nc.dram_tensor("mu_b", [128, D], f32, kind="ExternalInput")
    pt_d = nc.dram_tensor("pt", [DP, DC, D], bf16, kind="ExternalInput")
    w_d = nc.dram_tensor("w", [HP, HC * D], f32, kind="ExternalInput")
    wb_d = nc.dram_tensor("w_bf", [HP, HC * D], bf16, kind="ExternalInput")
    wtb_d = nc.dram_tensor("wt_bf", [DP, DC * H], bf16, kind="ExternalInput")
    rt_d = nc.dram_tensor("rt", [HP, HC, O], bf16, kind="ExternalInput")
    bb_d = nc.dram_tensor("b_b", [128, O], f32, kind="ExternalInput")
    out_d = nc.dram_tensor("out", [B, O], f32, kind="ExternalOutput")
    def pri_crit(b):
        # critical path of block b
        return 1000 + b * 1000

    def pri_fill(b):
        # fill work of block b: must rank BELOW crit of b+1 (it runs during
        # ring_{b+1}) but above crit of b+2
        return 1000 + (b + 1) * 1000 + 500

    def pri_whit(b):
        # whiten(b) must complete before epilogue of b-1: rank just below
        # crit(b-1), above fill bands of earlier blocks
        return 1000 + (b - 1) * 1000 + 400

    def pri_out(b):
        return 20_000_000 + b * 1000

    with TileContext(nc) as tc, ExitStack() as ctx:
        persist = ctx.enter_context(tc.tile_pool(name="persist", bufs=1))
        xpool = ctx.enter_context(tc.tile_pool(name="xpool", bufs=5))
        small = ctx.enter_context(tc.tile_pool(name="small", bufs=2))
        psA = ctx.enter_context(tc.tile_pool(name="psA", bufs=3, space="PSUM"))
        psB = ctx.enter_context(tc.tile_pool(name="psB", bufs=3, space="PSUM"))
        psT = ctx.enter_context(tc.tile_pool(name="psT", bufs=2, space="PSUM"))

        ident = persist.tile([128, 128], f32, tag="ident")
        make_identity(nc, ident)
        identb = persist.tile([128, 128], bf16, tag="identb")
        nc.vector.tensor_copy(identb, ident)
        # 0/1 masks (lr is folded into sxx_lr / syy_lr)
        maskSL = persist.tile([K, K], f32, tag="maskSL")
        nc.gpsimd.memset(maskSL, 1.0)
        nc.gpsimd.affine_select(
            out=maskSL, in_=maskSL, compare_op=AT.is_gt, fill=0.0,
            base=0, pattern=[[-1, K]], channel_multiplier=1,
        )
        maskSU = persist.tile([K, K], f32, tag="maskSU")
        nc.gpsimd.memset(maskSU, 1.0)
        nc.vector.tensor_sub(maskSU, maskSU, ident)
        nc.vector.tensor_sub(maskSU, maskSU, maskSL)

        mu_t = persist.tile([128, D], f32, tag="mu")
        nc.sync.dma_start(out=mu_t, in_=mu_d[:, :])
        pt_t = persist.tile([DP, DC, D], bf16, tag="pt")
        nc.sync.dma_start(out=pt_t, in_=pt_d[:, :, :])
        WTb = persist.tile([DP, DC * H], bf16, tag="WTb")
        nc.sync.dma_start(out=WTb, in_=wtb_d[:, :])
        Wb = persist.tile([HP, HC * D], bf16, tag="Wb")
        nc.sync.dma_start(out=Wb, in_=wb_d[:, :])
        W = persist.tile([HP, HC * D], f32, tag="W")
        nc.sync.dma_start(out=W, in_=w_d[:, :])
        RT = persist.tile([HP, HC, O], bf16, tag="RT")
        nc.sync.dma_start(out=RT, in_=rt_d[:, :, :])
        bb = persist.tile([128, O], f32, tag="bb")
        nc.sync.dma_start(out=bb, in_=bb_d[:, :])

        Xall = persist.tile([DP, NBLK, DC, K], bf16, tag="Xall")
        XTall = persist.tile([K, NBLK, D], bf16, tag="XTall")
        Sxxall = persist.tile([K, NBLK, K], bf16, tag="Sxxall")

        # ---------------- whitening ----------------
        def whiten(bi):
            xt = xpool.tile([128, D], f32, tag="xraw")
            nc.sync.dma_start(out=xt, in_=x_d[bi * K:(bi + 1) * K, :])
            xc = xpool.tile([128, D], f32, tag="xc")
            eng = nc.vector if bi == 0 else nc.gpsimd
            eng.tensor_sub(xc, xt, mu_t)
            xcb = xpool.tile([128, D], bf16, tag="xcb")
            nc.vector.tensor_copy(xcb, xc)
            xct = xpool.tile([DP, DC * K], bf16, tag="xct")
            for p in range(3):          # paired transposes -> one copy per pair
                tp = psT.tile([DP, 2 * K], bf16, tag="tt")
                nc.tensor.transpose(
                    tp[:, 0:K], xcb[:, (2 * p) * DP:(2 * p + 1) * DP], identb
                )
                nc.tensor.transpose(
                    tp[:, K:2 * K], xcb[:, (2 * p + 1) * DP:(2 * p + 2) * DP],
                    identb,
                )
                nc.scalar.copy(xct[:, (2 * p) * K:(2 * p + 2) * K], tp)
            tp = psT.tile([DP, K], bf16, tag="tt")
            nc.tensor.transpose(tp, xcb[:, 6 * DP:7 * DP], identb)
            nc.scalar.copy(xct[:, 6 * K:7 * K], tp)
            XTb = XTall[:, bi, :]
            for s in range(2):
                ps = psB.tile([K, DS], f32, tag="big")
                for ic in range(DC):
                    nc.tensor.matmul(
                        ps, xct[:, ic * K:(ic + 1) * K],
                        pt_t[:, ic, s * DS:(s + 1) * DS],
                        start=(ic == 0), stop=(ic == DC - 1),
                    )
                nc.vector.tensor_add(
                    XTb[:, s * DS:(s + 1) * DS], ps, xc[:, s * DS:(s + 1) * DS]
                )
            xa = Xall[:, bi, :, :]
            for p in range(3):
                tp = psT.tile([DP, 2 * K], bf16, tag="tt")
                nc.tensor.transpose(
                    tp[:, 0:K], XTb[:, (2 * p) * DP:(2 * p + 1) * DP], identb
                )
                nc.tensor.transpose(
                    tp[:, K:2 * K], XTb[:, (2 * p + 1) * DP:(2 * p + 2) * DP],
                    identb,
                )
                nc.scalar.copy(xa[:, 2 * p:2 * p + 2, :], tp)
            tp = psT.tile([DP, K], bf16, tag="tt")
            nc.tensor.transpose(tp, XTb[:, 6 * DP:7 * DP], identb)
            nc.scalar.copy(xa[:, 6, :], tp)
            ps = psA.tile([K, K], f32, tag="kk")
            for ic in range(DC):
                nc.tensor.matmul(
                    ps, xa[:, ic, :], xa[:, ic, :],
                    start=(ic == 0), stop=(ic == DC - 1),
                )
            nc.scalar.mul(Sxxall[:, bi, :], ps, LR)

        tc.cur_priority = 0
        whiten(0)
        tc.cur_priority = pri_whit(1)
        whiten(1)
        tc.cur_priority = pri_whit(2)
        whiten(2)
        tc.cur_priority = pri_whit(3)
        whiten(3)

        Psb = {}  # block -> SBUF f32 tile [HP, HC*K] holding W^(stale) X_block

        def open_P(nb):
            """Accumulate P_nb = W^(current) X_nb into an SBUF f32 tile."""
            pt_sb = small.tile([HP, HC * K], f32, tag="Psb")
            for hc in range(HC):
                ps = psB.tile([HP, K], f32, tag="big")
                for ic in range(DC):
                    nc.tensor.matmul(
                        ps,
                        WTb[:, ic * H + hc * HP:ic * H + (hc + 1) * HP],
                        Xall[:, nb, ic, :],
                        start=(ic == 0), stop=(ic == DC - 1),
                    )
                nc.scalar.copy(pt_sb[:, hc * K:(hc + 1) * K], ps)
            Psb[nb] = pt_sb

        # P_1 with the initial weights (correction applied in epilogue of block 0).
        # Priority BELOW crit(0): open_P(1) waits on whiten(1), and at higher
        # priority it would head-of-line-block block 0's PE stream.
        tc.cur_priority = 1450
        open_P(1)
        y0q_next = None
        pending_tail = [None]

        for b in range(NBLK):
            # ================= HEAD (critical) =================
            tc.cur_priority = pri_crit(b)
            if b == 0:
                y0q = small.tile([HP, HC * K], bf16, tag="y0")
                for hc in range(HC):
                    ps = psB.tile([HP, K], f32, tag="big")
                    for ic in range(DC):
                        nc.tensor.matmul(
                            ps,
                            WTb[:, ic * H + hc * HP:ic * H + (hc + 1) * HP],
                            Xall[:, 0, ic, :],
                            start=(ic == 0), stop=(ic == DC - 1),
                        )
                    nc.vector.tensor_copy(y0q[:, hc * K:(hc + 1) * K], ps)
            else:
                y0q = y0q_next
            ps_syy = psA.tile([K, K], f32, tag="kk")
            for hc in range(HC):
                nc.tensor.matmul(
                    ps_syy, y0q[:, hc * K:(hc + 1) * K],
                    y0q[:, hc * K:(hc + 1) * K],
                    start=(hc == 0), stop=(hc == HC - 1),
                )
            syy_lr = small.tile([K, K], bf16, tag="syl")
            nc.scalar.mul(syy_lr, ps_syy, LR)
            syy_ng = small.tile([K, K], bf16, tag="syn")
            nc.vector.tensor_scalar_mul(syy_ng, ps_syy, -LR)

            # ---- head fill (off critical path) ----
            tc.cur_priority = pri_fill(b)
            y0t = small.tile([K, H], bf16, tag="y0t")
            for hc in range(HC):
                tp = psT.tile([128, K], bf16, tag="tt")
                nc.tensor.transpose(tp, y0q[:, hc * K:(hc + 1) * K], identb)
                nc.scalar.copy(y0t[:, hc * HP:(hc + 1) * HP], tp)
            # ================= RING (critical) =================
            tc.cur_priority = pri_crit(b)
            sxx = Sxxall[:, b, :]
            s_sb = small.tile([K, K], bf16, tag="s")
            nc.vector.scalar_tensor_tensor(
                s_sb, ps_syy, -LR, sxx, op0=AT.mult, op1=AT.add
            )
            tA0 = small.tile([K, K], bf16, tag="ta")
            nc.gpsimd.tensor_mul(tA0, s_sb, maskSU)
            A = small.tile([K, K], bf16, tag="A")
            nc.vector.tensor_add(A, tA0, ident)
            Bm = small.tile([K, K], bf16, tag="B")
            Bprev = identb
            for m in range(1, ring_iters):
                if m == 2:
                    # emit the previous block's master-update chain here so the
                    # scheduler places it after this block's head/early ring in
                    # every engine stream; t0t must follow it (reads updated Wb)
                    if pending_tail[0] is not None:
                        pending_tail[0]()
                        pending_tail[0] = None
                    tc.cur_priority = pri_fill(b - 1) if b else pri_fill(0)
                    t0t = small.tile([K, D], bf16, tag="t0t")
                    for s in range(2):
                        ps = psB.tile([K, DS], f32, tag="big")
                        for hc in range(HC):
                            nc.tensor.matmul(
                                ps, y0q[:, hc * K:(hc + 1) * K],
                                Wb[:, hc * D + s * DS:hc * D + (s + 1) * DS],
                                start=(hc == 0), stop=(hc == HC - 1),
                            )
                        nc.vector.tensor_copy(t0t[:, s * DS:(s + 1) * DS], ps)
                    tc.cur_priority = pri_crit(b)
                z2 = psA.tile([K, K], f32, tag="kk")
                nc.tensor.matmul(z2, syy_ng, A, start=True, stop=True)
                z2s = small.tile([K, K], bf16, tag="z2")
                nc.scalar.copy(z2s, z2)
                r1 = psA.tile([K, K], f32, tag="kk")
                nc.tensor.matmul(r1, A, syy_lr, start=True, stop=True)
                s_sb = small.tile([K, K], bf16, tag="s")
                nc.vector.tensor_sub(s_sb, sxx, r1)
                g = psA.tile([K, K], f32, tag="kk")
                nc.tensor.matmul(g, A, z2s, start=True, stop=True)
                gm = small.tile([K, K], bf16, tag="gm")
                nc.vector.tensor_mul(gm, g, maskSL)
                b1 = psA.tile([K, K], f32, tag="kk")
                nc.tensor.matmul(b1, gm, Bprev, start=True, stop=True)
                nc.vector.tensor_add(Bm, b1, ident)
                Bprev = Bm
                ct = psA.tile([K, K], f32, tag="kk")
                nc.tensor.matmul(ct, s_sb, Bm, start=True, stop=True)
                nt = small.tile([K, K], bf16, tag="nt")
                nc.vector.tensor_mul(nt, ct, maskSL)
                a1 = psA.tile([K, K], f32, tag="kk")
                nc.tensor.matmul(a1, nt, A, start=True, stop=True)
                nc.vector.tensor_add(A, a1, ident)
            # final B-update (B_R from A_R), epilogue A-work interleaved to
            # fill PE gaps while z2s/gm bounce on scalar/vector
            z2 = psA.tile([K, K], f32, tag="kk")
            nc.tensor.matmul(z2, syy_ng, A, start=True, stop=True)
            z2s = small.tile([K, K], bf16, tag="z2")
            nc.scalar.copy(z2s, z2)
            ps_yt = psB.tile([K, H], f32, tag="big")
            nc.tensor.matmul(ps_yt, A, y0t, start=True, stop=True)
            g = psA.tile([K, K], f32, tag="kk")
            nc.tensor.matmul(g, A, z2s, start=True, stop=True)
            gm = small.tile([K, K], bf16, tag="gm")
            nc.vector.tensor_mul(gm, g, maskSL)
            ps_q0 = psB.tile([K, DS], f32, tag="big")
            nc.tensor.matmul(ps_q0, A, t0t[:, 0:DS], start=True, stop=True)
            b1 = psA.tile([K, K], f32, tag="kk")
            nc.tensor.matmul(b1, gm, Bprev, start=True, stop=True)
            ps_q1 = psB.tile([K, DS], f32, tag="big")
            nc.tensor.matmul(ps_q1, A, t0t[:, DS:D], start=True, stop=True)
            nc.vector.tensor_add(Bm, b1, ident)

            # ================= EPILOGUE (critical) =================
            yt = small.tile([K, H], bf16, tag="yt")
            nc.vector.tensor_copy(yt, ps_yt)
            q = small.tile([K, D], bf16, tag="q")
            nc.vector.tensor_sub(q[:, 0:DS], XTall[:, b, 0:DS], ps_q0)
            nc.vector.tensor_sub(q[:, DS:D], XTall[:, b, DS:D], ps_q1)
            ut = small.tile([K, D], bf16, tag="ut")
            for s in range(2):
                ps = psB.tile([K, DS], f32, tag="big")
                nc.tensor.matmul(
                    ps, Bm, q[:, s * DS:(s + 1) * DS], start=True, stop=True
                )
                nc.scalar.copy(ut[:, s * DS:(s + 1) * DS], ps)
            if b + 1 < NBLK:
                U = small.tile([DP, DC * K], bf16, tag="U")
                for p in range(3):
                    tp = psB.tile([DP, 2 * K], f32, tag="big")
                    nc.tensor.matmul(
                        tp[:, 0:K], q[:, (2 * p) * DP:(2 * p + 1) * DP], Bm,
                        start=True, stop=True,
                    )
                    nc.tensor.matmul(
                        tp[:, K:2 * K], q[:, (2 * p + 1) * DP:(2 * p + 2) * DP],
                        Bm, start=True, stop=True,
                    )
                    nc.scalar.copy(U[:, (2 * p) * K:(2 * p + 2) * K], tp)
                tp = psB.tile([DP, K], f32, tag="big")
                nc.tensor.matmul(
                    tp, q[:, 6 * DP:7 * DP], Bm, start=True, stop=True
                )
                nc.scalar.copy(U[:, 6 * K:7 * K], tp)
                psc = psA.tile([K, K], f32, tag="kk")
                for ic in range(DC):
                    nc.tensor.matmul(
                        psc, U[:, ic * K:(ic + 1) * K], Xall[:, b + 1, ic, :],
                        start=(ic == 0), stop=(ic == DC - 1),
                    )
                clr = small.tile([K, K], bf16, tag="clr")
                nc.scalar.mul(clr, psc, LR)
                y0q_next = small.tile([HP, HC * K], bf16, tag="y0")
                for hc in range(HC):
                    cps = psA.tile([K, K], f32, tag="kk")
                    nc.tensor.matmul(
                        cps, yt[:, hc * HP:(hc + 1) * HP], clr,
                        start=True, stop=True,
                    )
                    nc.vector.tensor_add(
                        y0q_next[:, hc * K:(hc + 1) * K], cps,
                        Psb[b + 1][:, hc * K:(hc + 1) * K],
                    )

            # ================= FILL TAIL =================
            tc.cur_priority = pri_fill(b)
            relu_y = small.tile([HP, HC * K], bf16, tag="ry")
            for hc in range(HC):
                ps2 = psA.tile([K, K], f32, tag="kk")
                nc.tensor.matmul(
                    ps2, y0t[:, hc * HP:(hc + 1) * HP], A, start=True, stop=True
                )
                nc.scalar.activation(
                    relu_y[:, hc * K:(hc + 1) * K], ps2,
                    mybir.ActivationFunctionType.Relu,
                )
            lg = psA.tile([K, O], f32, tag="kk")
            for hc in range(HC):
                nc.tensor.matmul(
                    lg, relu_y[:, hc * K:(hc + 1) * K], RT[:, hc, :],
                    start=(hc == 0), stop=(hc == HC - 1),
                )
            lgs = small.tile([K, O], f32, tag="lgs")
            tc.cur_priority = pri_out(b)
            nc.vector.tensor_add(lgs, lg, bb)
            nc.sync.dma_start(out=out_d[b * K:(b + 1) * K, :], in_=lgs)
            tc.cur_priority = pri_fill(b)

            if b + LOOKAHEAD < NBLK:
                tc.cur_priority = pri_whit(b + LOOKAHEAD)
                whiten(b + LOOKAHEAD)
                tc.cur_priority = pri_fill(b)
            if b + 1 < NBLK:
                # masters: W += lr * Y U^T (fp32, in place); Wb = cast(W);
                # WTb = PE-transpose of Wb. Deferred: emitted inside the NEXT
                # block's ring so engine streams order it behind that block's
                # critical head.
                def make_tail(b, yt, ut):
                    def tail():
                        tc.cur_priority = pri_fill(b)
                        for hc in range(HC):
                            for s in range(2):
                                ps = psB.tile([HP, DS], f32, tag="big")
                                nc.tensor.matmul(
                                    ps, yt[:, hc * HP:(hc + 1) * HP],
                                    ut[:, s * DS:(s + 1) * DS],
                                    start=True, stop=True,
                                )
                                wsl = W[:, hc * D + s * DS:hc * D + (s + 1) * DS]
                                nc.vector.scalar_tensor_tensor(
                                    wsl, ps, LR, wsl, op0=AT.mult, op1=AT.add
                                )
                                nc.scalar.copy(
                                    Wb[:, hc * D + s * DS:hc * D + (s + 1) * DS],
                                    wsl,
                                )
                        for dc in range(DC):
                            tp = psT.tile([DP, 2 * K], bf16, tag="tt")
                            nc.tensor.transpose(
                                tp[:, 0:HP],
                                Wb[:, 0 * D + dc * DP:0 * D + (dc + 1) * DP],
                                identb,
                            )
                            nc.tensor.transpose(
                                tp[:, HP:2 * HP],
                                Wb[:, 1 * D + dc * DP:1 * D + (dc + 1) * DP],
                                identb,
                            )
                            nc.scalar.copy(WTb[:, dc * H:(dc + 1) * H], tp)
                        if b + 2 < NBLK:
                            open_P(b + 2)
                    return tail

                pending_tail[0] = make_tail(b, yt, ut)

    _split_multiwait(nc)
    return nc


def prep_inputs(x, whiten_mean, whiten_mat, oja_W, readout_W, readout_b):
    """Host-side layout/dtype prep (no contractions)."""
    x = np.ascontiguousarray(x, dtype=np.float32)
    mu_b = np.broadcast_to(
        np.asarray(whiten_mean, dtype=np.float32)[None, :], (128, D)
    ).copy()
    P = np.asarray(whiten_mat, dtype=np.float32) - np.eye(D, dtype=np.float32)
    # pt[dp, ic, dout] = P^T[ic*112+dp, dout] = P[dout, ic*112+dp]
    pt = np.ascontiguousarray(
        P.T.reshape(DC, DP, D).transpose(1, 0, 2).astype(ml_dtypes.bfloat16)
    )
    Wf = np.asarray(oja_W, dtype=np.float32)
    w = np.ascontiguousarray(
        Wf.reshape(HC, HP, D).transpose(1, 0, 2).reshape(HP, HC * D)
    )
    wtb = np.ascontiguousarray(
        Wf.T.reshape(DC, DP, H).transpose(1, 0, 2).reshape(DP, DC * H)
    ).astype(ml_dtypes.bfloat16)
    Rf = np.asarray(readout_W, dtype=np.float32)
    rt = np.ascontiguousarray(
        Rf.T.reshape(HC, HP, O).transpose(1, 0, 2).astype(ml_dtypes.bfloat16)
    )
    b_b = np.broadcast_to(
        np.asarray(readout_b, dtype=np.float32)[None, :], (128, O)
    ).copy()
    return {
        "x": x, "mu_b": mu_b, "pt": pt, "w": w, "rt": rt, "b_b": b_b,
        "w_bf": w.astype(ml_dtypes.bfloat16), "wt_bf": wtb,
    }


_cached_nc = None


def _get_nc():
    global _cached_nc
    if _cached_nc is None:
        _cached_nc = build_nc()
    return _cached_nc


def kernel(x, whiten_mean, whiten_mat, oja_W, readout_W, readout_b, **run_kwargs):
    nc = _get_nc()
    ins = prep_inputs(x, whiten_mean, whiten_mat, oja_W, readout_W, readout_b)
    res = run_bass_kernel_spmd(
        nc, [ins] * N_CORES, core_ids=list(range(N_CORES)), **run_kwargs
    )
    out = res.results[0]["out"]
    if run_kwargs:
        kernel.last_result = res
    return out


# revision 23
# speedup vs baseline: 1.0273x; 1.0039x over previous
"""Trainium2 Bass kernel for nn_BioClassifier: whitening + sequential Oja scan + readout.

v2: restructured for critical-path latency. Same block-parallel-scan math as v1
(chunk the 2048-sample Oja scan into 16 blocks of K=128; per block a fixed-point
"ring" on K x K matrices closes the sequential recurrence exactly):
    Y = Y0 A,  U = (X - T0 A) B,   A = (I - lr*SU(C))^-1, B = (I + lr*SU(G))^-1
    C = U^T X, G = Y^T Y,  T0^T X = Syy
Key v2 changes vs v1:
  * lr folded into sxx_lr/syy_lr (bf16), iteration reordered so each ring cycle
    is 10 serial engine-hops (B-chain: z2->z2s->g->gm->b1->B; A-chain: ct->nt->
    a1->A) with r1/s prep hidden under the B-chain; iter-0 A-update is 3 vec ops
    (A1 = I + SU(s0), s0 = sxx_lr - syy_lr), no matmuls.
  * Y0 correction form: P_{n} = W^{(n-1)} X_n accumulates in an OPEN PSUM group
    during ring_{n-1} (off critical path); epilogue closes it with the rank-K
    correction  Y0_n = P_n + lr * Y * (U^T X_n), so the master-W update and the
    14 Y0 matmuls leave the serial path entirely.
  * single fp32 master W [H,D]; Wb = cast(W) on scalar; WTb = PE-transpose of Wb
    (bf16 transpose == transpose of bf16 cast, exact) - drops the WT fp32 master
    and its vector-engine update entirely.
  * XTall/Sxxall stored bf16 (Sxx pre-scaled by lr at whiten time).
  * engine rebalance: ring bounces on DVE, z2s/copies on scalar, xc-sub and
    the iter-0 mask-mul on gpsimd (which cannot touch PSUM); whitening +
    masters + P-opens emitted in priority bands (and the master-update tail
    emitted mid-ring of the NEXT block) so the in-order engine streams place
    them behind each block's critical chain.
  * RING_ITERS=4 (validated offline: rel err 1.49e-2 vs the 2e-2 gate;
    RING_ITERS=5 gives 8.7e-3 at ~+60us).
All 8 cores run the identical program (the scan is inherently sequential;
core 0's output is returned).
"""

import os
import sys
from contextlib import ExitStack

sys.path.insert(0, "/opt/trn_rl_repo")

import numpy as np
import ml_dtypes

import concourse.bass as bass
import concourse.mybir as mybir
from concourse.tile import TileContext
from concourse.masks import make_identity
from concourse.bass_utils import run_bass_kernel_spmd
from concourse.vector_clock import ScopedClock

LR = 1e-3
B, D, H, O = 2048, 784, 256, 10
K = 128
NBLK = B // K
DP, DC = 112, 7          # D = 784 = 7 * 112
HP, HC = 128, 2          # H = 256 = 2 * 128
DS = D // 2              # 392: matmul free-dim split for D-wide outputs

RING_ITERS = int(os.environ.get("RING_ITERS", "4"))
LOOKAHEAD = 4
N_CORES = 8

f32 = mybir.dt.float32
bf16 = mybir.dt.bfloat16
AT = mybir.AluOpType


def _install_ntff_hook():
    """The agent image's `antenv` lacks `axon_hooks`, so trace=True degrades.
    Synthesize the module and register the ctypes NTFF hook from trn_boot."""
    import types
    import antenv

    if getattr(antenv, "axon_hooks", None) is not None:
        return
    mod = types.ModuleType("antenv.axon_hooks")
    _hook_box = [None]
    mod.set_axon_ntff_profile_hook = lambda h: _hook_box.__setitem__(0, h)
    mod.get_axon_ntff_profile_hook = lambda: _hook_box[0]
    sys.modules["antenv.axon_hooks"] = mod
    antenv.axon_hooks = mod
    try:
        sys.path.insert(0, "/root/.axon_site")
        from trn_agent_boot.trn_boot import _ntff_profile_via_ctypes

        hook = _ntff_profile_via_ctypes("/opt/axon/libaxon_pjrt.so")
        if hook is not None:
            mod.set_axon_ntff_profile_hook(hook)
    except Exception:
        pass


try:
    _install_ntff_hook()
except Exception:
    pass

_drain_patched = False


def _patch_drain():
    """This walrus build only supports one sync-wait per CTRL instruction;
    split the Tile kernel-tail drain into one drain per semaphore wait."""
    global _drain_patched
    if _drain_patched:
        return

    def patched(self, tick_clock, wait_clock):
        drain_inst = self.nc.sync.drain()
        wait_clock.add_sem_waits(
            drain_inst.ins, ScopedClock({None: tick_clock.global_clock})
        )
        mi = drain_inst.ins
        si = mi.sync_info
        if si is not None and len(si.on_wait) > 1:
            waits = list(si.on_wait)
            mi.sync_info = mybir.SyncInfo(
                on_wait=[waits[0]], on_update=list(si.on_update)
            )
            for w in waits[1:]:
                d2 = self.nc.sync.drain()
                d2.ins.sync_info = mybir.SyncInfo(on_wait=[w], on_update=[])
        self.nc.all_engine_barrier()
        assert self.sems is not None
        popped = self.nc._tile_sem_poison_stack.pop()
        assert popped is self._sem_poison
        self.nc.clear_and_free_semaphores(list(self.sems.allocated().values()))
        self.nc.all_engine_barrier()

    TileContext._drain_and_barrier = patched
    _drain_patched = True


def _split_multiwait(nc, limit=1):
    """This walrus build supports only `limit` sync-waits per instruction.
    Hoist extra waits onto NoOps inserted just before, in the same engine
    stream (engines are in-order, so earlier waits are strictly safe)."""
    n_split = 0
    for f in nc.m.functions:
        for bb in f.blocks:
            insts = list(bb.instructions)
            if not any(
                i.sync_info is not None and len(i.sync_info.on_wait) > limit
                for i in insts
            ):
                continue
            new = []
            for inst in insts:
                si = inst.sync_info
                if si is not None and len(si.on_wait) > limit:
                    waits = list(si.on_wait)
                    for j, w in enumerate(waits[: len(waits) - limit]):
                        nop = mybir.InstNoOp(
                            name=f"{inst.name}-hw{j}", engine=inst.engine,
                            ins=[], outs=[],
                        )
                        nop.sync_info = mybir.SyncInfo(on_wait=[w], on_update=[])
                        new.append(nop)
                        n_split += 1
                    inst.sync_info = mybir.SyncInfo(
                        on_wait=waits[len(waits) - limit:],
                        on_update=list(si.on_update),
                    )
                new.append(inst)
            bb.instructions = new
    return n_split


def build_nc(ring_iters=RING_ITERS):
    _patch_drain()
    nc = bass.Bass()
    x_d = nc.dram_tensor("x", [B, D], f32, kind="ExternalInput")
    mu_d = nc.dram_tensor("mu_b", [128, D], f32, kind="ExternalInput")
    pt_d = nc.dram_tensor("pt", [DP, DC, D], bf16, kind="ExternalInput")
    w_d = nc.dram_tensor("w", [HP, HC * D], f32, kind="ExternalInput")
    wb_d = nc.dram_tensor("w_bf", [HP, HC * D], bf16, kind="ExternalInput")
    wtb_d = nc.dram_tensor("wt_bf", [DP, DC * H], bf16, kind="ExternalInput")
    rt_d = nc.dram_tensor("rt", [HP, HC, O], bf16, kind="ExternalInput")
    bb_d = nc.dram_tensor("b_b", [128, O], f32, kind="ExternalInput")
    out_d = nc.dram_tensor("out", [B, O], f32, kind="ExternalOutput")
    def pri_crit(b):
        # critical path of block b
        return 1000 + b * 1000

    def pri_fill(b):
        # fill work of block b: must rank BELOW crit of b+1 (it runs during
        # ring_{b+1}) but above crit of b+2
        return 1000 + (b + 1) * 1000 + 500

    def pri_whit(b):
        # whiten(b) must complete before epilogue of b-1: rank just below
        # crit(b-1), above fill bands of earlier blocks
        return 1000 + (b - 1) * 1000 + 400

    def pri_out(b):
        return 20_000_000 + b * 1000

    with TileContext(nc) as tc, ExitStack() as ctx:
        persist = ctx.enter_context(tc.tile_pool(name="persist", bufs=1))
        xpool = ctx.enter_context(tc.tile_pool(name="xpool", bufs=5))
        small = ctx.enter_context(tc.tile_pool(name="small", bufs=2))
        psA = ctx.enter_context(tc.tile_pool(name="psA", bufs=3, space="PSUM"))
        psB = ctx.enter_context(tc.tile_pool(name="psB", bufs=3, space="PSUM"))
        psT = ctx.enter_context(tc.tile_pool(name="psT", bufs=2, space="PSUM"))

        ident = persist.tile([128, 128], f32, tag="ident")
        make_identity(nc, ident)
        identb = persist.tile([128, 128], bf16, tag="identb")
        nc.vector.tensor_copy(identb, ident)
        # 0/1 masks (lr is folded into sxx_lr / syy_lr)
        maskSL = persist.tile([K, K], f32, tag="maskSL")
        nc.gpsimd.memset(maskSL, 1.0)
        nc.gpsimd.affine_select(
            out=maskSL, in_=maskSL, compare_op=AT.is_gt, fill=0.0,
            base=0, pattern=[[-1, K]], channel_multiplier=1,
        )
        maskSU = persist.tile([K, K], f32, tag="maskSU")
        nc.gpsimd.memset(maskSU, 1.0)
        nc.vector.tensor_sub(maskSU, maskSU, ident)
        nc.vector.tensor_sub(maskSU, maskSU, maskSL)

        mu_t = persist.tile([128, D], f32, tag="mu")
        nc.sync.dma_start(out=mu_t, in_=mu_d[:, :])
        pt_t = persist.tile([DP, DC, D], bf16, tag="pt")
        nc.sync.dma_start(out=pt_t, in_=pt_d[:, :, :])
        WTb = persist.tile([DP, DC * H], bf16, tag="WTb")
        nc.sync.dma_start(out=WTb, in_=wtb_d[:, :])
        Wb = persist.tile([HP, HC * D], bf16, tag="Wb")
        nc.sync.dma_start(out=Wb, in_=wb_d[:, :])
        W = persist.tile([HP, HC * D], f32, tag="W")
        nc.sync.dma_start(out=W, in_=w_d[:, :])
        RT = persist.tile([HP, HC, O], bf16, tag="RT")
        nc.sync.dma_start(out=RT, in_=rt_d[:, :, :])
        bb = persist.tile([128, O], f32, tag="bb")
        nc.sync.dma_start(out=bb, in_=bb_d[:, :])

        Xall = persist.tile([DP, NBLK, DC, K], bf16, tag="Xall")
        XTall = persist.tile([K, NBLK, D], bf16, tag="XTall")
        Sxxall = persist.tile([K, NBLK, K], bf16, tag="Sxxall")

        # ---------------- whitening ----------------
        def whiten(bi):
            xt = xpool.tile([128, D], f32, tag="xraw")
            nc.sync.dma_start(out=xt, in_=x_d[bi * K:(bi + 1) * K, :])
            xc = xpool.tile([128, D], f32, tag="xc")
            eng = nc.vector if bi == 0 else nc.gpsimd
            eng.tensor_sub(xc, xt, mu_t)
            xcb = xpool.tile([128, D], bf16, tag="xcb")
            nc.vector.tensor_copy(xcb, xc)
            xct = xpool.tile([DP, DC * K], bf16, tag="xct")
            for p in range(3):          # paired transposes -> one copy per pair
                tp = psT.tile([DP, 2 * K], bf16, tag="tt")
                nc.tensor.transpose(
                    tp[:, 0:K], xcb[:, (2 * p) * DP:(2 * p + 1) * DP], identb
                )
                nc.tensor.transpose(
                    tp[:, K:2 * K], xcb[:, (2 * p + 1) * DP:(2 * p + 2) * DP],
                    identb,
                )
                nc.scalar.copy(xct[:, (2 * p) * K:(2 * p + 2) * K], tp)
            tp = psT.tile([DP, K], bf16, tag="tt")
            nc.tensor.transpose(tp, xcb[:, 6 * DP:7 * DP], identb)
            nc.scalar.copy(xct[:, 6 * K:7 * K], tp)
            XTb = XTall[:, bi, :]
            for s in range(2):
                ps = psB.tile([K, DS], f32, tag="big")
                for ic in range(DC):
                    nc.tensor.matmul(
                        ps, xct[:, ic * K:(ic + 1) * K],
                        pt_t[:, ic, s * DS:(s + 1) * DS],
                        start=(ic == 0), stop=(ic == DC - 1),
                    )
                nc.vector.tensor_add(
                    XTb[:, s * DS:(s + 1) * DS], ps, xc[:, s * DS:(s + 1) * DS]
                )
            xa = Xall[:, bi, :, :]
            for p in range(3):
                tp = psT.tile([DP, 2 * K], bf16, tag="tt")
                nc.tensor.transpose(
                    tp[:, 0:K], XTb[:, (2 * p) * DP:(2 * p + 1) * DP], identb
                )
                nc.tensor.transpose(
                    tp[:, K:2 * K], XTb[:, (2 * p + 1) * DP:(2 * p + 2) * DP],
                    identb,
                )
                nc.scalar.copy(xa[:, 2 * p:2 * p + 2, :], tp)
            tp = psT.tile([DP, K], bf16, tag="tt")
            nc.tensor.transpose(tp, XTb[:, 6 * DP:7 * DP], identb)
            nc.scalar.copy(xa[:, 6, :], tp)
            ps = psA.tile([K, K], f32, tag="kk")
            for ic in range(DC):
                nc.tensor.matmul(
                    ps, xa[:, ic, :], xa[:, ic, :],
                    start=(ic == 0), stop=(ic == DC - 1),
                )
            nc.scalar.mul(Sxxall[:, bi, :], ps, LR)

        tc.cur_priority = 0
        whiten(0)
        tc.cur_priority = pri_whit(1)
        whiten(1)
        tc.cur_priority = pri_whit(2)
        whiten(2)
        tc.cur_priority = pri_whit(3)
        whiten(3)

        Psb = {}  # block -> SBUF f32 tile [HP, HC*K] holding W^(stale) X_block

        def open_P(nb):
            """Accumulate P_nb = W^(current) X_nb into an SBUF f32 tile."""
            pt_sb = small.tile([HP, HC * K], f32, tag="Psb")
            for hc in range(HC):
                ps = psB.tile([HP, K], f32, tag="big")
                for ic in range(DC):
                    nc.tensor.matmul(
                        ps,
                        WTb[:, ic * H + hc * HP:ic * H + (hc + 1) * HP],
                        Xall[:, nb, ic, :],
                        start=(ic == 0), stop=(ic == DC - 1),
                    )
                nc.scalar.copy(pt_sb[:, hc * K:(hc + 1) * K], ps)
            Psb[nb] = pt_sb

        # P_1 with the initial weights (correction applied in epilogue of block 0).
        # Priority BELOW crit(0): open_P(1) waits on whiten(1), and at higher
        # priority it would head-of-line-block block 0's PE stream.
        tc.cur_priority = 1450
        open_P(1)
        y0q_next = None
        pending_tail = [None]

        for b in range(NBLK):
            # ================= HEAD (critical) =================
            tc.cur_priority = pri_crit(b)
            if b == 0:
                y0q = small.tile([HP, HC * K], bf16, tag="y0")
                for hc in range(HC):
                    ps = psB.tile([HP, K], f32, tag="big")
                    for ic in range(DC):
                        nc.tensor.matmul(
                            ps,
                            WTb[:, ic * H + hc * HP:ic * H + (hc + 1) * HP],
                            Xall[:, 0, ic, :],
                            start=(ic == 0), stop=(ic == DC - 1),
                        )
                    nc.vector.tensor_copy(y0q[:, hc * K:(hc + 1) * K], ps)
            else:
                y0q = y0q_next
            ps_syy = psA.tile([K, K], f32, tag="kk")
            for hc in range(HC):
                nc.tensor.matmul(
                    ps_syy, y0q[:, hc * K:(hc + 1) * K],
                    y0q[:, hc * K:(hc + 1) * K],
                    start=(hc == 0), stop=(hc == HC - 1),
                )
            syy_lr = small.tile([K, K], bf16, tag="syl")
            nc.scalar.mul(syy_lr, ps_syy, LR)
            syy_ng = small.tile([K, K], bf16, tag="syn")
            nc.vector.tensor_scalar_mul(syy_ng, ps_syy, -LR)

            # ---- head fill (off critical path) ----
            tc.cur_priority = pri_fill(b)
            y0t = small.tile([K, H], bf16, tag="y0t")
            for hc in range(HC):
                tp = psT.tile([128, K], bf16, tag="tt")
                nc.tensor.transpose(tp, y0q[:, hc * K:(hc + 1) * K], identb)
                nc.scalar.copy(y0t[:, hc * HP:(hc + 1) * HP], tp)
            # ================= RING (critical) =================
            tc.cur_priority = pri_crit(b)
            sxx = Sxxall[:, b, :]
            s_sb = small.tile([K, K], bf16, tag="s")
            nc.vector.scalar_tensor_tensor(
                s_sb, ps_syy, -LR, sxx, op0=AT.mult, op1=AT.add
            )
            tA0 = small.tile([K, K], bf16, tag="ta")
            nc.gpsimd.tensor_mul(tA0, s_sb, maskSU)
            A = small.tile([K, K], bf16, tag="A")
            nc.vector.tensor_add(A, tA0, ident)
            Bm = small.tile([K, K], bf16, tag="B")
            Bprev = identb
            for m in range(1, ring_iters):
                if m == 2:
                    # emit the previous block's master-update chain here so the
                    # scheduler places it after this block's head/early ring in
                    # every engine stream; t0t must follow it (reads updated Wb)
                    if pending_tail[0] is not None:
                        pending_tail[0]()
                        pending_tail[0] = None
                    tc.cur_priority = pri_fill(b - 1) if b else pri_fill(0)
                    if b + 1 < NBLK:
                        t0t = small.tile([K, D], bf16, tag="t0t")
                        for s in range(2):
                            ps = psB.tile([K, DS], f32, tag="big")
                            for hc in range(HC):
                                nc.tensor.matmul(
                                    ps, y0q[:, hc * K:(hc + 1) * K],
                                    Wb[:, hc * D + s * DS:hc * D + (s + 1) * DS],
                                    start=(hc == 0), stop=(hc == HC - 1),
                                )
                            nc.vector.tensor_copy(
                                t0t[:, s * DS:(s + 1) * DS], ps
                            )
                    tc.cur_priority = pri_crit(b)
                z2 = psA.tile([K, K], f32, tag="kk")
                nc.tensor.matmul(z2, syy_ng, A, start=True, stop=True)
                z2s = small.tile([K, K], bf16, tag="z2")
                nc.scalar.copy(z2s, z2)
                r1 = psA.tile([K, K], f32, tag="kk")
                nc.tensor.matmul(r1, A, syy_lr, start=True, stop=True)
                s_sb = small.tile([K, K], bf16, tag="s")
                nc.vector.tensor_sub(s_sb, sxx, r1)
                g = psA.tile([K, K], f32, tag="kk")
                nc.tensor.matmul(g, A, z2s, start=True, stop=True)
                gm = small.tile([K, K], bf16, tag="gm")
                nc.vector.tensor_mul(gm, g, maskSL)
                b1 = psA.tile([K, K], f32, tag="kk")
                nc.tensor.matmul(b1, gm, Bprev, start=True, stop=True)
                nc.vector.tensor_add(Bm, b1, ident)
                Bprev = Bm
                ct = psA.tile([K, K], f32, tag="kk")
                nc.tensor.matmul(ct, s_sb, Bm, start=True, stop=True)
                nt = small.tile([K, K], bf16, tag="nt")
                nc.vector.tensor_mul(nt, ct, maskSL)
                a1 = psA.tile([K, K], f32, tag="kk")
                nc.tensor.matmul(a1, nt, A, start=True, stop=True)
                nc.vector.tensor_add(A, a1, ident)
            # final B-update (B_R from A_R), epilogue A-work interleaved to
            # fill PE gaps while z2s/gm bounce on scalar/vector.
            # The last block needs only its logits: skip B_R/yt/q/ut entirely.
            if b + 1 < NBLK:
                z2 = psA.tile([K, K], f32, tag="kk")
                nc.tensor.matmul(z2, syy_ng, A, start=True, stop=True)
                z2s = small.tile([K, K], bf16, tag="z2")
                nc.scalar.copy(z2s, z2)
                ps_yt = psB.tile([K, H], f32, tag="big")
                nc.tensor.matmul(ps_yt, A, y0t, start=True, stop=True)
                g = psA.tile([K, K], f32, tag="kk")
                nc.tensor.matmul(g, A, z2s, start=True, stop=True)
                gm = small.tile([K, K], bf16, tag="gm")
                nc.vector.tensor_mul(gm, g, maskSL)
                ps_q0 = psB.tile([K, DS], f32, tag="big")
                nc.tensor.matmul(ps_q0, A, t0t[:, 0:DS], start=True, stop=True)
                b1 = psA.tile([K, K], f32, tag="kk")
                nc.tensor.matmul(b1, gm, Bprev, start=True, stop=True)
                ps_q1 = psB.tile([K, DS], f32, tag="big")
                nc.tensor.matmul(ps_q1, A, t0t[:, DS:D], start=True, stop=True)
                nc.vector.tensor_add(Bm, b1, ident)

                # ================= EPILOGUE (critical) =================
                yt = small.tile([K, H], bf16, tag="yt")
                nc.vector.tensor_copy(yt, ps_yt)
                q = small.tile([K, D], bf16, tag="q")
                nc.vector.tensor_sub(q[:, 0:DS], XTall[:, b, 0:DS], ps_q0)
                nc.vector.tensor_sub(q[:, DS:D], XTall[:, b, DS:D], ps_q1)
                ut = small.tile([K, D], bf16, tag="ut")
                for s in range(2):
                    ps = psB.tile([K, DS], f32, tag="big")
                    nc.tensor.matmul(
                        ps, Bm, q[:, s * DS:(s + 1) * DS], start=True, stop=True
                    )
                    nc.scalar.copy(ut[:, s * DS:(s + 1) * DS], ps)
                U = small.tile([DP, DC * K], bf16, tag="U")
                for p in range(3):
                    tp = psB.tile([DP, 2 * K], f32, tag="big")
                    nc.tensor.matmul(
                        tp[:, 0:K], q[:, (2 * p) * DP:(2 * p + 1) * DP], Bm,
                        start=True, stop=True,
                    )
                    nc.tensor.matmul(
                        tp[:, K:2 * K], q[:, (2 * p + 1) * DP:(2 * p + 2) * DP],
                        Bm, start=True, stop=True,
                    )
                    nc.scalar.copy(U[:, (2 * p) * K:(2 * p + 2) * K], tp)
                tp = psB.tile([DP, K], f32, tag="big")
                nc.tensor.matmul(
                    tp, q[:, 6 * DP:7 * DP], Bm, start=True, stop=True
                )
                nc.scalar.copy(U[:, 6 * K:7 * K], tp)
                psc = psA.tile([K, K], f32, tag="kk")
                for ic in range(DC):
                    nc.tensor.matmul(
                        psc, U[:, ic * K:(ic + 1) * K], Xall[:, b + 1, ic, :],
                        start=(ic == 0), stop=(ic == DC - 1),
                    )
                clr = small.tile([K, K], bf16, tag="clr")
                nc.scalar.mul(clr, psc, LR)
                y0q_next = small.tile([HP, HC * K], bf16, tag="y0")
                for hc in range(HC):
                    cps = psA.tile([K, K], f32, tag="kk")
                    nc.tensor.matmul(
                        cps, yt[:, hc * HP:(hc + 1) * HP], clr,
                        start=True, stop=True,
                    )
                    nc.vector.tensor_add(
                        y0q_next[:, hc * K:(hc + 1) * K], cps,
                        Psb[b + 1][:, hc * K:(hc + 1) * K],
                    )

            # ================= FILL TAIL =================
            tc.cur_priority = pri_fill(b)
            relu_y = small.tile([HP, HC * K], bf16, tag="ry")
            for hc in range(HC):
                ps2 = psA.tile([K, K], f32, tag="kk")
                nc.tensor.matmul(
                    ps2, y0t[:, hc * HP:(hc + 1) * HP], A, start=True, stop=True
                )
                nc.scalar.activation(
                    relu_y[:, hc * K:(hc + 1) * K], ps2,
                    mybir.ActivationFunctionType.Relu,
                )
            lg = psA.tile([K, O], f32, tag="kk")
            for hc in range(HC):
                nc.tensor.matmul(
                    lg, relu_y[:, hc * K:(hc + 1) * K], RT[:, hc, :],
                    start=(hc == 0), stop=(hc == HC - 1),
                )
            lgs = small.tile([K, O], f32, tag="lgs")
            tc.cur_priority = pri_out(b)
            nc.vector.tensor_add(lgs, lg, bb)
            nc.sync.dma_start(out=out_d[b * K:(b + 1) * K, :], in_=lgs)
            tc.cur_priority = pri_fill(b)

            if b + LOOKAHEAD < NBLK:
                tc.cur_priority = pri_whit(b + LOOKAHEAD)
                whiten(b + LOOKAHEAD)
                tc.cur_priority = pri_fill(b)
            if b + 1 < NBLK:
                # masters: W += lr * Y U^T (fp32, in place); Wb = cast(W);
                # WTb = PE-transpose of Wb. Deferred: emitted inside the NEXT
                # block's ring so engine streams order it behind that block's
                # critical head.
                def make_tail(b, yt, ut):
                    def tail():
                        tc.cur_priority = pri_fill(b)
                        for hc in range(HC):
                            for s in range(2):
                                ps = psB.tile([HP, DS], f32, tag="big")
                                nc.tensor.matmul(
                                    ps, yt[:, hc * HP:(hc + 1) * HP],
                                    ut[:, s * DS:(s + 1) * DS],
                                    start=True, stop=True,
                                )
                                wsl = W[:, hc * D + s * DS:hc * D + (s + 1) * DS]
                                nc.vector.scalar_tensor_tensor(
                                    wsl, ps, LR, wsl, op0=AT.mult, op1=AT.add
                                )
                                nc.scalar.copy(
                                    Wb[:, hc * D + s * DS:hc * D + (s + 1) * DS],
                                    wsl,
                                )
                        for dc in range(DC):
                            tp = psT.tile([DP, 2 * K], bf16, tag="tt")
                            nc.tensor.transpose(
                                tp[:, 0:HP],
                                Wb[:, 0 * D + dc * DP:0 * D + (dc + 1) * DP],
                                identb,
                            )
                            nc.tensor.transpose(
                                tp[:, HP:2 * HP],
                                Wb[:, 1 * D + dc * DP:1 * D + (dc + 1) * DP],
                                identb,
                            )
                            nc.scalar.copy(WTb[:, dc * H:(dc + 1) * H], tp)
                        if b + 2 < NBLK:
                            open_P(b + 2)
                    return tail

                pending_tail[0] = make_tail(b, yt, ut)

    _split_multiwait(nc)
    return nc


def prep_inputs(x, whiten_mean, whiten_mat, oja_W, readout_W, readout_b):
    """Host-side layout/dtype prep (no contractions)."""
    x = np.ascontiguousarray(x, dtype=np.float32)
    mu_b = np.broadcast_to(
        np.asarray(whiten_mean, dtype=np.float32)[None, :], (128, D)
    ).copy()
    P = np.asarray(whiten_mat, dtype=np.float32) - np.eye(D, dtype=np.float32)
    # pt[dp, ic, dout] = P^T[ic*112+dp, dout] = P[dout, ic*112+dp]
    pt = np.ascontiguousarray(
        P.T.reshape(DC, DP, D).transpose(1, 0, 2).astype(ml_dtypes.bfloat16)
    )
    Wf = np.asarray(oja_W, dtype=np.float32)
    w = np.ascontiguousarray(
        Wf.reshape(HC, HP, D).transpose(1, 0, 2).reshape(HP, HC * D)
    )
    wtb = np.ascontiguousarray(
        Wf.T.reshape(DC, DP, H).transpose(1, 0, 2).reshape(DP, DC * H)
    ).astype(ml_dtypes.bfloat16)
    Rf = np.asarray(readout_W, dtype=np.float32)
    rt = np.ascontiguousarray(
        Rf.T.reshape(HC, HP, O).transpose(1, 0, 2).astype(ml_dtypes.bfloat16)
    )
    b_b = np.broadcast_to(
        np.asarray(readout_b, dtype=np.float32)[None, :], (128, O)
    ).copy()
    return {
        "x": x, "mu_b": mu_b, "pt": pt, "w": w, "rt": rt, "b_b": b_b,
        "w_bf": w.astype(ml_dtypes.bfloat16), "wt_bf": wtb,
    }


_cached_nc = None


def _get_nc():
    global _cached_nc
    if _cached_nc is None:
        _cached_nc = build_nc()
    return _cached_nc


def kernel(x, whiten_mean, whiten_mat, oja_W, readout_W, readout_b, **run_kwargs):
    nc = _get_nc()
    ins = prep_inputs(x, whiten_mean, whiten_mat, oja_W, readout_W, readout_b)
    res = run_bass_kernel_spmd(
        nc, [ins] * N_CORES, core_ids=list(range(N_CORES)), **run_kwargs
    )
    out = res.results[0]["out"]
    if run_kwargs:
        kernel.last_result = res
    return out


# revision 24
# speedup vs baseline: 1.0623x; 1.0341x over previous
"""Trainium2 Bass kernel for nn_BioClassifier: whitening + sequential Oja scan + readout.

v2: restructured for critical-path latency. Same block-parallel-scan math as v1
(chunk the 2048-sample Oja scan into 16 blocks of K=128; per block a fixed-point
"ring" on K x K matrices closes the sequential recurrence exactly):
    Y = Y0 A,  U = (X - T0 A) B,   A = (I - lr*SU(C))^-1, B = (I + lr*SU(G))^-1
    C = U^T X, G = Y^T Y,  T0^T X = Syy
Key v2 changes vs v1:
  * lr folded into sxx_lr/syy_lr (bf16), iteration reordered so each ring cycle
    is 10 serial engine-hops (B-chain: z2->z2s->g->gm->b1->B; A-chain: ct->nt->
    a1->A) with r1/s prep hidden under the B-chain; iter-0 A-update is 3 vec ops
    (A1 = I + SU(s0), s0 = sxx_lr - syy_lr), no matmuls.
  * Y0 correction form: P_{n} = W^{(n-1)} X_n accumulates in an OPEN PSUM group
    during ring_{n-1} (off critical path); epilogue closes it with the rank-K
    correction  Y0_n = P_n + lr * Y * (U^T X_n), so the master-W update and the
    14 Y0 matmuls leave the serial path entirely.
  * single fp32 master W [H,D]; Wb = cast(W) on scalar; WTb = PE-transpose of Wb
    (bf16 transpose == transpose of bf16 cast, exact) - drops the WT fp32 master
    and its vector-engine update entirely.
  * XTall/Sxxall stored bf16 (Sxx pre-scaled by lr at whiten time).
  * engine rebalance: ring bounces on DVE, z2s/copies on scalar, xc-sub and
    the iter-0 mask-mul on gpsimd (which cannot touch PSUM); whitening +
    masters + P-opens emitted in priority bands (and the master-update tail
    emitted mid-ring of the NEXT block) so the in-order engine streams place
    them behind each block's critical chain.
  * RING_ITERS=4 (validated offline: rel err 1.49e-2 vs the 2e-2 gate;
    RING_ITERS=5 gives 8.7e-3 at ~+60us).
All 8 cores run the identical program (the scan is inherently sequential;
core 0's output is returned).
"""

import os
import sys
from contextlib import ExitStack

sys.path.insert(0, "/opt/trn_rl_repo")

import numpy as np
import ml_dtypes

import concourse.bass as bass
import concourse.mybir as mybir
from concourse.tile import TileContext
from concourse.masks import make_identity
from concourse.bass_utils import run_bass_kernel_spmd
from concourse.vector_clock import ScopedClock

LR = 1e-3
B, D, H, O = 2048, 784, 256, 10
K = 128
NBLK = B // K
DP, DC = 112, 7          # D = 784 = 7 * 112
HP, HC = 128, 2          # H = 256 = 2 * 128
DS = D // 2              # 392: matmul free-dim split for D-wide outputs

RING_ITERS = int(os.environ.get("RING_ITERS", "4"))
LOOKAHEAD = 4
N_CORES = 8

f32 = mybir.dt.float32
bf16 = mybir.dt.bfloat16
AT = mybir.AluOpType


def _install_ntff_hook():
    """The agent image's `antenv` lacks `axon_hooks`, so trace=True degrades.
    Synthesize the module and register the ctypes NTFF hook from trn_boot."""
    import types
    import antenv

    if getattr(antenv, "axon_hooks", None) is not None:
        return
    mod = types.ModuleType("antenv.axon_hooks")
    _hook_box = [None]
    mod.set_axon_ntff_profile_hook = lambda h: _hook_box.__setitem__(0, h)
    mod.get_axon_ntff_profile_hook = lambda: _hook_box[0]
    sys.modules["antenv.axon_hooks"] = mod
    antenv.axon_hooks = mod
    try:
        sys.path.insert(0, "/root/.axon_site")
        from trn_agent_boot.trn_boot import _ntff_profile_via_ctypes

        hook = _ntff_profile_via_ctypes("/opt/axon/libaxon_pjrt.so")
        if hook is not None:
            mod.set_axon_ntff_profile_hook(hook)
    except Exception:
        pass


try:
    _install_ntff_hook()
except Exception:
    pass

_drain_patched = False


def _patch_drain():
    """This walrus build only supports one sync-wait per CTRL instruction;
    split the Tile kernel-tail drain into one drain per semaphore wait."""
    global _drain_patched
    if _drain_patched:
        return

    def patched(self, tick_clock, wait_clock):
        drain_inst = self.nc.sync.drain()
        wait_clock.add_sem_waits(
            drain_inst.ins, ScopedClock({None: tick_clock.global_clock})
        )
        mi = drain_inst.ins
        si = mi.sync_info
        if si is not None and len(si.on_wait) > 1:
            waits = list(si.on_wait)
            mi.sync_info = mybir.SyncInfo(
                on_wait=[waits[0]], on_update=list(si.on_update)
            )
            for w in waits[1:]:
                d2 = self.nc.sync.drain()
                d2.ins.sync_info = mybir.SyncInfo(on_wait=[w], on_update=[])
        self.nc.all_engine_barrier()
        assert self.sems is not None
        popped = self.nc._tile_sem_poison_stack.pop()
        assert popped is self._sem_poison
        self.nc.clear_and_free_semaphores(list(self.sems.allocated().values()))
        self.nc.all_engine_barrier()

    TileContext._drain_and_barrier = patched
    _drain_patched = True


def _split_multiwait(nc, limit=1):
    """This walrus build supports only `limit` sync-waits per instruction.
    Hoist extra waits onto NoOps inserted just before, in the same engine
    stream (engines are in-order, so earlier waits are strictly safe)."""
    n_split = 0
    for f in nc.m.functions:
        for bb in f.blocks:
            insts = list(bb.instructions)
            if not any(
                i.sync_info is not None and len(i.sync_info.on_wait) > limit
                for i in insts
            ):
                continue
            new = []
            for inst in insts:
                si = inst.sync_info
                if si is not None and len(si.on_wait) > limit:
                    waits = list(si.on_wait)
                    for j, w in enumerate(waits[: len(waits) - limit]):
                        nop = mybir.InstNoOp(
                            name=f"{inst.name}-hw{j}", engine=inst.engine,
                            ins=[], outs=[],
                        )
                        nop.sync_info = mybir.SyncInfo(on_wait=[w], on_update=[])
                        new.append(nop)
                        n_split += 1
                    inst.sync_info = mybir.SyncInfo(
                        on_wait=waits[len(waits) - limit:],
                        on_update=list(si.on_update),
                    )
                new.append(inst)
            bb.instructions = new
    return n_split


def build_nc(ring_iters=RING_ITERS):
    _patch_drain()
    nc = bass.Bass()
    x_d = nc.dram_tensor("x", [B, D], f32, kind="ExternalInput")
    mu_d = nc.dram_tensor("mu_b", [128, D], f32, kind="ExternalInput")
    pt_d = nc.dram_tensor("pt", [DP, DC, D], bf16, kind="ExternalInput")
    w_d = nc.dram_tensor("w", [HP, HC * D], f32, kind="ExternalInput")
    wb_d = nc.dram_tensor("w_bf", [HP, HC * D], bf16, kind="ExternalInput")
    wtb_d = nc.dram_tensor("wt_bf", [DP, DC * H], bf16, kind="ExternalInput")
    rt_d = nc.dram_tensor("rt", [HP, HC, O], bf16, kind="ExternalInput")
    bb_d = nc.dram_tensor("b_b", [128, O], f32, kind="ExternalInput")
    out_d = nc.dram_tensor("out", [B, O], f32, kind="ExternalOutput")
    def pri_crit(b):
        # critical path of block b
        return 1000 + b * 1000

    def pri_fill(b):
        # fill work of block b: must rank BELOW crit of b+1 (it runs during
        # ring_{b+1}) but above crit of b+2
        return 1000 + (b + 1) * 1000 + 500

    def pri_whit(b):
        # whiten(b) must complete before epilogue of b-1: rank just below
        # crit(b-1), above fill bands of earlier blocks
        return 1000 + (b - 1) * 1000 + 400

    def pri_out(b):
        return 20_000_000 + b * 1000

    with TileContext(nc) as tc, ExitStack() as ctx:
        persist = ctx.enter_context(tc.tile_pool(name="persist", bufs=1))
        xpool = ctx.enter_context(tc.tile_pool(name="xpool", bufs=5))
        small = ctx.enter_context(tc.tile_pool(name="small", bufs=2))
        psA = ctx.enter_context(tc.tile_pool(name="psA", bufs=3, space="PSUM"))
        psB = ctx.enter_context(tc.tile_pool(name="psB", bufs=3, space="PSUM"))
        psT = ctx.enter_context(tc.tile_pool(name="psT", bufs=2, space="PSUM"))

        ident = persist.tile([128, 128], f32, tag="ident")
        make_identity(nc, ident)
        identb = persist.tile([128, 128], bf16, tag="identb")
        nc.vector.tensor_copy(identb, ident)
        # 0/1 masks (lr is folded into sxx_lr / syy_lr)
        maskSL = persist.tile([K, K], f32, tag="maskSL")
        nc.gpsimd.memset(maskSL, 1.0)
        nc.gpsimd.affine_select(
            out=maskSL, in_=maskSL, compare_op=AT.is_gt, fill=0.0,
            base=0, pattern=[[-1, K]], channel_multiplier=1,
        )
        maskSU = persist.tile([K, K], f32, tag="maskSU")
        nc.gpsimd.memset(maskSU, 1.0)
        nc.vector.tensor_sub(maskSU, maskSU, ident)
        nc.vector.tensor_sub(maskSU, maskSU, maskSL)

        mu_t = persist.tile([128, D], f32, tag="mu")
        nc.sync.dma_start(out=mu_t, in_=mu_d[:, :])
        pt_t = persist.tile([DP, DC, D], bf16, tag="pt")
        nc.sync.dma_start(out=pt_t, in_=pt_d[:, :, :])
        WTb = persist.tile([DP, DC * H], bf16, tag="WTb")
        nc.sync.dma_start(out=WTb, in_=wtb_d[:, :])
        Wb = persist.tile([HP, HC * D], bf16, tag="Wb")
        nc.sync.dma_start(out=Wb, in_=wb_d[:, :])
        W = persist.tile([HP, HC * D], f32, tag="W")
        nc.sync.dma_start(out=W, in_=w_d[:, :])
        RT = persist.tile([HP, HC, O], bf16, tag="RT")
        nc.sync.dma_start(out=RT, in_=rt_d[:, :, :])
        bb = persist.tile([128, O], f32, tag="bb")
        nc.sync.dma_start(out=bb, in_=bb_d[:, :])

        Xall = persist.tile([DP, NBLK, DC, K], bf16, tag="Xall")
        XTall = persist.tile([K, NBLK, D], bf16, tag="XTall")
        Sxxall = persist.tile([K, NBLK, K], bf16, tag="Sxxall")

        # ---------------- whitening ----------------
        def whiten(bi):
            xt = xpool.tile([128, D], f32, tag="xraw")
            nc.sync.dma_start(out=xt, in_=x_d[bi * K:(bi + 1) * K, :])
            xc = xpool.tile([128, D], f32, tag="xc")
            eng = nc.vector if bi == 0 else nc.gpsimd
            eng.tensor_sub(xc, xt, mu_t)
            xcb = xpool.tile([128, D], bf16, tag="xcb")
            nc.vector.tensor_copy(xcb, xc)
            xct = xpool.tile([DP, DC * K], bf16, tag="xct")
            for p in range(3):          # paired transposes -> one copy per pair
                tp = psT.tile([DP, 2 * K], bf16, tag="tt")
                nc.tensor.transpose(
                    tp[:, 0:K], xcb[:, (2 * p) * DP:(2 * p + 1) * DP], identb
                )
                nc.tensor.transpose(
                    tp[:, K:2 * K], xcb[:, (2 * p + 1) * DP:(2 * p + 2) * DP],
                    identb,
                )
                nc.scalar.copy(xct[:, (2 * p) * K:(2 * p + 2) * K], tp)
            tp = psT.tile([DP, K], bf16, tag="tt")
            nc.tensor.transpose(tp, xcb[:, 6 * DP:7 * DP], identb)
            nc.scalar.copy(xct[:, 6 * K:7 * K], tp)
            XTb = XTall[:, bi, :]
            for s in range(2):
                ps = psB.tile([K, DS], f32, tag="big")
                for ic in range(DC):
                    nc.tensor.matmul(
                        ps, xct[:, ic * K:(ic + 1) * K],
                        pt_t[:, ic, s * DS:(s + 1) * DS],
                        start=(ic == 0), stop=(ic == DC - 1),
                    )
                nc.vector.tensor_add(
                    XTb[:, s * DS:(s + 1) * DS], ps, xc[:, s * DS:(s + 1) * DS]
                )
            xa = Xall[:, bi, :, :]
            for p in range(3):
                tp = psT.tile([DP, 2 * K], bf16, tag="tt")
                nc.tensor.transpose(
                    tp[:, 0:K], XTb[:, (2 * p) * DP:(2 * p + 1) * DP], identb
                )
                nc.tensor.transpose(
                    tp[:, K:2 * K], XTb[:, (2 * p + 1) * DP:(2 * p + 2) * DP],
                    identb,
                )
                nc.scalar.copy(xa[:, 2 * p:2 * p + 2, :], tp)
            tp = psT.tile([DP, K], bf16, tag="tt")
            nc.tensor.transpose(tp, XTb[:, 6 * DP:7 * DP], identb)
            nc.scalar.copy(xa[:, 6, :], tp)
            ps = psA.tile([K, K], f32, tag="kk")
            for ic in range(DC):
                nc.tensor.matmul(
                    ps, xa[:, ic, :], xa[:, ic, :],
                    start=(ic == 0), stop=(ic == DC - 1),
                )
            nc.scalar.mul(Sxxall[:, bi, :], ps, LR)

        tc.cur_priority = 0
        whiten(0)
        tc.cur_priority = pri_whit(1)
        whiten(1)
        tc.cur_priority = pri_whit(2)
        whiten(2)
        tc.cur_priority = pri_whit(3)
        whiten(3)

        Psb = {}  # block -> SBUF f32 tile [HP, HC*K] holding W^(stale) X_block

        def open_P(nb):
            """Accumulate P_nb = W^(current) X_nb into an SBUF f32 tile."""
            pt_sb = small.tile([HP, HC * K], f32, tag="Psb")
            for hc in range(HC):
                ps = psB.tile([HP, K], f32, tag="big")
                for ic in range(DC):
                    nc.tensor.matmul(
                        ps,
                        WTb[:, ic * H + hc * HP:ic * H + (hc + 1) * HP],
                        Xall[:, nb, ic, :],
                        start=(ic == 0), stop=(ic == DC - 1),
                    )
                nc.scalar.copy(pt_sb[:, hc * K:(hc + 1) * K], ps)
            Psb[nb] = pt_sb

        # P_1 with the initial weights (correction applied in epilogue of block 0).
        # Priority BELOW crit(0): open_P(1) waits on whiten(1), and at higher
        # priority it would head-of-line-block block 0's PE stream.
        tc.cur_priority = 1450
        open_P(1)
        y0q_next = None
        pending_tail = [None]

        for b in range(NBLK):
            # ================= HEAD (critical) =================
            tc.cur_priority = pri_crit(b)
            if b == 0:
                y0q = small.tile([HP, HC * K], bf16, tag="y0")
                for hc in range(HC):
                    ps = psB.tile([HP, K], f32, tag="big")
                    for ic in range(DC):
                        nc.tensor.matmul(
                            ps,
                            WTb[:, ic * H + hc * HP:ic * H + (hc + 1) * HP],
                            Xall[:, 0, ic, :],
                            start=(ic == 0), stop=(ic == DC - 1),
                        )
                    nc.vector.tensor_copy(y0q[:, hc * K:(hc + 1) * K], ps)
            else:
                y0q = y0q_next
            ps_syy = psA.tile([K, K], f32, tag="kk")
            for hc in range(HC):
                nc.tensor.matmul(
                    ps_syy, y0q[:, hc * K:(hc + 1) * K],
                    y0q[:, hc * K:(hc + 1) * K],
                    start=(hc == 0), stop=(hc == HC - 1),
                )
            syy_lr = small.tile([K, K], bf16, tag="syl")
            nc.scalar.mul(syy_lr, ps_syy, LR)
            syy_ng = small.tile([K, K], bf16, tag="syn")
            nc.vector.tensor_scalar_mul(syy_ng, ps_syy, -LR)

            # ---- head fill (off critical path) ----
            tc.cur_priority = pri_fill(b)
            y0t = small.tile([K, H], bf16, tag="y0t")
            for hc in range(HC):
                tp = psT.tile([128, K], bf16, tag="tt")
                nc.tensor.transpose(tp, y0q[:, hc * K:(hc + 1) * K], identb)
                nc.scalar.copy(y0t[:, hc * HP:(hc + 1) * HP], tp)
            # ================= RING (critical) =================
            tc.cur_priority = pri_crit(b)
            sxx = Sxxall[:, b, :]
            s_sb = small.tile([K, K], bf16, tag="s")
            nc.vector.scalar_tensor_tensor(
                s_sb, ps_syy, -LR, sxx, op0=AT.mult, op1=AT.add
            )
            tA0 = small.tile([K, K], bf16, tag="ta")
            nc.gpsimd.tensor_mul(tA0, s_sb, maskSU)
            A = small.tile([K, K], bf16, tag="A")
            nc.vector.tensor_add(A, tA0, ident)
            Bm = small.tile([K, K], bf16, tag="B")
            Bprev = identb
            for m in range(1, ring_iters):
                if m == 2:
                    # emit the previous block's master-update chain here so the
                    # scheduler places it after this block's head/early ring in
                    # every engine stream; t0t must follow it (reads updated Wb)
                    if pending_tail[0] is not None:
                        pending_tail[0]()
                        pending_tail[0] = None
                    tc.cur_priority = pri_fill(b - 1) if b else pri_fill(0)
                    if b + 1 < NBLK:
                        t0t = small.tile([K, D], bf16, tag="t0t")
                        for s in range(2):
                            ps = psB.tile([K, DS], f32, tag="big")
                            for hc in range(HC):
                                nc.tensor.matmul(
                                    ps, y0q[:, hc * K:(hc + 1) * K],
                                    Wb[:, hc * D + s * DS:hc * D + (s + 1) * DS],
                                    start=(hc == 0), stop=(hc == HC - 1),
                                )
                            nc.vector.tensor_copy(
                                t0t[:, s * DS:(s + 1) * DS], ps
                            )
                    tc.cur_priority = pri_crit(b)
                z2 = psA.tile([K, K], f32, tag="kk")
                nc.tensor.matmul(z2, syy_ng, A, start=True, stop=True)
                z2s = small.tile([K, K], bf16, tag="z2")
                nc.scalar.copy(z2s, z2)
                r1 = psA.tile([K, K], f32, tag="kk")
                nc.tensor.matmul(r1, A, syy_lr, start=True, stop=True)
                s_sb = small.tile([K, K], bf16, tag="s")
                nc.vector.tensor_sub(s_sb, sxx, r1)
                g = psA.tile([K, K], f32, tag="kk")
                nc.tensor.matmul(g, A, z2s, start=True, stop=True)
                gm = small.tile([K, K], bf16, tag="gm")
                nc.vector.tensor_mul(gm, g, maskSL)
                b1 = psA.tile([K, K], f32, tag="kk")
                nc.tensor.matmul(b1, gm, Bprev, start=True, stop=True)
                nc.vector.tensor_add(Bm, b1, ident)
                Bprev = Bm
                ct = psA.tile([K, K], f32, tag="kk")
                nc.tensor.matmul(ct, s_sb, Bm, start=True, stop=True)
                nt = small.tile([K, K], bf16, tag="nt")
                nc.vector.tensor_mul(nt, ct, maskSL)
                a1 = psA.tile([K, K], f32, tag="kk")
                nc.tensor.matmul(a1, nt, A, start=True, stop=True)
                nc.vector.tensor_add(A, a1, ident)
            # final B-update (B_R from A_R), epilogue A-work interleaved to
            # fill PE gaps while z2s/gm bounce on scalar/vector.
            # The last block needs only its logits: skip B_R/yt/q/ut entirely.
            if b + 1 < NBLK:
                z2 = psA.tile([K, K], f32, tag="kk")
                nc.tensor.matmul(z2, syy_ng, A, start=True, stop=True)
                z2s = small.tile([K, K], bf16, tag="z2")
                nc.scalar.copy(z2s, z2)
                ps_yt = psB.tile([K, H], f32, tag="big")
                nc.tensor.matmul(ps_yt, A, y0t, start=True, stop=True)
                g = psA.tile([K, K], f32, tag="kk")
                nc.tensor.matmul(g, A, z2s, start=True, stop=True)
                gm = small.tile([K, K], bf16, tag="gm")
                nc.vector.tensor_mul(gm, g, maskSL)
                ps_q0 = psB.tile([K, DS], f32, tag="big")
                nc.tensor.matmul(ps_q0, A, t0t[:, 0:DS], start=True, stop=True)
                b1 = psA.tile([K, K], f32, tag="kk")
                nc.tensor.matmul(b1, gm, Bprev, start=True, stop=True)
                ps_q1 = psB.tile([K, DS], f32, tag="big")
                nc.tensor.matmul(ps_q1, A, t0t[:, DS:D], start=True, stop=True)
                nc.vector.tensor_add(Bm, b1, ident)

                # ================= EPILOGUE (critical) =================
                yt = small.tile([K, H], bf16, tag="yt")
                nc.vector.tensor_copy(yt, ps_yt)
                q = small.tile([K, D], bf16, tag="q")
                nc.vector.tensor_sub(q[:, 0:DS], XTall[:, b, 0:DS], ps_q0)
                nc.vector.tensor_sub(q[:, DS:D], XTall[:, b, DS:D], ps_q1)
                # qT = transpose(q); qX = q @ X_{b+1} (depend only on q, so
                # they overlap the final B-chain); then C' = B_R^T qX is the
                # ONLY matmul left on the B_R -> clr path (was 14 via U).
                # Transposes ride psA "kk" (crit pool) to avoid psT contention
                # with whiten/WTb transposes.
                qT = small.tile([DP, DC * K], bf16, tag="U")
                for p in range(3):
                    tp = psA.tile([DP, 2 * K], bf16, tag="kk")
                    nc.tensor.transpose(
                        tp[:, 0:K], q[:, (2 * p) * DP:(2 * p + 1) * DP], identb
                    )
                    nc.tensor.transpose(
                        tp[:, K:2 * K], q[:, (2 * p + 1) * DP:(2 * p + 2) * DP],
                        identb,
                    )
                    nc.scalar.copy(qT[:, (2 * p) * K:(2 * p + 2) * K], tp)
                tp = psA.tile([DP, K], bf16, tag="kk")
                nc.tensor.transpose(tp, q[:, 6 * DP:7 * DP], identb)
                nc.scalar.copy(qT[:, 6 * K:7 * K], tp)
                psqx = psA.tile([K, K], f32, tag="kk")
                for ic in range(DC):
                    nc.tensor.matmul(
                        psqx, qT[:, ic * K:(ic + 1) * K], Xall[:, b + 1, ic, :],
                        start=(ic == 0), stop=(ic == DC - 1),
                    )
                qxs = small.tile([K, K], bf16, tag="qx")
                nc.scalar.copy(qxs, psqx)
                psc = psA.tile([K, K], f32, tag="kk")
                nc.tensor.matmul(psc, Bm, qxs, start=True, stop=True)
                clr = small.tile([K, K], bf16, tag="clr")
                nc.scalar.mul(clr, psc, LR)
                y0q_next = small.tile([HP, HC * K], bf16, tag="y0")
                for hc in range(HC):
                    cps = psA.tile([K, K], f32, tag="kk")
                    nc.tensor.matmul(
                        cps, yt[:, hc * HP:(hc + 1) * HP], clr,
                        start=True, stop=True,
                    )
                    nc.vector.tensor_add(
                        y0q_next[:, hc * K:(hc + 1) * K], cps,
                        Psb[b + 1][:, hc * K:(hc + 1) * K],
                    )
                ut = small.tile([K, D], bf16, tag="ut")
                for s in range(2):
                    ps = psB.tile([K, DS], f32, tag="big")
                    nc.tensor.matmul(
                        ps, Bm, q[:, s * DS:(s + 1) * DS], start=True, stop=True
                    )
                    nc.scalar.copy(ut[:, s * DS:(s + 1) * DS], ps)

            # ================= FILL TAIL =================
            tc.cur_priority = pri_fill(b)
            relu_y = small.tile([HP, HC * K], bf16, tag="ry")
            for hc in range(HC):
                ps2 = psA.tile([K, K], f32, tag="kk")
                nc.tensor.matmul(
                    ps2, y0t[:, hc * HP:(hc + 1) * HP], A, start=True, stop=True
                )
                nc.scalar.activation(
                    relu_y[:, hc * K:(hc + 1) * K], ps2,
                    mybir.ActivationFunctionType.Relu,
                )
            lg = psA.tile([K, O], f32, tag="kk")
            for hc in range(HC):
                nc.tensor.matmul(
                    lg, relu_y[:, hc * K:(hc + 1) * K], RT[:, hc, :],
                    start=(hc == 0), stop=(hc == HC - 1),
                )
            lgs = small.tile([K, O], f32, tag="lgs")
            tc.cur_priority = pri_out(b)
            nc.vector.tensor_add(lgs, lg, bb)
            nc.sync.dma_start(out=out_d[b * K:(b + 1) * K, :], in_=lgs)
            tc.cur_priority = pri_fill(b)

            if b + LOOKAHEAD < NBLK:
                tc.cur_priority = pri_whit(b + LOOKAHEAD)
                whiten(b + LOOKAHEAD)
                tc.cur_priority = pri_fill(b)
            if b + 1 < NBLK:
                # masters: W += lr * Y U^T (fp32, in place); Wb = cast(W);
                # WTb = PE-transpose of Wb. Deferred: emitted inside the NEXT
                # block's ring so engine streams order it behind that block's
                # critical head.
                def make_tail(b, yt, ut):
                    def tail():
                        tc.cur_priority = pri_fill(b)
                        for hc in range(HC):
                            for s in range(2):
                                ps = psB.tile([HP, DS], f32, tag="big")
                                nc.tensor.matmul(
                                    ps, yt[:, hc * HP:(hc + 1) * HP],
                                    ut[:, s * DS:(s + 1) * DS],
                                    start=True, stop=True,
                                )
                                wsl = W[:, hc * D + s * DS:hc * D + (s + 1) * DS]
                                nc.vector.scalar_tensor_tensor(
                                    wsl, ps, LR, wsl, op0=AT.mult, op1=AT.add
                                )
                                nc.scalar.copy(
                                    Wb[:, hc * D + s * DS:hc * D + (s + 1) * DS],
                                    wsl,
                                )
                        for dc in range(DC):
                            tp = psT.tile([DP, 2 * K], bf16, tag="tt")
                            nc.tensor.transpose(
                                tp[:, 0:HP],
                                Wb[:, 0 * D + dc * DP:0 * D + (dc + 1) * DP],
                                identb,
                            )
                            nc.tensor.transpose(
                                tp[:, HP:2 * HP],
                                Wb[:, 1 * D + dc * DP:1 * D + (dc + 1) * DP],
                                identb,
                            )
                            nc.scalar.copy(WTb[:, dc * H:(dc + 1) * H], tp)
                        if b + 2 < NBLK:
                            open_P(b + 2)
                    return tail

                pending_tail[0] = make_tail(b, yt, ut)

    _split_multiwait(nc)
    return nc


def prep_inputs(x, whiten_mean, whiten_mat, oja_W, readout_W, readout_b):
    """Host-side layout/dtype prep (no contractions)."""
    x = np.ascontiguousarray(x, dtype=np.float32)
    mu_b = np.broadcast_to(
        np.asarray(whiten_mean, dtype=np.float32)[None, :], (128, D)
    ).copy()
    P = np.asarray(whiten_mat, dtype=np.float32) - np.eye(D, dtype=np.float32)
    # pt[dp, ic, dout] = P^T[ic*112+dp, dout] = P[dout, ic*112+dp]
    pt = np.ascontiguousarray(
        P.T.reshape(DC, DP, D).transpose(1, 0, 2).astype(ml_dtypes.bfloat16)
    )
    Wf = np.asarray(oja_W, dtype=np.float32)
    w = np.ascontiguousarray(
        Wf.reshape(HC, HP, D).transpose(1, 0, 2).reshape(HP, HC * D)
    )
    wtb = np.ascontiguousarray(
        Wf.T.reshape(DC, DP, H).transpose(1, 0, 2).reshape(DP, DC * H)
    ).astype(ml_dtypes.bfloat16)
    Rf = np.asarray(readout_W, dtype=np.float32)
    rt = np.ascontiguousarray(
        Rf.T.reshape(HC, HP, O).transpose(1, 0, 2).astype(ml_dtypes.bfloat16)
    )
    b_b = np.broadcast_to(
        np.asarray(readout_b, dtype=np.float32)[None, :], (128, O)
    ).copy()
    return {
        "x": x, "mu_b": mu_b, "pt": pt, "w": w, "rt": rt, "b_b": b_b,
        "w_bf": w.astype(ml_dtypes.bfloat16), "wt_bf": wtb,
    }


_cached_nc = None


def _get_nc():
    global _cached_nc
    if _cached_nc is None:
        _cached_nc = build_nc()
    return _cached_nc


def kernel(x, whiten_mean, whiten_mat, oja_W, readout_W, readout_b, **run_kwargs):
    nc = _get_nc()
    ins = prep_inputs(x, whiten_mean, whiten_mat, oja_W, readout_W, readout_b)
    res = run_bass_kernel_spmd(
        nc, [ins] * N_CORES, core_ids=list(range(N_CORES)), **run_kwargs
    )
    out = res.results[0]["out"]
    if run_kwargs:
        kernel.last_result = res
    return out


# revision 25
# speedup vs baseline: 1.1062x; 1.0413x over previous
"""Trainium2 Bass kernel for nn_BioClassifier: whitening + sequential Oja scan + readout.

v2: restructured for critical-path latency. Same block-parallel-scan math as v1
(chunk the 2048-sample Oja scan into 16 blocks of K=128; per block a fixed-point
"ring" on K x K matrices closes the sequential recurrence exactly):
    Y = Y0 A,  U = (X - T0 A) B,   A = (I - lr*SU(C))^-1, B = (I + lr*SU(G))^-1
    C = U^T X, G = Y^T Y,  T0^T X = Syy
Key v2 changes vs v1:
  * lr folded into sxx_lr/syy_lr (bf16), iteration reordered so each ring cycle
    is 10 serial engine-hops (B-chain: z2->z2s->g->gm->b1->B; A-chain: ct->nt->
    a1->A) with r1/s prep hidden under the B-chain; iter-0 A-update is 3 vec ops
    (A1 = I + SU(s0), s0 = sxx_lr - syy_lr), no matmuls.
  * Y0 correction form: P_{n} = W^{(n-1)} X_n accumulates in an OPEN PSUM group
    during ring_{n-1} (off critical path); epilogue closes it with the rank-K
    correction  Y0_n = P_n + lr * Y * (U^T X_n), so the master-W update and the
    14 Y0 matmuls leave the serial path entirely.
  * single fp32 master W [H,D]; Wb = cast(W) on scalar; WTb = PE-transpose of Wb
    (bf16 transpose == transpose of bf16 cast, exact) - drops the WT fp32 master
    and its vector-engine update entirely.
  * XTall/Sxxall stored bf16 (Sxx pre-scaled by lr at whiten time).
  * engine rebalance: ring bounces on DVE, z2s/copies on scalar, xc-sub and
    the iter-0 mask-mul on gpsimd (which cannot touch PSUM); whitening +
    masters + P-opens emitted in priority bands (and the master-update tail
    emitted mid-ring of the NEXT block) so the in-order engine streams place
    them behind each block's critical chain.
  * RING_ITERS=4 (validated offline: rel err 1.49e-2 vs the 2e-2 gate;
    RING_ITERS=5 gives 8.7e-3 at ~+60us).
All 8 cores run the identical program (the scan is inherently sequential;
core 0's output is returned).
"""

import os
import sys
from contextlib import ExitStack

sys.path.insert(0, "/opt/trn_rl_repo")

import numpy as np
import ml_dtypes

import concourse.bass as bass
import concourse.mybir as mybir
from concourse.tile import TileContext
from concourse.masks import make_identity
from concourse.bass_utils import run_bass_kernel_spmd
from concourse.vector_clock import ScopedClock

LR = 1e-3
B, D, H, O = 2048, 784, 256, 10
K = 128
NBLK = B // K
DP, DC = 112, 7          # D = 784 = 7 * 112
HP, HC = 128, 2          # H = 256 = 2 * 128
DS = D // 2              # 392: matmul free-dim split for D-wide outputs

RING_ITERS = int(os.environ.get("RING_ITERS", "4"))
LOOKAHEAD = 4
N_CORES = 1

f32 = mybir.dt.float32
bf16 = mybir.dt.bfloat16
AT = mybir.AluOpType


def _install_ntff_hook():
    """The agent image's `antenv` lacks `axon_hooks`, so trace=True degrades.
    Synthesize the module and register the ctypes NTFF hook from trn_boot."""
    import types
    import antenv

    if getattr(antenv, "axon_hooks", None) is not None:
        return
    mod = types.ModuleType("antenv.axon_hooks")
    _hook_box = [None]
    mod.set_axon_ntff_profile_hook = lambda h: _hook_box.__setitem__(0, h)
    mod.get_axon_ntff_profile_hook = lambda: _hook_box[0]
    sys.modules["antenv.axon_hooks"] = mod
    antenv.axon_hooks = mod
    try:
        sys.path.insert(0, "/root/.axon_site")
        from trn_agent_boot.trn_boot import _ntff_profile_via_ctypes

        hook = _ntff_profile_via_ctypes("/opt/axon/libaxon_pjrt.so")
        if hook is not None:
            mod.set_axon_ntff_profile_hook(hook)
    except Exception:
        pass


try:
    _install_ntff_hook()
except Exception:
    pass

_drain_patched = False


def _patch_drain():
    """This walrus build only supports one sync-wait per CTRL instruction;
    split the Tile kernel-tail drain into one drain per semaphore wait."""
    global _drain_patched
    if _drain_patched:
        return

    def patched(self, tick_clock, wait_clock):
        drain_inst = self.nc.sync.drain()
        wait_clock.add_sem_waits(
            drain_inst.ins, ScopedClock({None: tick_clock.global_clock})
        )
        mi = drain_inst.ins
        si = mi.sync_info
        if si is not None and len(si.on_wait) > 1:
            waits = list(si.on_wait)
            mi.sync_info = mybir.SyncInfo(
                on_wait=[waits[0]], on_update=list(si.on_update)
            )
            for w in waits[1:]:
                d2 = self.nc.sync.drain()
                d2.ins.sync_info = mybir.SyncInfo(on_wait=[w], on_update=[])
        self.nc.all_engine_barrier()
        assert self.sems is not None
        popped = self.nc._tile_sem_poison_stack.pop()
        assert popped is self._sem_poison
        self.nc.clear_and_free_semaphores(list(self.sems.allocated().values()))
        self.nc.all_engine_barrier()

    TileContext._drain_and_barrier = patched
    _drain_patched = True


def _split_multiwait(nc, limit=1):
    """This walrus build supports only `limit` sync-waits per instruction.
    Hoist extra waits onto NoOps inserted just before, in the same engine
    stream (engines are in-order, so earlier waits are strictly safe)."""
    n_split = 0
    for f in nc.m.functions:
        for bb in f.blocks:
            insts = list(bb.instructions)
            if not any(
                i.sync_info is not None and len(i.sync_info.on_wait) > limit
                for i in insts
            ):
                continue
            new = []
            for inst in insts:
                si = inst.sync_info
                if si is not None and len(si.on_wait) > limit:
                    waits = list(si.on_wait)
                    for j, w in enumerate(waits[: len(waits) - limit]):
                        nop = mybir.InstNoOp(
                            name=f"{inst.name}-hw{j}", engine=inst.engine,
                            ins=[], outs=[],
                        )
                        nop.sync_info = mybir.SyncInfo(on_wait=[w], on_update=[])
                        new.append(nop)
                        n_split += 1
                    inst.sync_info = mybir.SyncInfo(
                        on_wait=waits[len(waits) - limit:],
                        on_update=list(si.on_update),
                    )
                new.append(inst)
            bb.instructions = new
    return n_split


def build_nc(ring_iters=RING_ITERS):
    _patch_drain()
    nc = bass.Bass()
    x_d = nc.dram_tensor("x", [B, D], f32, kind="ExternalInput")
    mu_d = nc.dram_tensor("mu_b", [128, D], f32, kind="ExternalInput")
    pt_d = nc.dram_tensor("pt", [DP, DC, D], bf16, kind="ExternalInput")
    w_d = nc.dram_tensor("w", [HP, HC * D], f32, kind="ExternalInput")
    wb_d = nc.dram_tensor("w_bf", [HP, HC * D], bf16, kind="ExternalInput")
    wtb_d = nc.dram_tensor("wt_bf", [DP, DC * H], bf16, kind="ExternalInput")
    rt_d = nc.dram_tensor("rt", [HP, HC, O], bf16, kind="ExternalInput")
    bb_d = nc.dram_tensor("b_b", [128, O], f32, kind="ExternalInput")
    out_d = nc.dram_tensor("out", [B, O], f32, kind="ExternalOutput")
    def pri_crit(b):
        # critical path of block b
        return 1000 + b * 1000

    def pri_fill(b):
        # fill work of block b: must rank BELOW crit of b+1 (it runs during
        # ring_{b+1}) but above crit of b+2
        return 1000 + (b + 1) * 1000 + 500

    def pri_whit(b):
        # whiten(b) must complete before epilogue of b-1: rank just below
        # crit(b-1), above fill bands of earlier blocks
        return 1000 + (b - 1) * 1000 + 400

    def pri_out(b):
        return 20_000_000 + b * 1000

    with TileContext(nc) as tc, ExitStack() as ctx:
        persist = ctx.enter_context(tc.tile_pool(name="persist", bufs=1))
        xpool = ctx.enter_context(tc.tile_pool(name="xpool", bufs=5))
        small = ctx.enter_context(tc.tile_pool(name="small", bufs=2))
        psA = ctx.enter_context(tc.tile_pool(name="psA", bufs=3, space="PSUM"))
        psB = ctx.enter_context(tc.tile_pool(name="psB", bufs=3, space="PSUM"))
        psT = ctx.enter_context(tc.tile_pool(name="psT", bufs=2, space="PSUM"))

        ident = persist.tile([128, 128], f32, tag="ident")
        make_identity(nc, ident)
        identb = persist.tile([128, 128], bf16, tag="identb")
        nc.vector.tensor_copy(identb, ident)
        # 0/1 masks (lr is folded into sxx_lr / syy_lr)
        maskSL = persist.tile([K, K], f32, tag="maskSL")
        nc.gpsimd.memset(maskSL, 1.0)
        nc.gpsimd.affine_select(
            out=maskSL, in_=maskSL, compare_op=AT.is_gt, fill=0.0,
            base=0, pattern=[[-1, K]], channel_multiplier=1,
        )
        maskSU = persist.tile([K, K], f32, tag="maskSU")
        nc.gpsimd.memset(maskSU, 1.0)
        nc.vector.tensor_sub(maskSU, maskSU, ident)
        nc.vector.tensor_sub(maskSU, maskSU, maskSL)

        mu_t = persist.tile([128, D], f32, tag="mu")
        nc.sync.dma_start(out=mu_t, in_=mu_d[:, :])
        pt_t = persist.tile([DP, DC, D], bf16, tag="pt")
        nc.sync.dma_start(out=pt_t, in_=pt_d[:, :, :])
        WTb = persist.tile([DP, DC * H], bf16, tag="WTb")
        nc.sync.dma_start(out=WTb, in_=wtb_d[:, :])
        Wb = persist.tile([HP, HC * D], bf16, tag="Wb")
        nc.sync.dma_start(out=Wb, in_=wb_d[:, :])
        W = persist.tile([HP, HC * D], f32, tag="W")
        nc.sync.dma_start(out=W, in_=w_d[:, :])
        RT = persist.tile([HP, HC, O], bf16, tag="RT")
        nc.sync.dma_start(out=RT, in_=rt_d[:, :, :])
        bb = persist.tile([128, O], f32, tag="bb")
        nc.sync.dma_start(out=bb, in_=bb_d[:, :])

        Xall = persist.tile([DP, NBLK, DC, K], bf16, tag="Xall")
        XTall = persist.tile([K, NBLK, D], bf16, tag="XTall")
        Sxxall = persist.tile([K, NBLK, K], bf16, tag="Sxxall")

        # ---------------- whitening ----------------
        def whiten(bi):
            xt = xpool.tile([128, D], f32, tag="xraw")
            nc.sync.dma_start(out=xt, in_=x_d[bi * K:(bi + 1) * K, :])
            xc = xpool.tile([128, D], f32, tag="xc")
            eng = nc.vector if bi == 0 else nc.gpsimd
            eng.tensor_sub(xc, xt, mu_t)
            xcb = xpool.tile([128, D], bf16, tag="xcb")
            nc.vector.tensor_copy(xcb, xc)
            xct = xpool.tile([DP, DC * K], bf16, tag="xct")
            for p in range(3):          # paired transposes -> one copy per pair
                tp = psT.tile([DP, 2 * K], bf16, tag="tt")
                nc.tensor.transpose(
                    tp[:, 0:K], xcb[:, (2 * p) * DP:(2 * p + 1) * DP], identb
                )
                nc.tensor.transpose(
                    tp[:, K:2 * K], xcb[:, (2 * p + 1) * DP:(2 * p + 2) * DP],
                    identb,
                )
                nc.scalar.copy(xct[:, (2 * p) * K:(2 * p + 2) * K], tp)
            tp = psT.tile([DP, K], bf16, tag="tt")
            nc.tensor.transpose(tp, xcb[:, 6 * DP:7 * DP], identb)
            nc.scalar.copy(xct[:, 6 * K:7 * K], tp)
            XTb = XTall[:, bi, :]
            for s in range(2):
                ps = psB.tile([K, DS], f32, tag="big")
                for ic in range(DC):
                    nc.tensor.matmul(
                        ps, xct[:, ic * K:(ic + 1) * K],
                        pt_t[:, ic, s * DS:(s + 1) * DS],
                        start=(ic == 0), stop=(ic == DC - 1),
                    )
                nc.vector.tensor_add(
                    XTb[:, s * DS:(s + 1) * DS], ps, xc[:, s * DS:(s + 1) * DS]
                )
            xa = Xall[:, bi, :, :]
            for p in range(3):
                tp = psT.tile([DP, 2 * K], bf16, tag="tt")
                nc.tensor.transpose(
                    tp[:, 0:K], XTb[:, (2 * p) * DP:(2 * p + 1) * DP], identb
                )
                nc.tensor.transpose(
                    tp[:, K:2 * K], XTb[:, (2 * p + 1) * DP:(2 * p + 2) * DP],
                    identb,
                )
                nc.scalar.copy(xa[:, 2 * p:2 * p + 2, :], tp)
            tp = psT.tile([DP, K], bf16, tag="tt")
            nc.tensor.transpose(tp, XTb[:, 6 * DP:7 * DP], identb)
            nc.scalar.copy(xa[:, 6, :], tp)
            ps = psA.tile([K, K], f32, tag="kk")
            for ic in range(DC):
                nc.tensor.matmul(
                    ps, xa[:, ic, :], xa[:, ic, :],
                    start=(ic == 0), stop=(ic == DC - 1),
                )
            nc.scalar.mul(Sxxall[:, bi, :], ps, LR)

        tc.cur_priority = 0
        whiten(0)
        tc.cur_priority = pri_whit(1)
        whiten(1)
        tc.cur_priority = pri_whit(2)
        whiten(2)
        tc.cur_priority = pri_whit(3)
        whiten(3)

        Psb = {}  # block -> SBUF f32 tile [HP, HC*K] holding W^(stale) X_block

        def open_P(nb):
            """Accumulate P_nb = W^(current) X_nb into an SBUF f32 tile."""
            pt_sb = small.tile([HP, HC * K], f32, tag="Psb")
            for hc in range(HC):
                ps = psB.tile([HP, K], f32, tag="big")
                for ic in range(DC):
                    nc.tensor.matmul(
                        ps,
                        WTb[:, ic * H + hc * HP:ic * H + (hc + 1) * HP],
                        Xall[:, nb, ic, :],
                        start=(ic == 0), stop=(ic == DC - 1),
                    )
                nc.scalar.copy(pt_sb[:, hc * K:(hc + 1) * K], ps)
            Psb[nb] = pt_sb

        # P_1 with the initial weights (correction applied in epilogue of block 0).
        # Priority BELOW crit(0): open_P(1) waits on whiten(1), and at higher
        # priority it would head-of-line-block block 0's PE stream.
        tc.cur_priority = 1450
        open_P(1)
        y0q_next = None
        pending_tail = [None]

        for b in range(NBLK):
            # ================= HEAD (critical) =================
            tc.cur_priority = pri_crit(b)
            if b == 0:
                y0q = small.tile([HP, HC * K], bf16, tag="y0")
                for hc in range(HC):
                    ps = psB.tile([HP, K], f32, tag="big")
                    for ic in range(DC):
                        nc.tensor.matmul(
                            ps,
                            WTb[:, ic * H + hc * HP:ic * H + (hc + 1) * HP],
                            Xall[:, 0, ic, :],
                            start=(ic == 0), stop=(ic == DC - 1),
                        )
                    nc.vector.tensor_copy(y0q[:, hc * K:(hc + 1) * K], ps)
            else:
                y0q = y0q_next
            ps_syy = psA.tile([K, K], f32, tag="kk")
            for hc in range(HC):
                nc.tensor.matmul(
                    ps_syy, y0q[:, hc * K:(hc + 1) * K],
                    y0q[:, hc * K:(hc + 1) * K],
                    start=(hc == 0), stop=(hc == HC - 1),
                )
            syy_lr = small.tile([K, K], bf16, tag="syl")
            nc.scalar.mul(syy_lr, ps_syy, LR)
            syy_ng = small.tile([K, K], bf16, tag="syn")
            nc.vector.tensor_scalar_mul(syy_ng, ps_syy, -LR)

            # ---- head fill (off critical path) ----
            tc.cur_priority = pri_fill(b)
            y0t = small.tile([K, H], bf16, tag="y0t")
            for hc in range(HC):
                tp = psT.tile([128, K], bf16, tag="tt")
                nc.tensor.transpose(tp, y0q[:, hc * K:(hc + 1) * K], identb)
                nc.scalar.copy(y0t[:, hc * HP:(hc + 1) * HP], tp)
            # ================= RING (critical) =================
            tc.cur_priority = pri_crit(b)
            sxx = Sxxall[:, b, :]
            s_sb = small.tile([K, K], bf16, tag="s")
            nc.vector.scalar_tensor_tensor(
                s_sb, ps_syy, -LR, sxx, op0=AT.mult, op1=AT.add
            )
            tA0 = small.tile([K, K], bf16, tag="ta")
            nc.gpsimd.tensor_mul(tA0, s_sb, maskSU)
            A = small.tile([K, K], bf16, tag="A")
            nc.vector.tensor_add(A, tA0, ident)
            Bm = small.tile([K, K], bf16, tag="B")
            Bprev = identb
            for m in range(1, ring_iters):
                if m == 2:
                    # emit the previous block's master-update chain here so the
                    # scheduler places it after this block's head/early ring in
                    # every engine stream; t0t must follow it (reads updated Wb)
                    if pending_tail[0] is not None:
                        pending_tail[0]()
                        pending_tail[0] = None
                    tc.cur_priority = pri_fill(b - 1) if b else pri_fill(0)
                    if b + 1 < NBLK:
                        t0t = small.tile([K, D], bf16, tag="t0t")
                        for s in range(2):
                            ps = psB.tile([K, DS], f32, tag="big")
                            for hc in range(HC):
                                nc.tensor.matmul(
                                    ps, y0q[:, hc * K:(hc + 1) * K],
                                    Wb[:, hc * D + s * DS:hc * D + (s + 1) * DS],
                                    start=(hc == 0), stop=(hc == HC - 1),
                                )
                            nc.vector.tensor_copy(
                                t0t[:, s * DS:(s + 1) * DS], ps
                            )
                    tc.cur_priority = pri_crit(b)
                z2 = psA.tile([K, K], f32, tag="kk")
                nc.tensor.matmul(z2, syy_ng, A, start=True, stop=True)
                z2s = small.tile([K, K], bf16, tag="z2")
                nc.scalar.copy(z2s, z2)
                r1 = psA.tile([K, K], f32, tag="kk")
                nc.tensor.matmul(r1, A, syy_lr, start=True, stop=True)
                s_sb = small.tile([K, K], bf16, tag="s")
                nc.vector.tensor_sub(s_sb, sxx, r1)
                g = psA.tile([K, K], f32, tag="kk")
                nc.tensor.matmul(g, A, z2s, start=True, stop=True)
                gm = small.tile([K, K], bf16, tag="gm")
                nc.vector.tensor_mul(gm, g, maskSL)
                b1 = psA.tile([K, K], f32, tag="kk")
                nc.tensor.matmul(b1, gm, Bprev, start=True, stop=True)
                nc.vector.tensor_add(Bm, b1, ident)
                Bprev = Bm
                ct = psA.tile([K, K], f32, tag="kk")
                nc.tensor.matmul(ct, s_sb, Bm, start=True, stop=True)
                nt = small.tile([K, K], bf16, tag="nt")
                nc.vector.tensor_mul(nt, ct, maskSL)
                a1 = psA.tile([K, K], f32, tag="kk")
                nc.tensor.matmul(a1, nt, A, start=True, stop=True)
                nc.vector.tensor_add(A, a1, ident)
            # final B-update (B_R from A_R), epilogue A-work interleaved to
            # fill PE gaps while z2s/gm bounce on scalar/vector.
            # The last block needs only its logits: skip B_R/yt/q/ut entirely.
            if b + 1 < NBLK:
                # q = XT - A^T t0t FIRST (only needs A): its halves feed the
                # qT transposes and the qX accumulation, which then overlap
                # the final B-chain; C' = B_R^T qX is the only matmul left on
                # the B_R -> clr path.
                z2 = psA.tile([K, K], f32, tag="kk")
                nc.tensor.matmul(z2, syy_ng, A, start=True, stop=True)
                z2s = small.tile([K, K], bf16, tag="z2")
                nc.scalar.copy(z2s, z2)
                ps_q0 = psB.tile([K, DS], f32, tag="big")
                nc.tensor.matmul(ps_q0, A, t0t[:, 0:DS], start=True, stop=True)
                q = small.tile([K, D], bf16, tag="q")
                nc.vector.tensor_sub(q[:, 0:DS], XTall[:, b, 0:DS], ps_q0)
                g = psA.tile([K, K], f32, tag="kk")
                nc.tensor.matmul(g, A, z2s, start=True, stop=True)
                gm = small.tile([K, K], bf16, tag="gm")
                nc.vector.tensor_mul(gm, g, maskSL)
                ps_q1 = psB.tile([K, DS], f32, tag="big")
                nc.tensor.matmul(ps_q1, A, t0t[:, DS:D], start=True, stop=True)
                nc.vector.tensor_sub(q[:, DS:D], XTall[:, b, DS:D], ps_q1)
                qT = small.tile([DP, DC * K], bf16, tag="U")
                tp = psA.tile([DP, 2 * K], bf16, tag="kk")
                nc.tensor.transpose(tp[:, 0:K], q[:, 0:DP], identb)
                nc.tensor.transpose(tp[:, K:2 * K], q[:, DP:2 * DP], identb)
                nc.scalar.copy(qT[:, 0:2 * K], tp)
                b1 = psA.tile([K, K], f32, tag="kk")
                nc.tensor.matmul(b1, gm, Bprev, start=True, stop=True)
                nc.vector.tensor_add(Bm, b1, ident)
                ps_yt = psB.tile([K, H], f32, tag="big")
                nc.tensor.matmul(ps_yt, A, y0t, start=True, stop=True)
                yt = small.tile([K, H], bf16, tag="yt")
                nc.vector.tensor_copy(yt, ps_yt)
                for p in range(1, 3):
                    tp = psA.tile([DP, 2 * K], bf16, tag="kk")
                    nc.tensor.transpose(
                        tp[:, 0:K], q[:, (2 * p) * DP:(2 * p + 1) * DP], identb
                    )
                    nc.tensor.transpose(
                        tp[:, K:2 * K], q[:, (2 * p + 1) * DP:(2 * p + 2) * DP],
                        identb,
                    )
                    nc.scalar.copy(qT[:, (2 * p) * K:(2 * p + 2) * K], tp)
                tp = psA.tile([DP, K], bf16, tag="kk")
                nc.tensor.transpose(tp, q[:, 6 * DP:7 * DP], identb)
                nc.scalar.copy(qT[:, 6 * K:7 * K], tp)
                psqx = psA.tile([K, K], f32, tag="kk")
                for ic in range(DC):
                    nc.tensor.matmul(
                        psqx, qT[:, ic * K:(ic + 1) * K], Xall[:, b + 1, ic, :],
                        start=(ic == 0), stop=(ic == DC - 1),
                    )
                qxs = small.tile([K, K], bf16, tag="qx")
                nc.scalar.copy(qxs, psqx)
                psc = psA.tile([K, K], f32, tag="kk")
                nc.tensor.matmul(psc, Bm, qxs, start=True, stop=True)
                clr = small.tile([K, K], bf16, tag="clr")
                nc.scalar.mul(clr, psc, LR)
                y0q_next = small.tile([HP, HC * K], bf16, tag="y0")
                for hc in range(HC):
                    cps = psA.tile([K, K], f32, tag="kk")
                    nc.tensor.matmul(
                        cps, yt[:, hc * HP:(hc + 1) * HP], clr,
                        start=True, stop=True,
                    )
                    nc.vector.tensor_add(
                        y0q_next[:, hc * K:(hc + 1) * K], cps,
                        Psb[b + 1][:, hc * K:(hc + 1) * K],
                    )
                ut = small.tile([K, D], bf16, tag="ut")
                for s in range(2):
                    ps = psB.tile([K, DS], f32, tag="big")
                    nc.tensor.matmul(
                        ps, Bm, q[:, s * DS:(s + 1) * DS], start=True, stop=True
                    )
                    nc.scalar.copy(ut[:, s * DS:(s + 1) * DS], ps)

            # ================= FILL TAIL =================
            tc.cur_priority = pri_fill(b)
            relu_y = small.tile([HP, HC * K], bf16, tag="ry")
            for hc in range(HC):
                ps2 = psA.tile([K, K], f32, tag="kk")
                nc.tensor.matmul(
                    ps2, y0t[:, hc * HP:(hc + 1) * HP], A, start=True, stop=True
                )
                nc.scalar.activation(
                    relu_y[:, hc * K:(hc + 1) * K], ps2,
                    mybir.ActivationFunctionType.Relu,
                )
            lg = psA.tile([K, O], f32, tag="kk")
            for hc in range(HC):
                nc.tensor.matmul(
                    lg, relu_y[:, hc * K:(hc + 1) * K], RT[:, hc, :],
                    start=(hc == 0), stop=(hc == HC - 1),
                )
            lgs = small.tile([K, O], f32, tag="lgs")
            tc.cur_priority = pri_out(b)
            nc.vector.tensor_add(lgs, lg, bb)
            nc.sync.dma_start(out=out_d[b * K:(b + 1) * K, :], in_=lgs)
            tc.cur_priority = pri_fill(b)

            if b + LOOKAHEAD < NBLK:
                tc.cur_priority = pri_whit(b + LOOKAHEAD)
                whiten(b + LOOKAHEAD)
                tc.cur_priority = pri_fill(b)
            if b + 1 < NBLK:
                # masters: W += lr * Y U^T (fp32, in place); Wb = cast(W);
                # WTb = PE-transpose of Wb. Deferred: emitted inside the NEXT
                # block's ring so engine streams order it behind that block's
                # critical head.
                def make_tail(b, yt, ut):
                    def tail():
                        tc.cur_priority = pri_fill(b)
                        for hc in range(HC):
                            for s in range(2):
                                ps = psB.tile([HP, DS], f32, tag="big")
                                nc.tensor.matmul(
                                    ps, yt[:, hc * HP:(hc + 1) * HP],
                                    ut[:, s * DS:(s + 1) * DS],
                                    start=True, stop=True,
                                )
                                wsl = W[:, hc * D + s * DS:hc * D + (s + 1) * DS]
                                nc.vector.scalar_tensor_tensor(
                                    wsl, ps, LR, wsl, op0=AT.mult, op1=AT.add
                                )
                                nc.scalar.copy(
                                    Wb[:, hc * D + s * DS:hc * D + (s + 1) * DS],
                                    wsl,
                                )
                        for dc in range(DC):
                            tp = psT.tile([DP, 2 * K], bf16, tag="tt")
                            nc.tensor.transpose(
                                tp[:, 0:HP],
                                Wb[:, 0 * D + dc * DP:0 * D + (dc + 1) * DP],
                                identb,
                            )
                            nc.tensor.transpose(
                                tp[:, HP:2 * HP],
                                Wb[:, 1 * D + dc * DP:1 * D + (dc + 1) * DP],
                                identb,
                            )
                            nc.scalar.copy(WTb[:, dc * H:(dc + 1) * H], tp)
                        if b + 2 < NBLK:
                            open_P(b + 2)
                    return tail

                pending_tail[0] = make_tail(b, yt, ut)

    _split_multiwait(nc)
    return nc


def prep_inputs(x, whiten_mean, whiten_mat, oja_W, readout_W, readout_b):
    """Host-side layout/dtype prep (no contractions)."""
    x = np.ascontiguousarray(x, dtype=np.float32)
    mu_b = np.broadcast_to(
        np.asarray(whiten_mean, dtype=np.float32)[None, :], (128, D)
    ).copy()
    P = np.asarray(whiten_mat, dtype=np.float32) - np.eye(D, dtype=np.float32)
    # pt[dp, ic, dout] = P^T[ic*112+dp, dout] = P[dout, ic*112+dp]
    pt = np.ascontiguousarray(
        P.T.reshape(DC, DP, D).transpose(1, 0, 2).astype(ml_dtypes.bfloat16)
    )
    Wf = np.asarray(oja_W, dtype=np.float32)
    w = np.ascontiguousarray(
        Wf.reshape(HC, HP, D).transpose(1, 0, 2).reshape(HP, HC * D)
    )
    wtb = np.ascontiguousarray(
        Wf.T.reshape(DC, DP, H).transpose(1, 0, 2).reshape(DP, DC * H)
    ).astype(ml_dtypes.bfloat16)
    Rf = np.asarray(readout_W, dtype=np.float32)
    rt = np.ascontiguousarray(
        Rf.T.reshape(HC, HP, O).transpose(1, 0, 2).astype(ml_dtypes.bfloat16)
    )
    b_b = np.broadcast_to(
        np.asarray(readout_b, dtype=np.float32)[None, :], (128, O)
    ).copy()
    return {
        "x": x, "mu_b": mu_b, "pt": pt, "w": w, "rt": rt, "b_b": b_b,
        "w_bf": w.astype(ml_dtypes.bfloat16), "wt_bf": wtb,
    }


_cached_nc = None


def _get_nc():
    global _cached_nc
    if _cached_nc is None:
        _cached_nc = build_nc()
    return _cached_nc


def kernel(x, whiten_mean, whiten_mat, oja_W, readout_W, readout_b, **run_kwargs):
    nc = _get_nc()
    ins = prep_inputs(x, whiten_mean, whiten_mat, oja_W, readout_W, readout_b)
    res = run_bass_kernel_spmd(
        nc, [ins] * N_CORES, core_ids=list(range(N_CORES)), **run_kwargs
    )
    out = res.results[0]["out"]
    if run_kwargs:
        kernel.last_result = res
    return out


# revision 27
# speedup vs baseline: 1.1208x; 1.0132x over previous
"""Trainium2 Bass kernel for nn_BioClassifier: whitening + sequential Oja scan + readout.

v2: restructured for critical-path latency. Same block-parallel-scan math as v1
(chunk the 2048-sample Oja scan into 16 blocks of K=128; per block a fixed-point
"ring" on K x K matrices closes the sequential recurrence exactly):
    Y = Y0 A,  U = (X - T0 A) B,   A = (I - lr*SU(C))^-1, B = (I + lr*SU(G))^-1
    C = U^T X, G = Y^T Y,  T0^T X = Syy
Key v2 changes vs v1:
  * lr folded into sxx_lr/syy_lr (bf16), iteration reordered so each ring cycle
    is 10 serial engine-hops (B-chain: z2->z2s->g->gm->b1->B; A-chain: ct->nt->
    a1->A) with r1/s prep hidden under the B-chain; iter-0 A-update is 3 vec ops
    (A1 = I + SU(s0), s0 = sxx_lr - syy_lr), no matmuls.
  * Y0 correction form: P_{n} = W^{(n-1)} X_n accumulates in an OPEN PSUM group
    during ring_{n-1} (off critical path); epilogue closes it with the rank-K
    correction  Y0_n = P_n + lr * Y * (U^T X_n), so the master-W update and the
    14 Y0 matmuls leave the serial path entirely.
  * single fp32 master W [H,D]; Wb = cast(W) on scalar; WTb = PE-transpose of Wb
    (bf16 transpose == transpose of bf16 cast, exact) - drops the WT fp32 master
    and its vector-engine update entirely.
  * XTall/Sxxall stored bf16 (Sxx pre-scaled by lr at whiten time).
  * engine rebalance: ring bounces on DVE, z2s/copies on scalar, xc-sub and
    the iter-0 mask-mul on gpsimd (which cannot touch PSUM); whitening +
    masters + P-opens emitted in priority bands (and the master-update tail
    emitted mid-ring of the NEXT block) so the in-order engine streams place
    them behind each block's critical chain.
  * RING_ITERS=4 (validated offline: rel err 1.49e-2 vs the 2e-2 gate;
    RING_ITERS=5 gives 8.7e-3 at ~+60us).
All 8 cores run the identical program (the scan is inherently sequential;
core 0's output is returned).
"""

import os
import sys
from contextlib import ExitStack

sys.path.insert(0, "/opt/trn_rl_repo")

import numpy as np
import ml_dtypes

import concourse.bass as bass
import concourse.mybir as mybir
from concourse.tile import TileContext
from concourse.masks import make_identity
from concourse.bass_utils import run_bass_kernel_spmd
from concourse.vector_clock import ScopedClock

LR = 1e-3
B, D, H, O = 2048, 784, 256, 10
K = 128
NBLK = B // K
DP, DC = 112, 7          # D = 784 = 7 * 112
HP, HC = 128, 2          # H = 256 = 2 * 128
DS = D // 2              # 392: matmul free-dim split for D-wide outputs

RING_ITERS = int(os.environ.get("RING_ITERS", "4"))
LOOKAHEAD = 4
N_CORES = 1

f32 = mybir.dt.float32
bf16 = mybir.dt.bfloat16
AT = mybir.AluOpType


def _install_ntff_hook():
    """The agent image's `antenv` lacks `axon_hooks`, so trace=True degrades.
    Synthesize the module and register the ctypes NTFF hook from trn_boot."""
    import types
    import antenv

    if getattr(antenv, "axon_hooks", None) is not None:
        return
    mod = types.ModuleType("antenv.axon_hooks")
    _hook_box = [None]
    mod.set_axon_ntff_profile_hook = lambda h: _hook_box.__setitem__(0, h)
    mod.get_axon_ntff_profile_hook = lambda: _hook_box[0]
    sys.modules["antenv.axon_hooks"] = mod
    antenv.axon_hooks = mod
    try:
        sys.path.insert(0, "/root/.axon_site")
        from trn_agent_boot.trn_boot import _ntff_profile_via_ctypes

        hook = _ntff_profile_via_ctypes("/opt/axon/libaxon_pjrt.so")
        if hook is not None:
            mod.set_axon_ntff_profile_hook(hook)
    except Exception:
        pass


try:
    _install_ntff_hook()
except Exception:
    pass

_drain_patched = False


def _patch_drain():
    """This walrus build only supports one sync-wait per CTRL instruction;
    split the Tile kernel-tail drain into one drain per semaphore wait."""
    global _drain_patched
    if _drain_patched:
        return

    def patched(self, tick_clock, wait_clock):
        drain_inst = self.nc.sync.drain()
        wait_clock.add_sem_waits(
            drain_inst.ins, ScopedClock({None: tick_clock.global_clock})
        )
        mi = drain_inst.ins
        si = mi.sync_info
        if si is not None and len(si.on_wait) > 1:
            waits = list(si.on_wait)
            mi.sync_info = mybir.SyncInfo(
                on_wait=[waits[0]], on_update=list(si.on_update)
            )
            for w in waits[1:]:
                d2 = self.nc.sync.drain()
                d2.ins.sync_info = mybir.SyncInfo(on_wait=[w], on_update=[])
        self.nc.all_engine_barrier()
        assert self.sems is not None
        popped = self.nc._tile_sem_poison_stack.pop()
        assert popped is self._sem_poison
        self.nc.clear_and_free_semaphores(list(self.sems.allocated().values()))
        self.nc.all_engine_barrier()

    TileContext._drain_and_barrier = patched
    _drain_patched = True


def _split_multiwait(nc, limit=1):
    """This walrus build supports only `limit` sync-waits per instruction.
    Hoist extra waits onto NoOps inserted just before, in the same engine
    stream (engines are in-order, so earlier waits are strictly safe)."""
    n_split = 0
    for f in nc.m.functions:
        for bb in f.blocks:
            insts = list(bb.instructions)
            if not any(
                i.sync_info is not None and len(i.sync_info.on_wait) > limit
                for i in insts
            ):
                continue
            new = []
            for inst in insts:
                si = inst.sync_info
                if si is not None and len(si.on_wait) > limit:
                    waits = list(si.on_wait)
                    for j, w in enumerate(waits[: len(waits) - limit]):
                        nop = mybir.InstNoOp(
                            name=f"{inst.name}-hw{j}", engine=inst.engine,
                            ins=[], outs=[],
                        )
                        nop.sync_info = mybir.SyncInfo(on_wait=[w], on_update=[])
                        new.append(nop)
                        n_split += 1
                    inst.sync_info = mybir.SyncInfo(
                        on_wait=waits[len(waits) - limit:],
                        on_update=list(si.on_update),
                    )
                new.append(inst)
            bb.instructions = new
    return n_split


def build_nc(ring_iters=RING_ITERS):
    _patch_drain()
    nc = bass.Bass()
    x_d = nc.dram_tensor("x", [B, D], f32, kind="ExternalInput")
    mu_d = nc.dram_tensor("mu_b", [128, D], f32, kind="ExternalInput")
    pt_d = nc.dram_tensor("pt", [DP, DC, D], bf16, kind="ExternalInput")
    w_d = nc.dram_tensor("w", [HP, HC * D], f32, kind="ExternalInput")
    wb_d = nc.dram_tensor("w_bf", [HP, HC * D], bf16, kind="ExternalInput")
    wtb_d = nc.dram_tensor("wt_bf", [DP, DC * H], bf16, kind="ExternalInput")
    rt_d = nc.dram_tensor("rt", [HP, HC, O], bf16, kind="ExternalInput")
    bb_d = nc.dram_tensor("b_b", [128, O], f32, kind="ExternalInput")
    out_d = nc.dram_tensor("out", [B, O], f32, kind="ExternalOutput")
    def pri_crit(b):
        # critical path of block b
        return 1000 + b * 1000

    def pri_fill(b):
        # fill work of block b: must rank BELOW crit of b+1 (it runs during
        # ring_{b+1}) but above crit of b+2
        return 1000 + (b + 1) * 1000 + 500

    def pri_whit(b):
        # whiten(b) must complete before epilogue of b-1: rank just below
        # crit(b-1), above fill bands of earlier blocks
        return 1000 + (b - 1) * 1000 + 400

    def pri_out(b):
        return 20_000_000 + b * 1000

    with TileContext(nc) as tc, ExitStack() as ctx:
        persist = ctx.enter_context(tc.tile_pool(name="persist", bufs=1))
        xpool = ctx.enter_context(tc.tile_pool(name="xpool", bufs=5))
        small = ctx.enter_context(tc.tile_pool(name="small", bufs=2))
        psA = ctx.enter_context(tc.tile_pool(name="psA", bufs=3, space="PSUM"))
        psB = ctx.enter_context(tc.tile_pool(name="psB", bufs=3, space="PSUM"))
        psT = ctx.enter_context(tc.tile_pool(name="psT", bufs=2, space="PSUM"))

        ident = persist.tile([128, 128], f32, tag="ident")
        make_identity(nc, ident)
        identb = persist.tile([128, 128], bf16, tag="identb")
        nc.vector.tensor_copy(identb, ident)
        # 0/1 masks (lr is folded into sxx_lr / syy_lr)
        maskSL = persist.tile([K, K], f32, tag="maskSL")
        nc.gpsimd.memset(maskSL, 1.0)
        nc.gpsimd.affine_select(
            out=maskSL, in_=maskSL, compare_op=AT.is_gt, fill=0.0,
            base=0, pattern=[[-1, K]], channel_multiplier=1,
        )
        maskSU = persist.tile([K, K], f32, tag="maskSU")
        nc.gpsimd.memset(maskSU, 1.0)
        nc.vector.tensor_sub(maskSU, maskSU, ident)
        nc.vector.tensor_sub(maskSU, maskSU, maskSL)

        mu_t = persist.tile([128, D], f32, tag="mu")
        nc.sync.dma_start(out=mu_t, in_=mu_d[:, :])
        # pt per-chunk so whiten-0's accumulation can start as chunks land
        pt_t = persist.tile([DP, DC, D], bf16, tag="pt")
        for ic in range(DC):
            nc.sync.dma_start(out=pt_t[:, ic, :], in_=pt_d[:, ic, :])
        WTb = persist.tile([DP, DC * H], bf16, tag="WTb")
        nc.sync.dma_start(out=WTb, in_=wtb_d[:, :])
        Wb = persist.tile([HP, HC * D], bf16, tag="Wb")
        nc.sync.dma_start(out=Wb, in_=wb_d[:, :])
        RT = persist.tile([HP, HC, O], bf16, tag="RT")
        nc.sync.dma_start(out=RT, in_=rt_d[:, :, :])
        bb = persist.tile([128, O], f32, tag="bb")
        nc.sync.dma_start(out=bb, in_=bb_d[:, :])
        # W fp32 master is first needed only at block-0's fill tail
        W = persist.tile([HP, HC * D], f32, tag="W")
        nc.sync.dma_start(out=W, in_=w_d[:, :])

        Xall = persist.tile([DP, NBLK, DC, K], bf16, tag="Xall")
        XTall = persist.tile([K, NBLK, D], bf16, tag="XTall")
        Sxxall = persist.tile([K, NBLK, K], bf16, tag="Sxxall")

        # ---------------- whitening ----------------
        def whiten(bi):
            xt = xpool.tile([128, D], f32, tag="xraw")
            nc.sync.dma_start(out=xt, in_=x_d[bi * K:(bi + 1) * K, :])
            xc = xpool.tile([128, D], f32, tag="xc")
            eng = nc.vector if bi == 0 else nc.gpsimd
            eng.tensor_sub(xc, xt, mu_t)
            # transpose xc as f32 (2cyc/col) and cast in the PSUM->SBUF copy:
            # avoids a separate bf16 cast of xc on the vector engine
            xct = xpool.tile([DP, DC * K], bf16, tag="xct")
            for ic in range(DC):
                tpf = psT.tile([DP, K], f32, tag="tt")
                nc.tensor.transpose(
                    tpf, xc[:, ic * DP:(ic + 1) * DP], ident
                )
                nc.scalar.copy(xct[:, ic * K:(ic + 1) * K], tpf)
            XTb = XTall[:, bi, :]
            for s in range(2):
                ps = psB.tile([K, DS], f32, tag="big")
                for ic in range(DC):
                    nc.tensor.matmul(
                        ps, xct[:, ic * K:(ic + 1) * K],
                        pt_t[:, ic, s * DS:(s + 1) * DS],
                        start=(ic == 0), stop=(ic == DC - 1),
                    )
                nc.vector.tensor_add(
                    XTb[:, s * DS:(s + 1) * DS], ps, xc[:, s * DS:(s + 1) * DS]
                )
            xa = Xall[:, bi, :, :]
            for p in range(3):
                tp = psT.tile([DP, 2 * K], bf16, tag="tt")
                nc.tensor.transpose(
                    tp[:, 0:K], XTb[:, (2 * p) * DP:(2 * p + 1) * DP], identb
                )
                nc.tensor.transpose(
                    tp[:, K:2 * K], XTb[:, (2 * p + 1) * DP:(2 * p + 2) * DP],
                    identb,
                )
                nc.scalar.copy(xa[:, 2 * p:2 * p + 2, :], tp)
            tp = psT.tile([DP, K], bf16, tag="tt")
            nc.tensor.transpose(tp, XTb[:, 6 * DP:7 * DP], identb)
            nc.scalar.copy(xa[:, 6, :], tp)
            ps = psA.tile([K, K], f32, tag="kk")
            for ic in range(DC):
                nc.tensor.matmul(
                    ps, xa[:, ic, :], xa[:, ic, :],
                    start=(ic == 0), stop=(ic == DC - 1),
                )
            nc.scalar.mul(Sxxall[:, bi, :], ps, LR)

        tc.cur_priority = 0
        whiten(0)
        tc.cur_priority = pri_whit(1)
        whiten(1)
        tc.cur_priority = pri_whit(2)
        whiten(2)
        tc.cur_priority = pri_whit(3)
        whiten(3)

        Psb = {}  # block -> SBUF f32 tile [HP, HC*K] holding W^(stale) X_block

        def open_P(nb):
            """Accumulate P_nb = W^(current) X_nb into an SBUF f32 tile."""
            pt_sb = small.tile([HP, HC * K], f32, tag="Psb")
            for hc in range(HC):
                ps = psB.tile([HP, K], f32, tag="big")
                for ic in range(DC):
                    nc.tensor.matmul(
                        ps,
                        WTb[:, ic * H + hc * HP:ic * H + (hc + 1) * HP],
                        Xall[:, nb, ic, :],
                        start=(ic == 0), stop=(ic == DC - 1),
                    )
                nc.scalar.copy(pt_sb[:, hc * K:(hc + 1) * K], ps)
            Psb[nb] = pt_sb

        # P_1 with the initial weights (correction applied in epilogue of block 0).
        # Priority BELOW crit(0): open_P(1) waits on whiten(1), and at higher
        # priority it would head-of-line-block block 0's PE stream.
        tc.cur_priority = 1450
        open_P(1)
        y0q_next = None
        pending_tail = [None]

        for b in range(NBLK):
            # ================= HEAD (critical) =================
            tc.cur_priority = pri_crit(b)
            if b == 0:
                y0q = small.tile([HP, HC * K], bf16, tag="y0")
                for hc in range(HC):
                    ps = psB.tile([HP, K], f32, tag="big")
                    for ic in range(DC):
                        nc.tensor.matmul(
                            ps,
                            WTb[:, ic * H + hc * HP:ic * H + (hc + 1) * HP],
                            Xall[:, 0, ic, :],
                            start=(ic == 0), stop=(ic == DC - 1),
                        )
                    nc.vector.tensor_copy(y0q[:, hc * K:(hc + 1) * K], ps)
            else:
                y0q = y0q_next
            ps_syy = psA.tile([K, K], f32, tag="kk")
            for hc in range(HC):
                nc.tensor.matmul(
                    ps_syy, y0q[:, hc * K:(hc + 1) * K],
                    y0q[:, hc * K:(hc + 1) * K],
                    start=(hc == 0), stop=(hc == HC - 1),
                )
            syy_lr = small.tile([K, K], bf16, tag="syl")
            nc.scalar.mul(syy_lr, ps_syy, LR)
            syy_ng = small.tile([K, K], bf16, tag="syn")
            nc.vector.tensor_scalar_mul(syy_ng, ps_syy, -LR)

            # ---- head fill (off critical path) ----
            tc.cur_priority = pri_fill(b)
            y0t = small.tile([K, H], bf16, tag="y0t")
            for hc in range(HC):
                tp = psT.tile([128, K], bf16, tag="tt")
                nc.tensor.transpose(tp, y0q[:, hc * K:(hc + 1) * K], identb)
                nc.scalar.copy(y0t[:, hc * HP:(hc + 1) * HP], tp)
            # ================= RING (critical) =================
            tc.cur_priority = pri_crit(b)
            sxx = Sxxall[:, b, :]
            s_sb = small.tile([K, K], bf16, tag="s")
            nc.vector.scalar_tensor_tensor(
                s_sb, ps_syy, -LR, sxx, op0=AT.mult, op1=AT.add
            )
            tA0 = small.tile([K, K], bf16, tag="ta")
            nc.gpsimd.tensor_mul(tA0, s_sb, maskSU)
            A = small.tile([K, K], bf16, tag="A")
            nc.vector.tensor_add(A, tA0, ident)
            Bm = small.tile([K, K], bf16, tag="B")
            Bprev = identb
            for m in range(1, ring_iters):
                if m == 2:
                    # emit the previous block's master-update chain here so the
                    # scheduler places it after this block's head/early ring in
                    # every engine stream; t0t must follow it (reads updated Wb)
                    if pending_tail[0] is not None:
                        pending_tail[0]()
                        pending_tail[0] = None
                    tc.cur_priority = pri_fill(b - 1) if b else pri_fill(0)
                    if b + 1 < NBLK:
                        t0t = small.tile([K, D], bf16, tag="t0t")
                        for s in range(2):
                            ps = psB.tile([K, DS], f32, tag="big")
                            for hc in range(HC):
                                nc.tensor.matmul(
                                    ps, y0q[:, hc * K:(hc + 1) * K],
                                    Wb[:, hc * D + s * DS:hc * D + (s + 1) * DS],
                                    start=(hc == 0), stop=(hc == HC - 1),
                                )
                            nc.vector.tensor_copy(
                                t0t[:, s * DS:(s + 1) * DS], ps
                            )
                    tc.cur_priority = pri_crit(b)
                z2 = psA.tile([K, K], f32, tag="kk")
                nc.tensor.matmul(z2, syy_ng, A, start=True, stop=True)
                z2s = small.tile([K, K], bf16, tag="z2")
                nc.scalar.copy(z2s, z2)
                r1 = psA.tile([K, K], f32, tag="kk")
                nc.tensor.matmul(r1, A, syy_lr, start=True, stop=True)
                s_sb = small.tile([K, K], bf16, tag="s")
                nc.vector.tensor_sub(s_sb, sxx, r1)
                g = psA.tile([K, K], f32, tag="kk")
                nc.tensor.matmul(g, A, z2s, start=True, stop=True)
                gm = small.tile([K, K], bf16, tag="gm")
                nc.vector.tensor_mul(gm, g, maskSL)
                b1 = psA.tile([K, K], f32, tag="kk")
                nc.tensor.matmul(b1, gm, Bprev, start=True, stop=True)
                nc.vector.tensor_add(Bm, b1, ident)
                Bprev = Bm
                ct = psA.tile([K, K], f32, tag="kk")
                nc.tensor.matmul(ct, s_sb, Bm, start=True, stop=True)
                nt = small.tile([K, K], bf16, tag="nt")
                nc.vector.tensor_mul(nt, ct, maskSL)
                a1 = psA.tile([K, K], f32, tag="kk")
                nc.tensor.matmul(a1, nt, A, start=True, stop=True)
                nc.vector.tensor_add(A, a1, ident)
            # final B-update (B_R from A_R), epilogue A-work interleaved to
            # fill PE gaps while z2s/gm bounce on scalar/vector.
            # The last block needs only its logits: skip B_R/yt/q/ut entirely.
            if b + 1 < NBLK:
                # q = XT - A^T t0t FIRST (only needs A): its halves feed the
                # qT transposes and the qX accumulation, which then overlap
                # the final B-chain; C' = B_R^T qX is the only matmul left on
                # the B_R -> clr path.
                z2 = psA.tile([K, K], f32, tag="kk")
                nc.tensor.matmul(z2, syy_ng, A, start=True, stop=True)
                z2s = small.tile([K, K], bf16, tag="z2")
                nc.scalar.copy(z2s, z2)
                ps_q0 = psB.tile([K, DS], f32, tag="big")
                nc.tensor.matmul(ps_q0, A, t0t[:, 0:DS], start=True, stop=True)
                q = small.tile([K, D], bf16, tag="q")
                nc.vector.tensor_sub(q[:, 0:DS], XTall[:, b, 0:DS], ps_q0)
                g = psA.tile([K, K], f32, tag="kk")
                nc.tensor.matmul(g, A, z2s, start=True, stop=True)
                gm = small.tile([K, K], bf16, tag="gm")
                nc.vector.tensor_mul(gm, g, maskSL)
                ps_q1 = psB.tile([K, DS], f32, tag="big")
                nc.tensor.matmul(ps_q1, A, t0t[:, DS:D], start=True, stop=True)
                nc.vector.tensor_sub(q[:, DS:D], XTall[:, b, DS:D], ps_q1)
                qT = small.tile([DP, DC * K], bf16, tag="U")
                tp = psA.tile([DP, 2 * K], bf16, tag="kk")
                nc.tensor.transpose(tp[:, 0:K], q[:, 0:DP], identb)
                nc.tensor.transpose(tp[:, K:2 * K], q[:, DP:2 * DP], identb)
                nc.scalar.copy(qT[:, 0:2 * K], tp)
                b1 = psA.tile([K, K], f32, tag="kk")
                nc.tensor.matmul(b1, gm, Bprev, start=True, stop=True)
                nc.vector.tensor_add(Bm, b1, ident)
                ps_yt = psB.tile([K, H], f32, tag="big")
                nc.tensor.matmul(ps_yt, A, y0t, start=True, stop=True)
                yt = small.tile([K, H], bf16, tag="yt")
                nc.vector.tensor_copy(yt, ps_yt)
                for p in range(1, 3):
                    tp = psA.tile([DP, 2 * K], bf16, tag="kk")
                    nc.tensor.transpose(
                        tp[:, 0:K], q[:, (2 * p) * DP:(2 * p + 1) * DP], identb
                    )
                    nc.tensor.transpose(
                        tp[:, K:2 * K], q[:, (2 * p + 1) * DP:(2 * p + 2) * DP],
                        identb,
                    )
                    nc.scalar.copy(qT[:, (2 * p) * K:(2 * p + 2) * K], tp)
                tp = psA.tile([DP, K], bf16, tag="kk")
                nc.tensor.transpose(tp, q[:, 6 * DP:7 * DP], identb)
                nc.scalar.copy(qT[:, 6 * K:7 * K], tp)
                psqx = psA.tile([K, K], f32, tag="kk")
                for ic in range(DC):
                    nc.tensor.matmul(
                        psqx, qT[:, ic * K:(ic + 1) * K], Xall[:, b + 1, ic, :],
                        start=(ic == 0), stop=(ic == DC - 1),
                    )
                qxs = small.tile([K, K], bf16, tag="qx")
                nc.scalar.copy(qxs, psqx)
                psc = psA.tile([K, K], f32, tag="kk")
                nc.tensor.matmul(psc, Bm, qxs, start=True, stop=True)
                clr = small.tile([K, K], bf16, tag="clr")
                nc.scalar.mul(clr, psc, LR)
                y0q_next = small.tile([HP, HC * K], bf16, tag="y0")
                for hc in range(HC):
                    cps = psA.tile([K, K], f32, tag="kk")
                    nc.tensor.matmul(
                        cps, yt[:, hc * HP:(hc + 1) * HP], clr,
                        start=True, stop=True,
                    )
                    nc.vector.tensor_add(
                        y0q_next[:, hc * K:(hc + 1) * K], cps,
                        Psb[b + 1][:, hc * K:(hc + 1) * K],
                    )
                ut = small.tile([K, D], bf16, tag="ut")
                for s in range(2):
                    ps = psB.tile([K, DS], f32, tag="big")
                    nc.tensor.matmul(
                        ps, Bm, q[:, s * DS:(s + 1) * DS], start=True, stop=True
                    )
                    nc.scalar.copy(ut[:, s * DS:(s + 1) * DS], ps)

            # ================= FILL TAIL =================
            tc.cur_priority = pri_fill(b)
            relu_y = small.tile([HP, HC * K], bf16, tag="ry")
            for hc in range(HC):
                ps2 = psA.tile([K, K], f32, tag="kk")
                nc.tensor.matmul(
                    ps2, y0t[:, hc * HP:(hc + 1) * HP], A, start=True, stop=True
                )
                nc.scalar.activation(
                    relu_y[:, hc * K:(hc + 1) * K], ps2,
                    mybir.ActivationFunctionType.Relu,
                )
            lg = psA.tile([K, O], f32, tag="kk")
            for hc in range(HC):
                nc.tensor.matmul(
                    lg, relu_y[:, hc * K:(hc + 1) * K], RT[:, hc, :],
                    start=(hc == 0), stop=(hc == HC - 1),
                )
            lgs = small.tile([K, O], f32, tag="lgs")
            tc.cur_priority = pri_out(b)
            nc.vector.tensor_add(lgs, lg, bb)
            nc.sync.dma_start(out=out_d[b * K:(b + 1) * K, :], in_=lgs)
            tc.cur_priority = pri_fill(b)

            if b + LOOKAHEAD < NBLK:
                tc.cur_priority = pri_whit(b + LOOKAHEAD)
                whiten(b + LOOKAHEAD)
                tc.cur_priority = pri_fill(b)
            if b + 1 < NBLK:
                # masters: W += lr * Y U^T (fp32, in place); Wb = cast(W);
                # WTb = PE-transpose of Wb. Deferred: emitted inside the NEXT
                # block's ring so engine streams order it behind that block's
                # critical head.
                def make_tail(b, yt, ut):
                    def tail():
                        tc.cur_priority = pri_fill(b)
                        for hc in range(HC):
                            for s in range(2):
                                ps = psB.tile([HP, DS], f32, tag="big")
                                nc.tensor.matmul(
                                    ps, yt[:, hc * HP:(hc + 1) * HP],
                                    ut[:, s * DS:(s + 1) * DS],
                                    start=True, stop=True,
                                )
                                wsl = W[:, hc * D + s * DS:hc * D + (s + 1) * DS]
                                nc.vector.scalar_tensor_tensor(
                                    wsl, ps, LR, wsl, op0=AT.mult, op1=AT.add
                                )
                                nc.scalar.copy(
                                    Wb[:, hc * D + s * DS:hc * D + (s + 1) * DS],
                                    wsl,
                                )
                        for dc in range(DC):
                            tp = psT.tile([DP, 2 * K], bf16, tag="tt")
                            nc.tensor.transpose(
                                tp[:, 0:HP],
                                Wb[:, 0 * D + dc * DP:0 * D + (dc + 1) * DP],
                                identb,
                            )
                            nc.tensor.transpose(
                                tp[:, HP:2 * HP],
                                Wb[:, 1 * D + dc * DP:1 * D + (dc + 1) * DP],
                                identb,
                            )
                            nc.scalar.copy(WTb[:, dc * H:(dc + 1) * H], tp)
                        if b + 2 < NBLK:
                            open_P(b + 2)
                    return tail

                pending_tail[0] = make_tail(b, yt, ut)

    _split_multiwait(nc)
    return nc


def prep_inputs(x, whiten_mean, whiten_mat, oja_W, readout_W, readout_b):
    """Host-side layout/dtype prep (no contractions)."""
    x = np.ascontiguousarray(x, dtype=np.float32)
    mu_b = np.broadcast_to(
        np.asarray(whiten_mean, dtype=np.float32)[None, :], (128, D)
    ).copy()
    P = np.asarray(whiten_mat, dtype=np.float32) - np.eye(D, dtype=np.float32)
    # pt[dp, ic, dout] = P^T[ic*112+dp, dout] = P[dout, ic*112+dp]
    pt = np.ascontiguousarray(
        P.T.reshape(DC, DP, D).transpose(1, 0, 2).astype(ml_dtypes.bfloat16)
    )
    Wf = np.asarray(oja_W, dtype=np.float32)
    w = np.ascontiguousarray(
        Wf.reshape(HC, HP, D).transpose(1, 0, 2).reshape(HP, HC * D)
    )
    wtb = np.ascontiguousarray(
        Wf.T.reshape(DC, DP, H).transpose(1, 0, 2).reshape(DP, DC * H)
    ).astype(ml_dtypes.bfloat16)
    Rf = np.asarray(readout_W, dtype=np.float32)
    rt = np.ascontiguousarray(
        Rf.T.reshape(HC, HP, O).transpose(1, 0, 2).astype(ml_dtypes.bfloat16)
    )
    b_b = np.broadcast_to(
        np.asarray(readout_b, dtype=np.float32)[None, :], (128, O)
    ).copy()
    return {
        "x": x, "mu_b": mu_b, "pt": pt, "w": w, "rt": rt, "b_b": b_b,
        "w_bf": w.astype(ml_dtypes.bfloat16), "wt_bf": wtb,
    }


_cached_nc = None


def _get_nc():
    global _cached_nc
    if _cached_nc is None:
        _cached_nc = build_nc()
    return _cached_nc


def kernel(x, whiten_mean, whiten_mat, oja_W, readout_W, readout_b, **run_kwargs):
    nc = _get_nc()
    ins = prep_inputs(x, whiten_mean, whiten_mat, oja_W, readout_W, readout_b)
    res = run_bass_kernel_spmd(
        nc, [ins] * N_CORES, core_ids=list(range(N_CORES)), **run_kwargs
    )
    out = res.results[0]["out"]
    if run_kwargs:
        kernel.last_result = res
    return out


# revision 28
# speedup vs baseline: 1.1250x; 1.0037x over previous
"""Trainium2 Bass kernel for nn_BioClassifier: whitening + sequential Oja scan + readout.

v2: restructured for critical-path latency. Same block-parallel-scan math as v1
(chunk the 2048-sample Oja scan into 16 blocks of K=128; per block a fixed-point
"ring" on K x K matrices closes the sequential recurrence exactly):
    Y = Y0 A,  U = (X - T0 A) B,   A = (I - lr*SU(C))^-1, B = (I + lr*SU(G))^-1
    C = U^T X, G = Y^T Y,  T0^T X = Syy
Key v2 changes vs v1:
  * lr folded into sxx_lr/syy_lr (bf16), iteration reordered so each ring cycle
    is 10 serial engine-hops (B-chain: z2->z2s->g->gm->b1->B; A-chain: ct->nt->
    a1->A) with r1/s prep hidden under the B-chain; iter-0 A-update is 3 vec ops
    (A1 = I + SU(s0), s0 = sxx_lr - syy_lr), no matmuls.
  * Y0 correction form: P_{n} = W^{(n-1)} X_n accumulates in an OPEN PSUM group
    during ring_{n-1} (off critical path); epilogue closes it with the rank-K
    correction  Y0_n = P_n + lr * Y * (U^T X_n), so the master-W update and the
    14 Y0 matmuls leave the serial path entirely.
  * single fp32 master W [H,D]; Wb = cast(W) on scalar; WTb = PE-transpose of Wb
    (bf16 transpose == transpose of bf16 cast, exact) - drops the WT fp32 master
    and its vector-engine update entirely.
  * XTall/Sxxall stored bf16 (Sxx pre-scaled by lr at whiten time).
  * engine rebalance: ring bounces on DVE, z2s/copies on scalar, xc-sub and
    the iter-0 mask-mul on gpsimd (which cannot touch PSUM); whitening +
    masters + P-opens emitted in priority bands (and the master-update tail
    emitted mid-ring of the NEXT block) so the in-order engine streams place
    them behind each block's critical chain.
  * RING_ITERS=4 (validated offline: rel err 1.49e-2 vs the 2e-2 gate;
    RING_ITERS=5 gives 8.7e-3 at ~+60us).
All 8 cores run the identical program (the scan is inherently sequential;
core 0's output is returned).
"""

import os
import sys
from contextlib import ExitStack

sys.path.insert(0, "/opt/trn_rl_repo")

import numpy as np
import ml_dtypes

import concourse.bass as bass
import concourse.mybir as mybir
from concourse.tile import TileContext
from concourse.masks import make_identity
from concourse.bass_utils import run_bass_kernel_spmd
from concourse.vector_clock import ScopedClock

LR = 1e-3
B, D, H, O = 2048, 784, 256, 10
K = 128
NBLK = B // K
DP, DC = 112, 7          # D = 784 = 7 * 112
HP, HC = 128, 2          # H = 256 = 2 * 128
DS = D // 2              # 392: matmul free-dim split for D-wide outputs

RING_ITERS = int(os.environ.get("RING_ITERS", "4"))
LOOKAHEAD = 4
N_CORES = 1

f32 = mybir.dt.float32
bf16 = mybir.dt.bfloat16
AT = mybir.AluOpType


def _install_ntff_hook():
    """The agent image's `antenv` lacks `axon_hooks`, so trace=True degrades.
    Synthesize the module and register the ctypes NTFF hook from trn_boot."""
    import types
    import antenv

    if getattr(antenv, "axon_hooks", None) is not None:
        return
    mod = types.ModuleType("antenv.axon_hooks")
    _hook_box = [None]
    mod.set_axon_ntff_profile_hook = lambda h: _hook_box.__setitem__(0, h)
    mod.get_axon_ntff_profile_hook = lambda: _hook_box[0]
    sys.modules["antenv.axon_hooks"] = mod
    antenv.axon_hooks = mod
    try:
        sys.path.insert(0, "/root/.axon_site")
        from trn_agent_boot.trn_boot import _ntff_profile_via_ctypes

        hook = _ntff_profile_via_ctypes("/opt/axon/libaxon_pjrt.so")
        if hook is not None:
            mod.set_axon_ntff_profile_hook(hook)
    except Exception:
        pass


try:
    _install_ntff_hook()
except Exception:
    pass

_drain_patched = False


def _patch_drain():
    """This walrus build only supports one sync-wait per CTRL instruction;
    split the Tile kernel-tail drain into one drain per semaphore wait."""
    global _drain_patched
    if _drain_patched:
        return

    def patched(self, tick_clock, wait_clock):
        drain_inst = self.nc.sync.drain()
        wait_clock.add_sem_waits(
            drain_inst.ins, ScopedClock({None: tick_clock.global_clock})
        )
        mi = drain_inst.ins
        si = mi.sync_info
        if si is not None and len(si.on_wait) > 1:
            waits = list(si.on_wait)
            mi.sync_info = mybir.SyncInfo(
                on_wait=[waits[0]], on_update=list(si.on_update)
            )
            for w in waits[1:]:
                d2 = self.nc.sync.drain()
                d2.ins.sync_info = mybir.SyncInfo(on_wait=[w], on_update=[])
        self.nc.all_engine_barrier()
        assert self.sems is not None
        popped = self.nc._tile_sem_poison_stack.pop()
        assert popped is self._sem_poison
        self.nc.clear_and_free_semaphores(list(self.sems.allocated().values()))
        self.nc.all_engine_barrier()

    TileContext._drain_and_barrier = patched
    _drain_patched = True


def _split_multiwait(nc, limit=1):
    """This walrus build supports only `limit` sync-waits per instruction.
    Hoist extra waits onto NoOps inserted just before, in the same engine
    stream (engines are in-order, so earlier waits are strictly safe)."""
    n_split = 0
    for f in nc.m.functions:
        for bb in f.blocks:
            insts = list(bb.instructions)
            if not any(
                i.sync_info is not None and len(i.sync_info.on_wait) > limit
                for i in insts
            ):
                continue
            new = []
            for inst in insts:
                si = inst.sync_info
                if si is not None and len(si.on_wait) > limit:
                    waits = list(si.on_wait)
                    for j, w in enumerate(waits[: len(waits) - limit]):
                        nop = mybir.InstNoOp(
                            name=f"{inst.name}-hw{j}", engine=inst.engine,
                            ins=[], outs=[],
                        )
                        nop.sync_info = mybir.SyncInfo(on_wait=[w], on_update=[])
                        new.append(nop)
                        n_split += 1
                    inst.sync_info = mybir.SyncInfo(
                        on_wait=waits[len(waits) - limit:],
                        on_update=list(si.on_update),
                    )
                new.append(inst)
            bb.instructions = new
    return n_split


def build_nc(ring_iters=RING_ITERS):
    _patch_drain()
    nc = bass.Bass()
    x_d = nc.dram_tensor("x", [B, D], f32, kind="ExternalInput")
    mu_d = nc.dram_tensor("mu_b", [128, D], f32, kind="ExternalInput")
    pt_d = nc.dram_tensor("pt", [DP, DC, D], bf16, kind="ExternalInput")
    w_d = nc.dram_tensor("w", [HP, HC * D], f32, kind="ExternalInput")
    wb_d = nc.dram_tensor("w_bf", [HP, HC * D], bf16, kind="ExternalInput")
    wtb_d = nc.dram_tensor("wt_bf", [DP, DC * H], bf16, kind="ExternalInput")
    rt_d = nc.dram_tensor("rt", [HP, HC, O], bf16, kind="ExternalInput")
    bb_d = nc.dram_tensor("b_b", [128, O], f32, kind="ExternalInput")
    out_d = nc.dram_tensor("out", [B, O], f32, kind="ExternalOutput")
    def pri_crit(b):
        # critical path of block b
        return 1000 + b * 1000

    def pri_fill(b):
        # fill work of block b: must rank BELOW crit of b+1 (it runs during
        # ring_{b+1}) but above crit of b+2
        return 1000 + (b + 1) * 1000 + 500

    def pri_whit(b):
        # whiten(b) must complete before epilogue of b-1: rank just below
        # crit(b-1), above fill bands of earlier blocks
        return 1000 + (b - 1) * 1000 + 400

    def pri_out(b):
        return 20_000_000 + b * 1000

    with TileContext(nc) as tc, ExitStack() as ctx:
        persist = ctx.enter_context(tc.tile_pool(name="persist", bufs=1))
        xpool = ctx.enter_context(tc.tile_pool(name="xpool", bufs=5))
        small = ctx.enter_context(tc.tile_pool(name="small", bufs=2))
        psA = ctx.enter_context(tc.tile_pool(name="psA", bufs=3, space="PSUM"))
        psB = ctx.enter_context(tc.tile_pool(name="psB", bufs=3, space="PSUM"))
        psT = ctx.enter_context(tc.tile_pool(name="psT", bufs=2, space="PSUM"))

        ident = persist.tile([128, 128], f32, tag="ident")
        make_identity(nc, ident)
        identb = persist.tile([128, 128], bf16, tag="identb")
        nc.vector.tensor_copy(identb, ident)
        # 0/1 masks (lr is folded into sxx_lr / syy_lr)
        maskSL = persist.tile([K, K], f32, tag="maskSL")
        nc.gpsimd.memset(maskSL, 1.0)
        nc.gpsimd.affine_select(
            out=maskSL, in_=maskSL, compare_op=AT.is_gt, fill=0.0,
            base=0, pattern=[[-1, K]], channel_multiplier=1,
        )
        maskSU = persist.tile([K, K], f32, tag="maskSU")
        nc.gpsimd.memset(maskSU, 1.0)
        nc.vector.tensor_sub(maskSU, maskSU, ident)
        nc.vector.tensor_sub(maskSU, maskSU, maskSL)

        mu_t = persist.tile([128, D], f32, tag="mu")
        nc.sync.dma_start(out=mu_t, in_=mu_d[:, :])
        # pt per-chunk so whiten-0's accumulation can start as chunks land
        pt_t = persist.tile([DP, DC, D], bf16, tag="pt")
        for ic in range(DC):
            nc.sync.dma_start(out=pt_t[:, ic, :], in_=pt_d[:, ic, :])
        WTb = persist.tile([DP, DC * H], bf16, tag="WTb")
        nc.sync.dma_start(out=WTb, in_=wtb_d[:, :])
        Wb = persist.tile([HP, HC * D], bf16, tag="Wb")
        nc.sync.dma_start(out=Wb, in_=wb_d[:, :])
        RT = persist.tile([HP, HC, O], bf16, tag="RT")
        nc.sync.dma_start(out=RT, in_=rt_d[:, :, :])
        bb = persist.tile([128, O], f32, tag="bb")
        nc.sync.dma_start(out=bb, in_=bb_d[:, :])
        # W fp32 master is first needed only at block-0's fill tail
        W = persist.tile([HP, HC * D], f32, tag="W")
        nc.sync.dma_start(out=W, in_=w_d[:, :])

        Xall = persist.tile([DP, NBLK, DC, K], bf16, tag="Xall")
        XTall = persist.tile([K, NBLK, D], bf16, tag="XTall")
        Sxxall = persist.tile([K, NBLK, K], bf16, tag="Sxxall")

        # ---------------- whitening ----------------
        def whiten(bi):
            xt = xpool.tile([128, D], f32, tag="xraw")
            nc.sync.dma_start(out=xt, in_=x_d[bi * K:(bi + 1) * K, :])
            xc = xpool.tile([128, D], f32, tag="xc")
            eng = nc.vector if bi == 0 else nc.gpsimd
            eng.tensor_sub(xc, xt, mu_t)
            # transpose xc as f32 (2cyc/col) and cast in the PSUM->SBUF copy:
            # avoids a separate bf16 cast of xc on the vector engine
            xct = xpool.tile([DP, DC * K], bf16, tag="xct")
            for ic in range(DC):
                tpf = psT.tile([DP, K], f32, tag="tt")
                nc.tensor.transpose(
                    tpf, xc[:, ic * DP:(ic + 1) * DP], ident
                )
                nc.scalar.copy(xct[:, ic * K:(ic + 1) * K], tpf)
            XTb = XTall[:, bi, :]
            for s in range(2):
                ps = psB.tile([K, DS], f32, tag="big")
                for ic in range(DC):
                    nc.tensor.matmul(
                        ps, xct[:, ic * K:(ic + 1) * K],
                        pt_t[:, ic, s * DS:(s + 1) * DS],
                        start=(ic == 0), stop=(ic == DC - 1),
                    )
                nc.vector.tensor_add(
                    XTb[:, s * DS:(s + 1) * DS], ps, xc[:, s * DS:(s + 1) * DS]
                )
            xa = Xall[:, bi, :, :]
            for p in range(3):
                tp = psT.tile([DP, 2 * K], bf16, tag="tt")
                nc.tensor.transpose(
                    tp[:, 0:K], XTb[:, (2 * p) * DP:(2 * p + 1) * DP], identb
                )
                nc.tensor.transpose(
                    tp[:, K:2 * K], XTb[:, (2 * p + 1) * DP:(2 * p + 2) * DP],
                    identb,
                )
                nc.scalar.copy(xa[:, 2 * p:2 * p + 2, :], tp)
            tp = psT.tile([DP, K], bf16, tag="tt")
            nc.tensor.transpose(tp, XTb[:, 6 * DP:7 * DP], identb)
            nc.scalar.copy(xa[:, 6, :], tp)
            ps = psA.tile([K, K], f32, tag="kk")
            for ic in range(DC):
                nc.tensor.matmul(
                    ps, xa[:, ic, :], xa[:, ic, :],
                    start=(ic == 0), stop=(ic == DC - 1),
                )
            nc.scalar.mul(Sxxall[:, bi, :], ps, LR)

        tc.cur_priority = 0
        whiten(0)
        tc.cur_priority = pri_whit(1)
        whiten(1)
        tc.cur_priority = pri_whit(2)
        whiten(2)
        tc.cur_priority = pri_whit(3)
        whiten(3)

        Psb = {}  # block -> SBUF f32 tile [HP, HC*K] holding W^(stale) X_block

        def open_P(nb):
            """Accumulate P_nb = W^(current) X_nb into an SBUF f32 tile."""
            pt_sb = small.tile([HP, HC * K], f32, tag="Psb")
            for hc in range(HC):
                ps = psB.tile([HP, K], f32, tag="big")
                for ic in range(DC):
                    nc.tensor.matmul(
                        ps,
                        WTb[:, ic * H + hc * HP:ic * H + (hc + 1) * HP],
                        Xall[:, nb, ic, :],
                        start=(ic == 0), stop=(ic == DC - 1),
                    )
                nc.scalar.copy(pt_sb[:, hc * K:(hc + 1) * K], ps)
            Psb[nb] = pt_sb

        # P_1 with the initial weights (correction applied in epilogue of block 0).
        # Priority BELOW crit(0): open_P(1) waits on whiten(1), and at higher
        # priority it would head-of-line-block block 0's PE stream.
        tc.cur_priority = 1450
        open_P(1)
        y0q_next = None
        pending_tail = [None]

        for b in range(NBLK):
            # ================= HEAD (critical) =================
            tc.cur_priority = pri_crit(b)
            if b == 0:
                y0q = small.tile([HP, HC * K], bf16, tag="y0")
                for hc in range(HC):
                    ps = psB.tile([HP, K], f32, tag="big")
                    for ic in range(DC):
                        nc.tensor.matmul(
                            ps,
                            WTb[:, ic * H + hc * HP:ic * H + (hc + 1) * HP],
                            Xall[:, 0, ic, :],
                            start=(ic == 0), stop=(ic == DC - 1),
                        )
                    nc.vector.tensor_copy(y0q[:, hc * K:(hc + 1) * K], ps)
            else:
                y0q = y0q_next
            ps_syy = psA.tile([K, K], f32, tag="kk")
            for hc in range(HC):
                nc.tensor.matmul(
                    ps_syy, y0q[:, hc * K:(hc + 1) * K],
                    y0q[:, hc * K:(hc + 1) * K],
                    start=(hc == 0), stop=(hc == HC - 1),
                )
            syy_lr = small.tile([K, K], bf16, tag="syl")
            syy_ng = small.tile([K, K], bf16, tag="syn")

            # ---- head fill (off critical path) ----
            tc.cur_priority = pri_fill(b)
            y0t = small.tile([K, H], bf16, tag="y0t")
            for hc in range(HC):
                tp = psT.tile([128, K], bf16, tag="tt")
                nc.tensor.transpose(tp, y0q[:, hc * K:(hc + 1) * K], identb)
                nc.scalar.copy(y0t[:, hc * HP:(hc + 1) * HP], tp)
            # ================= RING (critical) =================
            tc.cur_priority = pri_crit(b)
            sxx = Sxxall[:, b, :]
            s_sb = small.tile([K, K], bf16, tag="s")
            nc.vector.scalar_tensor_tensor(
                s_sb, ps_syy, -LR, sxx, op0=AT.mult, op1=AT.add
            )
            tA0 = small.tile([K, K], bf16, tag="ta")
            nc.gpsimd.tensor_mul(tA0, s_sb, maskSU)
            A = small.tile([K, K], bf16, tag="A")
            nc.vector.tensor_add(A, tA0, ident)
            nc.scalar.mul(syy_lr, ps_syy, LR)
            nc.vector.tensor_scalar_mul(syy_ng, ps_syy, -LR)
            Bm = small.tile([K, K], bf16, tag="B")
            Bprev = identb
            for m in range(1, ring_iters):
                if m == 2:
                    # emit the previous block's master-update chain here so the
                    # scheduler places it after this block's head/early ring in
                    # every engine stream; t0t must follow it (reads updated Wb)
                    if pending_tail[0] is not None:
                        pending_tail[0]()
                        pending_tail[0] = None
                    tc.cur_priority = pri_fill(b - 1) if b else pri_fill(0)
                    if b + 1 < NBLK:
                        t0t = small.tile([K, D], bf16, tag="t0t")
                        for s in range(2):
                            ps = psB.tile([K, DS], f32, tag="big")
                            for hc in range(HC):
                                nc.tensor.matmul(
                                    ps, y0q[:, hc * K:(hc + 1) * K],
                                    Wb[:, hc * D + s * DS:hc * D + (s + 1) * DS],
                                    start=(hc == 0), stop=(hc == HC - 1),
                                )
                            nc.vector.tensor_copy(
                                t0t[:, s * DS:(s + 1) * DS], ps
                            )
                    tc.cur_priority = pri_crit(b)
                z2 = psA.tile([K, K], f32, tag="kk")
                nc.tensor.matmul(z2, syy_ng, A, start=True, stop=True)
                z2s = small.tile([K, K], bf16, tag="z2")
                nc.scalar.copy(z2s, z2)
                r1 = psA.tile([K, K], f32, tag="kk")
                nc.tensor.matmul(r1, A, syy_lr, start=True, stop=True)
                s_sb = small.tile([K, K], bf16, tag="s")
                nc.vector.tensor_sub(s_sb, sxx, r1)
                g = psA.tile([K, K], f32, tag="kk")
                nc.tensor.matmul(g, A, z2s, start=True, stop=True)
                gm = small.tile([K, K], bf16, tag="gm")
                nc.vector.tensor_mul(gm, g, maskSL)
                b1 = psA.tile([K, K], f32, tag="kk")
                nc.tensor.matmul(b1, gm, Bprev, start=True, stop=True)
                nc.vector.tensor_add(Bm, b1, ident)
                Bprev = Bm
                ct = psA.tile([K, K], f32, tag="kk")
                nc.tensor.matmul(ct, s_sb, Bm, start=True, stop=True)
                nt = small.tile([K, K], bf16, tag="nt")
                nc.vector.tensor_mul(nt, ct, maskSL)
                a1 = psA.tile([K, K], f32, tag="kk")
                nc.tensor.matmul(a1, nt, A, start=True, stop=True)
                nc.vector.tensor_add(A, a1, ident)
            # final B-update (B_R from A_R), epilogue A-work interleaved to
            # fill PE gaps while z2s/gm bounce on scalar/vector.
            # The last block needs only its logits: skip B_R/yt/q/ut entirely.
            if b + 1 < NBLK:
                # q = XT - A^T t0t FIRST (only needs A): its halves feed the
                # qT transposes and the qX accumulation, which then overlap
                # the final B-chain; C' = B_R^T qX is the only matmul left on
                # the B_R -> clr path.
                z2 = psA.tile([K, K], f32, tag="kk")
                nc.tensor.matmul(z2, syy_ng, A, start=True, stop=True)
                z2s = small.tile([K, K], bf16, tag="z2")
                nc.scalar.copy(z2s, z2)
                ps_q0 = psB.tile([K, DS], f32, tag="big")
                nc.tensor.matmul(ps_q0, A, t0t[:, 0:DS], start=True, stop=True)
                q = small.tile([K, D], bf16, tag="q")
                nc.vector.tensor_sub(q[:, 0:DS], XTall[:, b, 0:DS], ps_q0)
                g = psA.tile([K, K], f32, tag="kk")
                nc.tensor.matmul(g, A, z2s, start=True, stop=True)
                gm = small.tile([K, K], bf16, tag="gm")
                nc.vector.tensor_mul(gm, g, maskSL)
                ps_q1 = psB.tile([K, DS], f32, tag="big")
                nc.tensor.matmul(ps_q1, A, t0t[:, DS:D], start=True, stop=True)
                nc.vector.tensor_sub(q[:, DS:D], XTall[:, b, DS:D], ps_q1)
                # transpose q's first half (chunks 0-2, cols < DS) and start
                # the qX accumulation on them while the second q-half is still
                # in flight; finish with chunks 3-6 after sub1
                qT = small.tile([DP, DC * K], bf16, tag="U")
                tp = psA.tile([DP, 2 * K], bf16, tag="kk")
                nc.tensor.transpose(tp[:, 0:K], q[:, 0:DP], identb)
                nc.tensor.transpose(tp[:, K:2 * K], q[:, DP:2 * DP], identb)
                nc.scalar.copy(qT[:, 0:2 * K], tp)
                tp = psA.tile([DP, K], bf16, tag="kk")
                nc.tensor.transpose(tp, q[:, 2 * DP:3 * DP], identb)
                nc.scalar.copy(qT[:, 2 * K:3 * K], tp)
                b1 = psA.tile([K, K], f32, tag="kk")
                nc.tensor.matmul(b1, gm, Bprev, start=True, stop=True)
                nc.vector.tensor_add(Bm, b1, ident)
                psqx = psA.tile([K, K], f32, tag="kk")
                for ic in range(3):
                    nc.tensor.matmul(
                        psqx, qT[:, ic * K:(ic + 1) * K], Xall[:, b + 1, ic, :],
                        start=(ic == 0), stop=False,
                    )
                ps_yt = psB.tile([K, H], f32, tag="big")
                nc.tensor.matmul(ps_yt, A, y0t, start=True, stop=True)
                yt = small.tile([K, H], bf16, tag="yt")
                nc.vector.tensor_copy(yt, ps_yt)
                for p in (3, 5):
                    tp = psA.tile([DP, 2 * K], bf16, tag="kk")
                    nc.tensor.transpose(
                        tp[:, 0:K], q[:, p * DP:(p + 1) * DP], identb
                    )
                    nc.tensor.transpose(
                        tp[:, K:2 * K], q[:, (p + 1) * DP:(p + 2) * DP],
                        identb,
                    )
                    nc.scalar.copy(qT[:, p * K:(p + 2) * K], tp)
                for ic in range(3, DC):
                    nc.tensor.matmul(
                        psqx, qT[:, ic * K:(ic + 1) * K], Xall[:, b + 1, ic, :],
                        start=False, stop=(ic == DC - 1),
                    )
                qxs = small.tile([K, K], bf16, tag="qx")
                nc.scalar.copy(qxs, psqx)
                psc = psA.tile([K, K], f32, tag="kk")
                nc.tensor.matmul(psc, Bm, qxs, start=True, stop=True)
                clr = small.tile([K, K], bf16, tag="clr")
                nc.scalar.mul(clr, psc, LR)
                y0q_next = small.tile([HP, HC * K], bf16, tag="y0")
                for hc in range(HC):
                    cps = psA.tile([K, K], f32, tag="kk")
                    nc.tensor.matmul(
                        cps, yt[:, hc * HP:(hc + 1) * HP], clr,
                        start=True, stop=True,
                    )
                    nc.vector.tensor_add(
                        y0q_next[:, hc * K:(hc + 1) * K], cps,
                        Psb[b + 1][:, hc * K:(hc + 1) * K],
                    )
                ut = small.tile([K, D], bf16, tag="ut")
                for s in range(2):
                    ps = psB.tile([K, DS], f32, tag="big")
                    nc.tensor.matmul(
                        ps, Bm, q[:, s * DS:(s + 1) * DS], start=True, stop=True
                    )
                    nc.scalar.copy(ut[:, s * DS:(s + 1) * DS], ps)

            # ================= FILL TAIL =================
            tc.cur_priority = pri_fill(b)
            relu_y = small.tile([HP, HC * K], bf16, tag="ry")
            for hc in range(HC):
                ps2 = psA.tile([K, K], f32, tag="kk")
                nc.tensor.matmul(
                    ps2, y0t[:, hc * HP:(hc + 1) * HP], A, start=True, stop=True
                )
                nc.scalar.activation(
                    relu_y[:, hc * K:(hc + 1) * K], ps2,
                    mybir.ActivationFunctionType.Relu,
                )
            lg = psA.tile([K, O], f32, tag="kk")
            for hc in range(HC):
                nc.tensor.matmul(
                    lg, relu_y[:, hc * K:(hc + 1) * K], RT[:, hc, :],
                    start=(hc == 0), stop=(hc == HC - 1),
                )
            lgs = small.tile([K, O], f32, tag="lgs")
            tc.cur_priority = pri_out(b)
            nc.vector.tensor_add(lgs, lg, bb)
            nc.sync.dma_start(out=out_d[b * K:(b + 1) * K, :], in_=lgs)
            tc.cur_priority = pri_fill(b)

            if b + LOOKAHEAD < NBLK:
                tc.cur_priority = pri_whit(b + LOOKAHEAD)
                whiten(b + LOOKAHEAD)
                tc.cur_priority = pri_fill(b)
            if b + 1 < NBLK:
                # masters: W += lr * Y U^T (fp32, in place); Wb = cast(W);
                # WTb = PE-transpose of Wb. Deferred: emitted inside the NEXT
                # block's ring so engine streams order it behind that block's
                # critical head.
                def make_tail(b, yt, ut):
                    def tail():
                        tc.cur_priority = pri_fill(b)
                        for hc in range(HC):
                            for s in range(2):
                                ps = psB.tile([HP, DS], f32, tag="big")
                                nc.tensor.matmul(
                                    ps, yt[:, hc * HP:(hc + 1) * HP],
                                    ut[:, s * DS:(s + 1) * DS],
                                    start=True, stop=True,
                                )
                                wsl = W[:, hc * D + s * DS:hc * D + (s + 1) * DS]
                                nc.vector.scalar_tensor_tensor(
                                    wsl, ps, LR, wsl, op0=AT.mult, op1=AT.add
                                )
                                nc.scalar.copy(
                                    Wb[:, hc * D + s * DS:hc * D + (s + 1) * DS],
                                    wsl,
                                )
                        for dc in range(DC):
                            tp = psT.tile([DP, 2 * K], bf16, tag="tt")
                            nc.tensor.transpose(
                                tp[:, 0:HP],
                                Wb[:, 0 * D + dc * DP:0 * D + (dc + 1) * DP],
                                identb,
                            )
                            nc.tensor.transpose(
                                tp[:, HP:2 * HP],
                                Wb[:, 1 * D + dc * DP:1 * D + (dc + 1) * DP],
                                identb,
                            )
                            nc.scalar.copy(WTb[:, dc * H:(dc + 1) * H], tp)
                        if b + 2 < NBLK:
                            open_P(b + 2)
                    return tail

                pending_tail[0] = make_tail(b, yt, ut)

    _split_multiwait(nc)
    return nc


def prep_inputs(x, whiten_mean, whiten_mat, oja_W, readout_W, readout_b):
    """Host-side layout/dtype prep (no contractions)."""
    x = np.ascontiguousarray(x, dtype=np.float32)
    mu_b = np.broadcast_to(
        np.asarray(whiten_mean, dtype=np.float32)[None, :], (128, D)
    ).copy()
    P = np.asarray(whiten_mat, dtype=np.float32) - np.eye(D, dtype=np.float32)
    # pt[dp, ic, dout] = P^T[ic*112+dp, dout] = P[dout, ic*112+dp]
    pt = np.ascontiguousarray(
        P.T.reshape(DC, DP, D).transpose(1, 0, 2).astype(ml_dtypes.bfloat16)
    )
    Wf = np.asarray(oja_W, dtype=np.float32)
    w = np.ascontiguousarray(
        Wf.reshape(HC, HP, D).transpose(1, 0, 2).reshape(HP, HC * D)
    )
    wtb = np.ascontiguousarray(
        Wf.T.reshape(DC, DP, H).transpose(1, 0, 2).reshape(DP, DC * H)
    ).astype(ml_dtypes.bfloat16)
    Rf = np.asarray(readout_W, dtype=np.float32)
    rt = np.ascontiguousarray(
        Rf.T.reshape(HC, HP, O).transpose(1, 0, 2).astype(ml_dtypes.bfloat16)
    )
    b_b = np.broadcast_to(
        np.asarray(readout_b, dtype=np.float32)[None, :], (128, O)
    ).copy()
    return {
        "x": x, "mu_b": mu_b, "pt": pt, "w": w, "rt": rt, "b_b": b_b,
        "w_bf": w.astype(ml_dtypes.bfloat16), "wt_bf": wtb,
    }


_cached_nc = None


def _get_nc():
    global _cached_nc
    if _cached_nc is None:
        _cached_nc = build_nc()
    return _cached_nc


def kernel(x, whiten_mean, whiten_mat, oja_W, readout_W, readout_b, **run_kwargs):
    nc = _get_nc()
    ins = prep_inputs(x, whiten_mean, whiten_mat, oja_W, readout_W, readout_b)
    res = run_bass_kernel_spmd(
        nc, [ins] * N_CORES, core_ids=list(range(N_CORES)), **run_kwargs
    )
    out = res.results[0]["out"]
    if run_kwargs:
        kernel.last_result = res
    return out


# revision 29
# speedup vs baseline: 1.1431x; 1.0161x over previous
"""Trainium2 Bass kernel for nn_BioClassifier: whitening + sequential Oja scan + readout.

v2: restructured for critical-path latency. Same block-parallel-scan math as v1
(chunk the 2048-sample Oja scan into 16 blocks of K=128; per block a fixed-point
"ring" on K x K matrices closes the sequential recurrence exactly):
    Y = Y0 A,  U = (X - T0 A) B,   A = (I - lr*SU(C))^-1, B = (I + lr*SU(G))^-1
    C = U^T X, G = Y^T Y,  T0^T X = Syy
Key v2 changes vs v1:
  * lr folded into sxx_lr/syy_lr (bf16), iteration reordered so each ring cycle
    is 10 serial engine-hops (B-chain: z2->z2s->g->gm->b1->B; A-chain: ct->nt->
    a1->A) with r1/s prep hidden under the B-chain; iter-0 A-update is 3 vec ops
    (A1 = I + SU(s0), s0 = sxx_lr - syy_lr), no matmuls.
  * Y0 correction form: P_{n} = W^{(n-1)} X_n accumulates in an OPEN PSUM group
    during ring_{n-1} (off critical path); epilogue closes it with the rank-K
    correction  Y0_n = P_n + lr * Y * (U^T X_n), so the master-W update and the
    14 Y0 matmuls leave the serial path entirely.
  * single fp32 master W [H,D]; Wb = cast(W) on scalar; WTb = PE-transpose of Wb
    (bf16 transpose == transpose of bf16 cast, exact) - drops the WT fp32 master
    and its vector-engine update entirely.
  * XTall/Sxxall stored bf16 (Sxx pre-scaled by lr at whiten time).
  * engine rebalance: ring bounces on DVE, z2s/copies on scalar, xc-sub and
    the iter-0 mask-mul on gpsimd (which cannot touch PSUM); whitening +
    masters + P-opens emitted in priority bands (and the master-update tail
    emitted mid-ring of the NEXT block) so the in-order engine streams place
    them behind each block's critical chain.
  * RING_ITERS=4 (validated offline: rel err 1.49e-2 vs the 2e-2 gate;
    RING_ITERS=5 gives 8.7e-3 at ~+60us).
All 8 cores run the identical program (the scan is inherently sequential;
core 0's output is returned).
"""

import os
import sys
from contextlib import ExitStack

sys.path.insert(0, "/opt/trn_rl_repo")

import numpy as np
import ml_dtypes

import concourse.bass as bass
import concourse.mybir as mybir
from concourse.tile import TileContext
from concourse.masks import make_identity
from concourse.bass_utils import run_bass_kernel_spmd
from concourse.vector_clock import ScopedClock

LR = 1e-3
B, D, H, O = 2048, 784, 256, 10
K = 128
NBLK = B // K
DP, DC = 112, 7          # D = 784 = 7 * 112
HP, HC = 128, 2          # H = 256 = 2 * 128
DS = D // 2              # 392: matmul free-dim split for D-wide outputs

RING_ITERS = int(os.environ.get("RING_ITERS", "4"))
LOOKAHEAD = 4
N_CORES = 1

f32 = mybir.dt.float32
bf16 = mybir.dt.bfloat16
AT = mybir.AluOpType


def _install_ntff_hook():
    """The agent image's `antenv` lacks `axon_hooks`, so trace=True degrades.
    Synthesize the module and register the ctypes NTFF hook from trn_boot."""
    import types
    import antenv

    if getattr(antenv, "axon_hooks", None) is not None:
        return
    mod = types.ModuleType("antenv.axon_hooks")
    _hook_box = [None]
    mod.set_axon_ntff_profile_hook = lambda h: _hook_box.__setitem__(0, h)
    mod.get_axon_ntff_profile_hook = lambda: _hook_box[0]
    sys.modules["antenv.axon_hooks"] = mod
    antenv.axon_hooks = mod
    try:
        sys.path.insert(0, "/root/.axon_site")
        from trn_agent_boot.trn_boot import _ntff_profile_via_ctypes

        hook = _ntff_profile_via_ctypes("/opt/axon/libaxon_pjrt.so")
        if hook is not None:
            mod.set_axon_ntff_profile_hook(hook)
    except Exception:
        pass


try:
    _install_ntff_hook()
except Exception:
    pass

_drain_patched = False


def _patch_drain():
    """This walrus build only supports one sync-wait per CTRL instruction;
    split the Tile kernel-tail drain into one drain per semaphore wait."""
    global _drain_patched
    if _drain_patched:
        return

    def patched(self, tick_clock, wait_clock):
        drain_inst = self.nc.sync.drain()
        wait_clock.add_sem_waits(
            drain_inst.ins, ScopedClock({None: tick_clock.global_clock})
        )
        mi = drain_inst.ins
        si = mi.sync_info
        if si is not None and len(si.on_wait) > 1:
            waits = list(si.on_wait)
            mi.sync_info = mybir.SyncInfo(
                on_wait=[waits[0]], on_update=list(si.on_update)
            )
            for w in waits[1:]:
                d2 = self.nc.sync.drain()
                d2.ins.sync_info = mybir.SyncInfo(on_wait=[w], on_update=[])
        self.nc.all_engine_barrier()
        assert self.sems is not None
        popped = self.nc._tile_sem_poison_stack.pop()
        assert popped is self._sem_poison
        self.nc.clear_and_free_semaphores(list(self.sems.allocated().values()))
        self.nc.all_engine_barrier()

    TileContext._drain_and_barrier = patched
    _drain_patched = True


def _split_multiwait(nc, limit=1):
    """This walrus build supports only `limit` sync-waits per instruction.
    Hoist extra waits onto NoOps inserted just before, in the same engine
    stream (engines are in-order, so earlier waits are strictly safe)."""
    n_split = 0
    for f in nc.m.functions:
        for bb in f.blocks:
            insts = list(bb.instructions)
            if not any(
                i.sync_info is not None and len(i.sync_info.on_wait) > limit
                for i in insts
            ):
                continue
            new = []
            for inst in insts:
                si = inst.sync_info
                if si is not None and len(si.on_wait) > limit:
                    waits = list(si.on_wait)
                    for j, w in enumerate(waits[: len(waits) - limit]):
                        nop = mybir.InstNoOp(
                            name=f"{inst.name}-hw{j}", engine=inst.engine,
                            ins=[], outs=[],
                        )
                        nop.sync_info = mybir.SyncInfo(on_wait=[w], on_update=[])
                        new.append(nop)
                        n_split += 1
                    inst.sync_info = mybir.SyncInfo(
                        on_wait=waits[len(waits) - limit:],
                        on_update=list(si.on_update),
                    )
                new.append(inst)
            bb.instructions = new
    return n_split


def build_nc(ring_iters=RING_ITERS):
    _patch_drain()
    nc = bass.Bass()
    x_d = nc.dram_tensor("x", [B, D], f32, kind="ExternalInput")
    mu_d = nc.dram_tensor("mu_b", [128, D], f32, kind="ExternalInput")
    pt_d = nc.dram_tensor("pt", [DP, DC, D], bf16, kind="ExternalInput")
    w_d = nc.dram_tensor("w", [HP, HC * D], f32, kind="ExternalInput")
    wb_d = nc.dram_tensor("w_bf", [HP, HC * D], bf16, kind="ExternalInput")
    wtb_d = nc.dram_tensor("wt_bf", [DP, DC * H], bf16, kind="ExternalInput")
    rt_d = nc.dram_tensor("rt", [HP, HC, O], bf16, kind="ExternalInput")
    bb_d = nc.dram_tensor("b_b", [128, O], f32, kind="ExternalInput")
    out_d = nc.dram_tensor("out", [B, O], f32, kind="ExternalOutput")
    def pri_crit(b):
        # critical path of block b
        return 1000 + b * 1000

    def pri_fill(b):
        # fill work of block b: must rank BELOW crit of b+1 (it runs during
        # ring_{b+1}) but above crit of b+2
        return 1000 + (b + 1) * 1000 + 500

    def pri_whit(b):
        # whiten(b) must complete before epilogue of b-1: rank just below
        # crit(b-1), above fill bands of earlier blocks
        return 1000 + (b - 1) * 1000 + 400

    def pri_out(b):
        return 20_000_000 + b * 1000

    with TileContext(nc) as tc, ExitStack() as ctx:
        persist = ctx.enter_context(tc.tile_pool(name="persist", bufs=1))
        xpool = ctx.enter_context(tc.tile_pool(name="xpool", bufs=5))
        small = ctx.enter_context(tc.tile_pool(name="small", bufs=2))
        psA = ctx.enter_context(tc.tile_pool(name="psA", bufs=3, space="PSUM"))
        psB = ctx.enter_context(tc.tile_pool(name="psB", bufs=3, space="PSUM"))
        psT = ctx.enter_context(tc.tile_pool(name="psT", bufs=2, space="PSUM"))

        ident = persist.tile([128, 128], f32, tag="ident")
        make_identity(nc, ident)
        identb = persist.tile([128, 128], bf16, tag="identb")
        nc.vector.tensor_copy(identb, ident)
        # 0/1 masks (lr is folded into sxx_lr / syy_lr)
        maskSL = persist.tile([K, K], f32, tag="maskSL")
        nc.gpsimd.memset(maskSL, 1.0)
        nc.gpsimd.affine_select(
            out=maskSL, in_=maskSL, compare_op=AT.is_gt, fill=0.0,
            base=0, pattern=[[-1, K]], channel_multiplier=1,
        )
        maskSU = persist.tile([K, K], f32, tag="maskSU")
        nc.gpsimd.memset(maskSU, 1.0)
        nc.vector.tensor_sub(maskSU, maskSU, ident)
        nc.vector.tensor_sub(maskSU, maskSU, maskSL)

        mu_t = persist.tile([128, D], f32, tag="mu")
        nc.sync.dma_start(out=mu_t, in_=mu_d[:, :])
        # pt per-chunk so whiten-0's accumulation can start as chunks land
        pt_t = persist.tile([DP, DC, D], bf16, tag="pt")
        for ic in range(DC):
            nc.sync.dma_start(out=pt_t[:, ic, :], in_=pt_d[:, ic, :])
        WTb = persist.tile([DP, DC * H], bf16, tag="WTb")
        nc.sync.dma_start(out=WTb, in_=wtb_d[:, :])
        Wb = persist.tile([HP, HC * D], bf16, tag="Wb")
        nc.sync.dma_start(out=Wb, in_=wb_d[:, :])
        RT = persist.tile([HP, HC, O], bf16, tag="RT")
        nc.sync.dma_start(out=RT, in_=rt_d[:, :, :])
        bb = persist.tile([128, O], f32, tag="bb")
        nc.sync.dma_start(out=bb, in_=bb_d[:, :])
        # W fp32 master is first needed only at block-0's fill tail
        W = persist.tile([HP, HC * D], f32, tag="W")
        nc.sync.dma_start(out=W, in_=w_d[:, :])

        Xall = persist.tile([DP, NBLK, DC, K], bf16, tag="Xall")
        XTall = persist.tile([K, NBLK, D], bf16, tag="XTall")
        Sxxall = persist.tile([K, NBLK, K], bf16, tag="Sxxall")

        # ---------------- whitening ----------------
        def whiten(bi):
            xt = xpool.tile([128, D], f32, tag="xraw")
            nc.sync.dma_start(out=xt, in_=x_d[bi * K:(bi + 1) * K, :])
            xc = xpool.tile([128, D], f32, tag="xc")
            eng = nc.vector if bi == 0 else nc.gpsimd
            eng.tensor_sub(xc, xt, mu_t)
            # transpose xc as f32 (2cyc/col) and cast in the PSUM->SBUF copy:
            # avoids a separate bf16 cast of xc on the vector engine
            xct = xpool.tile([DP, DC * K], bf16, tag="xct")
            for ic in range(DC):
                tpf = psT.tile([DP, K], f32, tag="tt")
                nc.tensor.transpose(
                    tpf, xc[:, ic * DP:(ic + 1) * DP], ident
                )
                nc.scalar.copy(xct[:, ic * K:(ic + 1) * K], tpf)
            XTb = XTall[:, bi, :]
            for s in range(2):
                ps = psB.tile([K, DS], f32, tag="big")
                for ic in range(DC):
                    nc.tensor.matmul(
                        ps, xct[:, ic * K:(ic + 1) * K],
                        pt_t[:, ic, s * DS:(s + 1) * DS],
                        start=(ic == 0), stop=(ic == DC - 1),
                    )
                nc.vector.tensor_add(
                    XTb[:, s * DS:(s + 1) * DS], ps, xc[:, s * DS:(s + 1) * DS]
                )
            xa = Xall[:, bi, :, :]
            for p in range(3):
                tp = psT.tile([DP, 2 * K], bf16, tag="tt")
                nc.tensor.transpose(
                    tp[:, 0:K], XTb[:, (2 * p) * DP:(2 * p + 1) * DP], identb
                )
                nc.tensor.transpose(
                    tp[:, K:2 * K], XTb[:, (2 * p + 1) * DP:(2 * p + 2) * DP],
                    identb,
                )
                nc.scalar.copy(xa[:, 2 * p:2 * p + 2, :], tp)
            tp = psT.tile([DP, K], bf16, tag="tt")
            nc.tensor.transpose(tp, XTb[:, 6 * DP:7 * DP], identb)
            nc.scalar.copy(xa[:, 6, :], tp)
            ps = psA.tile([K, K], f32, tag="kk")
            for ic in range(DC):
                nc.tensor.matmul(
                    ps, xa[:, ic, :], xa[:, ic, :],
                    start=(ic == 0), stop=(ic == DC - 1),
                )
            nc.scalar.mul(Sxxall[:, bi, :], ps, LR)

        tc.cur_priority = 0
        whiten(0)

        Psb = {}  # block -> SBUF f32 tile [HP, HC*K] holding W^(stale) X_block

        def open_P(nb):
            """Accumulate P_nb = W^(current) X_nb into an SBUF f32 tile."""
            pt_sb = small.tile([HP, HC * K], f32, tag="Psb")
            for hc in range(HC):
                ps = psB.tile([HP, K], f32, tag="big")
                for ic in range(DC):
                    nc.tensor.matmul(
                        ps,
                        WTb[:, ic * H + hc * HP:ic * H + (hc + 1) * HP],
                        Xall[:, nb, ic, :],
                        start=(ic == 0), stop=(ic == DC - 1),
                    )
                nc.scalar.copy(pt_sb[:, hc * K:(hc + 1) * K], ps)
            Psb[nb] = pt_sb

        y0q_next = None
        pending_tail = [None]

        for b in range(NBLK):
            # ================= HEAD (critical) =================
            tc.cur_priority = pri_crit(b)
            if b == 0:
                y0q = small.tile([HP, HC * K], bf16, tag="y0")
                for hc in range(HC):
                    ps = psB.tile([HP, K], f32, tag="big")
                    for ic in range(DC):
                        nc.tensor.matmul(
                            ps,
                            WTb[:, ic * H + hc * HP:ic * H + (hc + 1) * HP],
                            Xall[:, 0, ic, :],
                            start=(ic == 0), stop=(ic == DC - 1),
                        )
                    nc.vector.tensor_copy(y0q[:, hc * K:(hc + 1) * K], ps)
            else:
                y0q = y0q_next
            ps_syy = psA.tile([K, K], f32, tag="kk")
            for hc in range(HC):
                nc.tensor.matmul(
                    ps_syy, y0q[:, hc * K:(hc + 1) * K],
                    y0q[:, hc * K:(hc + 1) * K],
                    start=(hc == 0), stop=(hc == HC - 1),
                )
            syy_lr = small.tile([K, K], bf16, tag="syl")
            syy_ng = small.tile([K, K], bf16, tag="syn")

            # ---- head fill (off critical path) ----
            tc.cur_priority = pri_fill(b)
            y0t = small.tile([K, H], bf16, tag="y0t")
            for hc in range(HC):
                tp = psT.tile([128, K], bf16, tag="tt")
                nc.tensor.transpose(tp, y0q[:, hc * K:(hc + 1) * K], identb)
                nc.scalar.copy(y0t[:, hc * HP:(hc + 1) * HP], tp)
            if b == 0:
                # emitted AFTER block-0's critical head so the engine streams
                # don't head-of-line-block it behind whitening
                tc.cur_priority = pri_whit(1)
                whiten(1)
                tc.cur_priority = pri_whit(2)
                whiten(2)
                tc.cur_priority = 1450
                open_P(1)
                tc.cur_priority = pri_fill(b)
            # ================= RING (critical) =================
            tc.cur_priority = pri_crit(b)
            sxx = Sxxall[:, b, :]
            s_sb = small.tile([K, K], bf16, tag="s")
            nc.vector.scalar_tensor_tensor(
                s_sb, ps_syy, -LR, sxx, op0=AT.mult, op1=AT.add
            )
            tA0 = small.tile([K, K], bf16, tag="ta")
            nc.gpsimd.tensor_mul(tA0, s_sb, maskSU)
            A = small.tile([K, K], bf16, tag="A")
            nc.vector.tensor_add(A, tA0, ident)
            nc.scalar.mul(syy_lr, ps_syy, LR)
            nc.vector.tensor_scalar_mul(syy_ng, ps_syy, -LR)
            Bm = small.tile([K, K], bf16, tag="B")
            Bprev = identb
            for m in range(1, ring_iters):
                if m == 2:
                    # emit the previous block's master-update chain here so the
                    # scheduler places it after this block's head/early ring in
                    # every engine stream; t0t must follow it (reads updated Wb)
                    if pending_tail[0] is not None:
                        pending_tail[0]()
                        pending_tail[0] = None
                    tc.cur_priority = pri_fill(b - 1) if b else pri_fill(0)
                    if b + 1 < NBLK:
                        t0t = small.tile([K, D], bf16, tag="t0t")
                        for s in range(2):
                            ps = psB.tile([K, DS], f32, tag="big")
                            for hc in range(HC):
                                nc.tensor.matmul(
                                    ps, y0q[:, hc * K:(hc + 1) * K],
                                    Wb[:, hc * D + s * DS:hc * D + (s + 1) * DS],
                                    start=(hc == 0), stop=(hc == HC - 1),
                                )
                            nc.vector.tensor_copy(
                                t0t[:, s * DS:(s + 1) * DS], ps
                            )
                    tc.cur_priority = pri_crit(b)
                z2 = psA.tile([K, K], f32, tag="kk")
                nc.tensor.matmul(z2, syy_ng, A, start=True, stop=True)
                z2s = small.tile([K, K], bf16, tag="z2")
                nc.scalar.copy(z2s, z2)
                r1 = psA.tile([K, K], f32, tag="kk")
                nc.tensor.matmul(r1, A, syy_lr, start=True, stop=True)
                s_sb = small.tile([K, K], bf16, tag="s")
                nc.vector.tensor_sub(s_sb, sxx, r1)
                g = psA.tile([K, K], f32, tag="kk")
                nc.tensor.matmul(g, A, z2s, start=True, stop=True)
                gm = small.tile([K, K], bf16, tag="gm")
                nc.vector.tensor_mul(gm, g, maskSL)
                b1 = psA.tile([K, K], f32, tag="kk")
                nc.tensor.matmul(b1, gm, Bprev, start=True, stop=True)
                nc.vector.tensor_add(Bm, b1, ident)
                Bprev = Bm
                ct = psA.tile([K, K], f32, tag="kk")
                nc.tensor.matmul(ct, s_sb, Bm, start=True, stop=True)
                nt = small.tile([K, K], bf16, tag="nt")
                nc.vector.tensor_mul(nt, ct, maskSL)
                a1 = psA.tile([K, K], f32, tag="kk")
                nc.tensor.matmul(a1, nt, A, start=True, stop=True)
                nc.vector.tensor_add(A, a1, ident)
            # final B-update (B_R from A_R), epilogue A-work interleaved to
            # fill PE gaps while z2s/gm bounce on scalar/vector.
            # The last block needs only its logits: skip B_R/yt/q/ut entirely.
            if b + 1 < NBLK:
                # q = XT - A^T t0t FIRST (only needs A): its halves feed the
                # qT transposes and the qX accumulation, which then overlap
                # the final B-chain; C' = B_R^T qX is the only matmul left on
                # the B_R -> clr path.
                z2 = psA.tile([K, K], f32, tag="kk")
                nc.tensor.matmul(z2, syy_ng, A, start=True, stop=True)
                z2s = small.tile([K, K], bf16, tag="z2")
                nc.scalar.copy(z2s, z2)
                ps_q0 = psB.tile([K, DS], f32, tag="big")
                nc.tensor.matmul(ps_q0, A, t0t[:, 0:DS], start=True, stop=True)
                q = small.tile([K, D], bf16, tag="q")
                nc.vector.tensor_sub(q[:, 0:DS], XTall[:, b, 0:DS], ps_q0)
                g = psA.tile([K, K], f32, tag="kk")
                nc.tensor.matmul(g, A, z2s, start=True, stop=True)
                gm = small.tile([K, K], bf16, tag="gm")
                nc.vector.tensor_mul(gm, g, maskSL)
                ps_q1 = psB.tile([K, DS], f32, tag="big")
                nc.tensor.matmul(ps_q1, A, t0t[:, DS:D], start=True, stop=True)
                nc.vector.tensor_sub(q[:, DS:D], XTall[:, b, DS:D], ps_q1)
                # transpose q's first half (chunks 0-2, cols < DS) and start
                # the qX accumulation on them while the second q-half is still
                # in flight; finish with chunks 3-6 after sub1
                qT = small.tile([DP, DC * K], bf16, tag="U")
                tp = psA.tile([DP, 2 * K], bf16, tag="kk")
                nc.tensor.transpose(tp[:, 0:K], q[:, 0:DP], identb)
                nc.tensor.transpose(tp[:, K:2 * K], q[:, DP:2 * DP], identb)
                nc.scalar.copy(qT[:, 0:2 * K], tp)
                tp = psA.tile([DP, K], bf16, tag="kk")
                nc.tensor.transpose(tp, q[:, 2 * DP:3 * DP], identb)
                nc.scalar.copy(qT[:, 2 * K:3 * K], tp)
                b1 = psA.tile([K, K], f32, tag="kk")
                nc.tensor.matmul(b1, gm, Bprev, start=True, stop=True)
                nc.vector.tensor_add(Bm, b1, ident)
                psqx = psA.tile([K, K], f32, tag="kk")
                for ic in range(3):
                    nc.tensor.matmul(
                        psqx, qT[:, ic * K:(ic + 1) * K], Xall[:, b + 1, ic, :],
                        start=(ic == 0), stop=False,
                    )
                ps_yt = psB.tile([K, H], f32, tag="big")
                nc.tensor.matmul(ps_yt, A, y0t, start=True, stop=True)
                yt = small.tile([K, H], bf16, tag="yt")
                nc.vector.tensor_copy(yt, ps_yt)
                for p in (3, 5):
                    tp = psA.tile([DP, 2 * K], bf16, tag="kk")
                    nc.tensor.transpose(
                        tp[:, 0:K], q[:, p * DP:(p + 1) * DP], identb
                    )
                    nc.tensor.transpose(
                        tp[:, K:2 * K], q[:, (p + 1) * DP:(p + 2) * DP],
                        identb,
                    )
                    nc.scalar.copy(qT[:, p * K:(p + 2) * K], tp)
                for ic in range(3, DC):
                    nc.tensor.matmul(
                        psqx, qT[:, ic * K:(ic + 1) * K], Xall[:, b + 1, ic, :],
                        start=False, stop=(ic == DC - 1),
                    )
                qxs = small.tile([K, K], bf16, tag="qx")
                nc.scalar.copy(qxs, psqx)
                psc = psA.tile([K, K], f32, tag="kk")
                nc.tensor.matmul(psc, Bm, qxs, start=True, stop=True)
                clr = small.tile([K, K], bf16, tag="clr")
                nc.scalar.mul(clr, psc, LR)
                y0q_next = small.tile([HP, HC * K], bf16, tag="y0")
                for hc in range(HC):
                    cps = psA.tile([K, K], f32, tag="kk")
                    nc.tensor.matmul(
                        cps, yt[:, hc * HP:(hc + 1) * HP], clr,
                        start=True, stop=True,
                    )
                    nc.vector.tensor_add(
                        y0q_next[:, hc * K:(hc + 1) * K], cps,
                        Psb[b + 1][:, hc * K:(hc + 1) * K],
                    )
                ut = small.tile([K, D], bf16, tag="ut")
                for s in range(2):
                    ps = psB.tile([K, DS], f32, tag="big")
                    nc.tensor.matmul(
                        ps, Bm, q[:, s * DS:(s + 1) * DS], start=True, stop=True
                    )
                    nc.scalar.copy(ut[:, s * DS:(s + 1) * DS], ps)

            # ================= FILL TAIL =================
            tc.cur_priority = pri_fill(b)
            relu_y = small.tile([HP, HC * K], bf16, tag="ry")
            for hc in range(HC):
                ps2 = psA.tile([K, K], f32, tag="kk")
                nc.tensor.matmul(
                    ps2, y0t[:, hc * HP:(hc + 1) * HP], A, start=True, stop=True
                )
                nc.scalar.activation(
                    relu_y[:, hc * K:(hc + 1) * K], ps2,
                    mybir.ActivationFunctionType.Relu,
                )
            lg = psA.tile([K, O], f32, tag="kk")
            for hc in range(HC):
                nc.tensor.matmul(
                    lg, relu_y[:, hc * K:(hc + 1) * K], RT[:, hc, :],
                    start=(hc == 0), stop=(hc == HC - 1),
                )
            lgs = small.tile([K, O], f32, tag="lgs")
            tc.cur_priority = pri_out(b)
            nc.vector.tensor_add(lgs, lg, bb)
            nc.sync.dma_start(out=out_d[b * K:(b + 1) * K, :], in_=lgs)
            tc.cur_priority = pri_fill(b)

            if b == 0:
                tc.cur_priority = pri_whit(3)
                whiten(3)
            if b + LOOKAHEAD < NBLK:
                tc.cur_priority = pri_whit(b + LOOKAHEAD)
                whiten(b + LOOKAHEAD)
                tc.cur_priority = pri_fill(b)
            if b + 1 < NBLK:
                # masters: W += lr * Y U^T (fp32, in place); Wb = cast(W);
                # WTb = PE-transpose of Wb. Deferred: emitted inside the NEXT
                # block's ring so engine streams order it behind that block's
                # critical head.
                def make_tail(b, yt, ut):
                    def tail():
                        tc.cur_priority = pri_fill(b)
                        for hc in range(HC):
                            for s in range(2):
                                ps = psB.tile([HP, DS], f32, tag="big")
                                nc.tensor.matmul(
                                    ps, yt[:, hc * HP:(hc + 1) * HP],
                                    ut[:, s * DS:(s + 1) * DS],
                                    start=True, stop=True,
                                )
                                wsl = W[:, hc * D + s * DS:hc * D + (s + 1) * DS]
                                nc.vector.scalar_tensor_tensor(
                                    wsl, ps, LR, wsl, op0=AT.mult, op1=AT.add
                                )
                                nc.scalar.copy(
                                    Wb[:, hc * D + s * DS:hc * D + (s + 1) * DS],
                                    wsl,
                                )
                        for dc in range(DC):
                            tp = psT.tile([DP, 2 * K], bf16, tag="tt")
                            nc.tensor.transpose(
                                tp[:, 0:HP],
                                Wb[:, 0 * D + dc * DP:0 * D + (dc + 1) * DP],
                                identb,
                            )
                            nc.tensor.transpose(
                                tp[:, HP:2 * HP],
                                Wb[:, 1 * D + dc * DP:1 * D + (dc + 1) * DP],
                                identb,
                            )
                            nc.scalar.copy(WTb[:, dc * H:(dc + 1) * H], tp)
                        if b + 2 < NBLK:
                            open_P(b + 2)
                    return tail

                pending_tail[0] = make_tail(b, yt, ut)

    _split_multiwait(nc)
    return nc


def prep_inputs(x, whiten_mean, whiten_mat, oja_W, readout_W, readout_b):
    """Host-side layout/dtype prep (no contractions)."""
    x = np.ascontiguousarray(x, dtype=np.float32)
    mu_b = np.broadcast_to(
        np.asarray(whiten_mean, dtype=np.float32)[None, :], (128, D)
    ).copy()
    P = np.asarray(whiten_mat, dtype=np.float32) - np.eye(D, dtype=np.float32)
    # pt[dp, ic, dout] = P^T[ic*112+dp, dout] = P[dout, ic*112+dp]
    pt = np.ascontiguousarray(
        P.T.reshape(DC, DP, D).transpose(1, 0, 2).astype(ml_dtypes.bfloat16)
    )
    Wf = np.asarray(oja_W, dtype=np.float32)
    w = np.ascontiguousarray(
        Wf.reshape(HC, HP, D).transpose(1, 0, 2).reshape(HP, HC * D)
    )
    wtb = np.ascontiguousarray(
        Wf.T.reshape(DC, DP, H).transpose(1, 0, 2).reshape(DP, DC * H)
    ).astype(ml_dtypes.bfloat16)
    Rf = np.asarray(readout_W, dtype=np.float32)
    rt = np.ascontiguousarray(
        Rf.T.reshape(HC, HP, O).transpose(1, 0, 2).astype(ml_dtypes.bfloat16)
    )
    b_b = np.broadcast_to(
        np.asarray(readout_b, dtype=np.float32)[None, :], (128, O)
    ).copy()
    return {
        "x": x, "mu_b": mu_b, "pt": pt, "w": w, "rt": rt, "b_b": b_b,
        "w_bf": w.astype(ml_dtypes.bfloat16), "wt_bf": wtb,
    }


_cached_nc = None


def _get_nc():
    global _cached_nc
    if _cached_nc is None:
        _cached_nc = build_nc()
    return _cached_nc


def kernel(x, whiten_mean, whiten_mat, oja_W, readout_W, readout_b, **run_kwargs):
    nc = _get_nc()
    ins = prep_inputs(x, whiten_mean, whiten_mat, oja_W, readout_W, readout_b)
    res = run_bass_kernel_spmd(
        nc, [ins] * N_CORES, core_ids=list(range(N_CORES)), **run_kwargs
    )
    out = res.results[0]["out"]
    if run_kwargs:
        kernel.last_result = res
    return out
